# revision 1
# baseline (speedup 1.0000x reference)
"""Trainium2 Bass kernel for nn_AlternateAttention (3-block transformer:
global attention, lower-band attention, upper-band attention, each with MLP).

Sharding: 8 cores = 2 batches x 4 sequence chunks of 1024 tokens, each core
processing an extended window of 1152 tokens (64-token halo each side) so the
banded blocks need no inter-core communication. Block 0 (global attention)
needs full-sequence K/V; each core recomputes them from the replicated raw
input (LN1+KV projection over the full 4096 tokens of its batch).

Layout: activations live TRANSPOSED in SBUF ([feature, token]) so projections
are natural matmuls (lhsT = W^T chunks). LN stats are computed with ones-
matmul partition reductions; per-token mean/rstd are broadcast with K=1
matmuls. Softmax is computed without max subtraction (scores are O(1) by
construction), in the scores^T layout, with row-of-ones appended to V to get
the normalizer for free.
"""
import contextlib
import numpy as np
import ml_dtypes

# ---- problem constants (hardcoded per contract) ----
B, S, E, H, W_BAND, FF = 2, 4096, 512, 8, 16, 2048
HD = E // H                      # 64
N_CORES = 8
CHUNKS = 4                       # sequence chunks per batch
OWN = S // CHUNKS                # 1024
HALO = 64
T = OWN + 2 * HALO               # 1152 = 9*128
P = 128
NT = 384                         # token tile (3 per T)
NQT = T // NT                    # 3
NKC = S // P                     # 32 k-chunks for global attention
NTC = T // P                     # 9 token chunks of ext window
EC = E // P                      # 4 feature strips
FFC = FF // P                    # 16

EXT_STARTS = [max(0, min(OWN * c - HALO, S - T)) for c in range(CHUNKS)]
OWN_OFF = [OWN * c - EXT_STARTS[c] for c in range(CHUNKS)]

_EXEC_CACHE = {}
_PHASE_MARKS = []
_PHASE_OF = {}

bf16 = ml_dtypes.bfloat16


# ------------------------------------------------------------------
# device program
# ------------------------------------------------------------------
def _patch_act_tables():
    # The bacc table-load placement maps exp -> 'exp_and_others' and
    # ln -> 'natural_log', thrashing the ACT table RAM (~1.3us per switch,
    # dozens of switches). Restrict the choice to the two sets that cover
    # everything this kernel uses so exp/ln never evict each other.
    import concourse.hw_specs as hw_specs
    import concourse.bacc as bacc_mod
    import concourse.bass_interp as bass_interp
    if getattr(hw_specs, "_aa_patched", False):
        return
    orig = hw_specs.get_activation_tables
    keep = {"natural_log_exp_and_others", "gelu_apprx_tanh_and_others"}

    def _gat(arch):
        tabs = orig(arch)
        return {k: (v if k in keep else set()) for k, v in tabs.items()}

    hw_specs.get_activation_tables = _gat
    bacc_mod.get_activation_tables = _gat
    bass_interp.get_activation_tables = _gat
    hw_specs._aa_patched = True


def _build_nc(use_bias: bool, repeat: int = 1):
    import concourse.bacc as bacc
    import concourse.mybir as mybir
    import concourse.tile as tile

    _patch_act_tables()

    f32 = mybir.dt.float32
    b16 = mybir.dt.bfloat16
    AF = mybir.ActivationFunctionType
    OP = mybir.AluOpType

    nc = bacc.Bacc("TRN2", target_bir_lowering=False, debug=False,
                   num_devices=N_CORES)
    _PHASE_MARKS.clear()

    def mark(label):
        n = sum(len(b.instructions) for b in nc.main_func.blocks)
        _PHASE_MARKS.append((label, n))

    # ---- dram tensors ----
    xT = nc.dram_tensor("xT", [E, T], f32, kind="ExternalInput")
    xfT = nc.dram_tensor("xfT", [E, S], b16, kind="ExternalInput")
    wq, wo, w1, w2, bias_d = [], [], [], [], []
    for i in range(3):
        wq.append(nc.dram_tensor(f"qkvT{i}", [EC, P, 3 * E], b16, kind="ExternalInput"))
        wo.append(nc.dram_tensor(f"outT{i}", [EC, P, E], b16, kind="ExternalInput"))
        w1.append(nc.dram_tensor(f"fc1T{i}", [EC, P, FF], b16, kind="ExternalInput"))
        w2.append(nc.dram_tensor(f"fc2T{i}", [FFC, P, E], b16, kind="ExternalInput"))
        if use_bias:
            # packed per-feature biases for transposed-layout outputs:
            # [q(4xP) | k(4xP) | out(4xP) | fc1(16xP) | fc2(4xP)] -> [P, 32]
            bias_d.append(nc.dram_tensor(f"bias{i}", [P, 32], f32, kind="ExternalInput"))
            bias_d.append(nc.dram_tensor(f"vbias{i}", [1, E], b16, kind="ExternalInput"))
    # 0/1 bf16 stripe-mask tiles for banded attention (384-query tiles,
    # 4 relative k-chunks each)
    mask_d = {}
    for bnd in ("lo", "up"):
        for c in range(4):
            nm = f"m_{bnd}{c}"
            mask_d[nm] = nc.dram_tensor(nm, [P, NT], b16, kind="ExternalInput")
    yT = nc.dram_tensor("yT", [E, T], f32, kind="ExternalOutput")

    from concourse.tile import add_dep_helper as _adh

    with tile.TileContext(nc) as tc, contextlib.ExitStack() as ctx:
        pool = lambda name, bufs, **kw: ctx.enter_context(
            tc.tile_pool(name=name, bufs=bufs, **kw))

        # Order ACT instructions across table-set boundaries (exp/ln vs gelu)
        # so the activation-table RAM isn't thrashed (~1.3us per reload).
        # Within a set no ordering is imposed.
        _act_sets = {AF.Gelu_apprx_tanh: 1}
        _prev_set = [None]
        _prev_insts = [[]]
        _cur_insts = [[]]

        def act(out, in_, func, **kw):
            inst = nc.scalar.activation(out, in_, func, **kw)
            if func == AF.Copy:
                return inst
            s = _act_sets.get(func, 0)
            if _prev_set[0] is None:
                _prev_set[0] = s
            if s != _prev_set[0]:
                _prev_insts[0] = _cur_insts[0][-64:]
                _cur_insts[0] = []
                _prev_set[0] = s
            for p in _prev_insts[0]:
                _adh(inst.ins, p.ins, sync=True, reason="act-table batch order")
            _cur_insts[0].append(inst)
            return inst

        # ---- pools live for the whole kernel ----
        p_x = pool("x", 2)           # residual strips f32, tags x0..x3
        p_h = pool("h", 1)           # LN output strips bf16
        p_xb = pool("xb", 2)         # per-nt bf16 casts for stats
        p_sq = pool("sq", 4)         # per-nt squares bf16
        p_qt = pool("qt", 1)         # QT [P, EC, T] bf16
        p_ot = pool("ot", 1)         # attention out strips bf16
        p_wqkv = pool("wqkv", 1)     # qkv weights [P, EC, 3E]
        p_wout = pool("wout", 1)
        p_probs = pool("probs", 4 if not use_bias else 3)   # exp outputs bf16
        p_tmp = pool("tmp", 2)       # f32 [P, NT] temporaries
        p_sm = pool("sm", 2)         # small [1, n] stat vectors
        p_c = pool("const", 1)       # ones, masks, biases
        # PSUM: exactly 8 banks, hand-assigned tags
        p_ps = pool("ps", 1, space="PSUM")
        # sc0-sc2: attention scores (+ LN stat sums share sc0/sc1, rstd
        # broadcast shares sc2 - temporally disjoint from scores)
        # po0/po1: attention output accumulators
        # mm0/mm1: gemm accumulators
        # bc0: mean broadcast / softmax 1/l broadcast

        ones = p_c.tile([P, P], b16)
        nc.vector.memset(ones[:], 1.0)
        eps1 = p_c.tile([1, 1], f32)
        nc.vector.memset(eps1[:], 1e-5)
        masks = {}
        for nm, d in mask_d.items():
            mt = p_c.tile([P, NT], b16, tag=f"mask_{nm}", name=f"mask_{nm}")
            nc.sync.dma_start(mt[:], d[:])
            masks[nm] = mt
        bias_t, vbias_t = [], []
        if use_bias:
            for i in range(3):
                bt = p_c.tile([P, 32], f32, tag=f"bias{i}", name=f"bias{i}")
                nc.sync.dma_start(bt[:], bias_d[2 * i][:])
                bias_t.append(bt)
                vt = p_c.tile([1, E], b16, tag=f"vbias{i}", name=f"vbias{i}")
                nc.sync.dma_start(vt[:], bias_d[2 * i + 1][:])
                vbias_t.append(vt)

        def bslice(i, group, oc):
            base = {"qkv": 0, "out": 8, "fc1": 12, "fc2": 28}[group]
            return bias_t[i][:, base + oc:base + oc + 1]

        def add_vbias(i, ps):
            # V projection output is in normal layout [token, feat]: bias
            # varies along the free dim -> broadcast with a K=1 matmul.
            vb = p_ps.tile([P, 512], f32, tag="bc0", name="vb")
            nc.tensor.matmul(vb[:], ones[0:1, :], vbias_t[i][:],
                             start=True, stop=True)
            vbs = p_tmp.tile([P, 512], f32, tag="vbs", name="vbs")
            nc.vector.tensor_copy(vbs[:], vb[:])
            nc.vector.tensor_add(ps[:], ps[:], vbs[:])

        def psum_to_sbuf(dst_ap, ps_ap, i, group, oc, eng="dve"):
            """copy matmul accumulator to sbuf, adding bias if enabled"""
            if eng == "act" and not use_bias:
                act(dst_ap, ps_ap, AF.Copy)
            elif use_bias:
                nc.vector.tensor_scalar(dst_ap, ps_ap, bslice(i, group, oc),
                                        None, OP.add)
            else:
                nc.vector.tensor_copy(dst_ap, ps_ap)

        def _emit_once():
            # load residual strips
            xs = []
            for s in range(EC):
                t = p_x.tile([P, T], f32, tag=f"x{s}", name=f"xin{s}")
                nc.sync.dma_start(t[:], xT[P * s:P * (s + 1), :])
                xs.append(t)

            # ---------- layernorm ----------
            def layernorm(x_strips, Tn, in_f32, htag):
                """x_strips: 4 strips [P, Tn] (f32 or bf16) -> 4 bf16 strips"""
                hs = [p_h.tile([P, Tn], b16, tag=f"{htag}{s}", name=f"{htag}{s}")
                      for s in range(EC)]
                step = 512 if Tn % 512 == 0 else NT
                nss = [(k * step, min(step, Tn - k * step))
                       for k in range((Tn + step - 1) // step)]
                for (o, n) in nss:
                    sl = slice(o, o + n)
                    s1 = p_ps.tile([1, 512], f32, tag="sc0", name="s1")
                    s2 = p_ps.tile([1, 512], f32, tag="sc1", name="s2")
                    xb_nts = {}
                    for s in range(EC):
                        if in_f32:
                            xb_nt = p_xb.tile([P, 512], b16, tag=f"xbn{s}",
                                              name="xbn")
                            nc.vector.tensor_copy(xb_nt[:, :n], x_strips[s][:, sl])
                            rhs_x = xb_nt[:, :n]
                            xb_nts[s] = rhs_x
                        else:
                            rhs_x = x_strips[s][:, sl]
                        nc.tensor.matmul(s1[:, :n], ones[:, 0:1], rhs_x,
                                         start=(s == 0), stop=(s == EC - 1))
                        sq_nt = p_sq.tile([P, 512], b16, tag="sqn", name="sqn")
                        nc.vector.tensor_mul(sq_nt[:, :n], rhs_x, rhs_x)
                        nc.tensor.matmul(s2[:, :n], ones[:, 0:1], sq_nt[:, :n],
                                         start=(s == 0), stop=(s == EC - 1))
                    m_b = p_sm.tile([1, 512], b16, tag="m_b", name="m_b")
                    nc.vector.tensor_scalar(m_b[:, :n], s1[:, :n], 1.0 / E, None, OP.mult)
                    stt = p_sm.tile([1, 1024], f32, tag="stt", name="stt")
                    sa, sb = stt[:, 0:n], stt[:, 512:512 + n]
                    nc.vector.tensor_scalar(sa, s2[:, :n], 1.0 / E, None, OP.mult)
                    nc.vector.tensor_mul(sb, m_b[:, :n], m_b[:, :n])
                    nc.vector.tensor_sub(sa, sa, sb)
                    act(sb, sa, AF.Ln, bias=eps1[:])
                    lnv = sb
                    r_b = p_sm.tile([1, 512], b16, tag="r_b", name="r_b")
                    act(r_b[:, :n], lnv, AF.Exp, scale=-0.5)
                    mB = p_ps.tile([P, 512], f32, tag="bc0", name="mB")
                    nc.tensor.matmul(mB[:, :n], ones[0:1, :], m_b[:, :n],
                                     start=True, stop=True)
                    rB = p_ps.tile([P, 512], f32, tag="sc2", name="rB")
                    nc.tensor.matmul(rB[:, :n], ones[0:1, :], r_b[:, :n],
                                     start=True, stop=True)
                    mBs = p_tmp.tile([P, 512], b16, tag="mBs", name="mBs")
                    act(mBs[:, :n], mB[:, :n], AF.Copy)
                    rBs = p_tmp.tile([P, 512], b16, tag="rBs", name="rBs")
                    act(rBs[:, :n], rB[:, :n], AF.Copy)
                    for s in range(EC):
                        if in_f32:
                            xbs = xb_nts[s]
                        else:
                            xbs = x_strips[s][:, sl]
                        t0 = p_tmp.tile([P, 512], b16, tag="lnt", name="lnt")
                        nc.vector.tensor_sub(t0[:, :n], xbs, mBs[:, :n])
                        nc.vector.tensor_mul(hs[s][:, sl], t0[:, :n], rBs[:, :n])
                return hs

            # ---------- transposed GEMM ----------
            _gm_cycle = [0]
            _sc_cycle = [0]
            _po_cycle = [0]

            def ps_tile(cycle=False):
                t = ("mm0", "mm1", "po0", "po1")[_gm_cycle[0] % 4]
                _gm_cycle[0] += 1
                return p_ps.tile([P, 512], f32, tag=t, name="gps")

            def sc_tile(tags=("sc0", "sc1", "sc2")):
                t = tags[_sc_cycle[0] % len(tags)]
                _sc_cycle[0] += 1
                return p_ps.tile([P, 512], f32, tag=t, name="sps")

            def po_tile(tags=("po0", "po1")):
                t = tags[_po_cycle[0] % len(tags)]
                _po_cycle[0] += 1
                return p_ps.tile([HD + 1, 512], f32, tag=t, name="po")

            def gemm(w_tile, col0, n_oc, x_strips, Tn, post, n_ec=EC):
                """out^T[oc] = sum_ec w_tile[:,ec,col0+oc*P:...].T @ x_strips[ec]"""
                step = 512 if Tn % 512 == 0 else NT
                nss = [(k * step, min(step, Tn - k * step))
                       for k in range((Tn + step - 1) // step)]
                for (o, n) in nss:
                    for oc in range(n_oc):
                        ps = ps_tile()
                        for ec in range(n_ec):
                            nc.tensor.matmul(
                                ps[:, :n],
                                w_tile[:, ec, col0 + oc * P:col0 + (oc + 1) * P],
                                x_strips[ec][:, o:o + n],
                                start=(ec == 0), stop=(ec == n_ec - 1))
                        post(oc, o, n, ps)

            # ---------- attention core (shared) ----------
            def attn_head_qt(kt_tile, q_tile, v_tile, h_, qt0, qn, kcs, mask_for,
                             ot_strips, windows=None):
                """one (head, query-tile): scores^T -> exp -> (mask) -> AV -> scale"""
                hp, hh = h_ // 2, h_ % 2
                banded = windows is not None
                po = po_tile(("po0", "po1", "sc2") if banded else ("po0", "po1"))
                first = True
                n_kc = len(kcs)
                covered = []  # disjoint sorted [lo, hi) q-ranges with a start=True writer
                for idx, (kc, mk) in enumerate(zip(kcs, mask_for)):
                    qo, qw = windows[idx] if windows is not None else (0, qn)
                    sps = sc_tile(("sc0", "sc1") if banded else ("sc0", "sc1", "sc2"))
                    nc.tensor.matmul(
                        sps[:, :qw],
                        kt_tile[HD * hh:HD * (hh + 1), hp, kc * P:(kc + 1) * P],
                        q_tile[HD * hh:HD * (hh + 1), hp,
                               qt0 + qo:qt0 + qo + qw],
                        start=True, stop=True)
                    pr = p_probs.tile([P, 512], b16, tag="pr", name="pr")
                    act(pr[:, :qw], sps[:, :qw], AF.Exp, scale=0.125)
                    if mk is not None:
                        prm = p_probs.tile([P, 512], b16, tag="prm", name="prm")
                        nc.vector.tensor_mul(prm[:, :qw], pr[:, :qw],
                                             masks[mk][:, qo:qo + qw])
                        pr = prm
                    if windows is None:
                        nc.tensor.matmul(po[:, :qn], v_tile[:, kc, h_, :],
                                         pr[:, :qn],
                                         start=first, stop=(idx == n_kc - 1))
                    else:
                        # split the AV into start=True parts (first writer of
                        # those q-columns; PSUM has_written is per element) and
                        # accumulate parts over already-written columns
                        parts = []
                        pos = qo
                        for (clo, chi) in covered + [(qo + qw, qo + qw)]:
                            if pos >= qo + qw:
                                break
                            if chi <= pos:
                                continue
                            if clo > pos:
                                parts.append((pos, min(clo, qo + qw), True))
                            if clo < qo + qw:
                                lo = max(clo, pos)
                                hi = min(chi, qo + qw)
                                if lo < hi:
                                    parts.append((lo, hi, False))
                            pos = max(pos, chi)
                        for (lo, hi, is_new) in parts:
                            nc.tensor.matmul(
                                po[:, lo:hi], v_tile[:, kc, h_, :],
                                pr[:, lo - qo:hi - qo],
                                start=is_new, stop=False,
                                skip_group_check=True)
                        covered.append((qo, qo + qw))
                        covered = sorted(covered)
                        merged = []
                        for (lo, hi) in covered:
                            if merged and lo <= merged[-1][1]:
                                merged[-1] = (merged[-1][0], max(hi, merged[-1][1]))
                            else:
                                merged.append((lo, hi))
                        covered = merged
                    first = False
                ou = p_tmp.tile([HD + 1, 512], b16, tag="ou", name="ou")
                with nc.allow_low_precision(reason="bf16 softmax normalizer"):
                    nc.vector.tensor_copy(ou[:, :qn], po[:, :qn])  # frees po bank
                    linv = p_sm.tile([1, 512], b16, tag="linv", name="linv")
                    nc.vector.reciprocal(linv[:, :qn], ou[HD:HD + 1, :qn])
                lB = p_ps.tile([P, 512], f32, tag="bc0", name="lB")
                nc.tensor.matmul(lB[:HD, :qn], ones[0:1, :HD], linv[:, :qn],
                                 start=True, stop=True)
                nc.vector.tensor_mul(
                    ot_strips[hp][HD * hh:HD * (hh + 1), qt0:qt0 + qn],
                    ou[:HD, :qn], lB[:HD, :qn])

            # ==================================================================
            # BLOCK 0: global attention
            # ==================================================================
            wqkv = p_wqkv.tile([P, EC, 3 * E], b16, tag="wqkv", name="wqkv0")
            for ec in range(EC):
                nc.sync.dma_start(wqkv[:, ec, :], wq[0][ec])
            wout = p_wout.tile([P, EC, E], b16, tag="wout", name="wout0")
            for ec in range(EC):
                nc.sync.dma_start(wout[:, ec, :], wo[0][ec])

            mark("b0.ln1+q")
            h1 = layernorm(xs, T, True, "h")
            # Q projection (ext window)
            qt_t = p_qt.tile([P, EC, T], b16, tag="qt", name="qt0")
            gemm(wqkv, 0, EC, h1, T,
                 lambda oc, o, n, ps: psum_to_sbuf(qt_t[:, oc, o:o + n], ps[:, :n],
                                                   0, "qkv", oc))

            mark("b0.kv")
            ot0 = p_ot.tile([P, EC, T], b16, tag="ot", name="ot0")
            ot_strips = [ot0[:, s, :] for s in range(EC)]
            with tc.tile_pool(name="kvfull", bufs=1) as p_kv, \
                 tc.tile_pool(name="xpan", bufs=3 if not use_bias else 1) as p_xp:
                ktf = p_kv.tile([P, EC, S], b16, tag="ktf", name="ktf")
                vf = p_kv.tile([P, NKC, H, HD + 1], b16, tag="vf", name="vf")
                nc.vector.memset(vf[:, :, :, HD:HD + 1], 1.0)
                for pan in range(S // 512):
                    xp = p_xp.tile([P, EC, 512], b16, tag="xp", name="xp")
                    for s in range(EC):
                        nc.sync.dma_start(xp[:, s, :],
                                          xfT[P * s:P * (s + 1), 512 * pan:512 * (pan + 1)])
                    hp_ = layernorm([xp[:, s, :] for s in range(EC)], 512, False, "hp")
                    # K^T columns for this panel
                    for oc in range(EC):
                        ps = ps_tile()
                        for ec in range(EC):
                            nc.tensor.matmul(ps[:],
                                             wqkv[:, ec, E + oc * P:E + (oc + 1) * P],
                                             hp_[ec][:],
                                             start=(ec == 0), stop=(ec == EC - 1))
                        psum_to_sbuf(ktf[:, oc, 512 * pan:512 * (pan + 1)], ps[:],
                                     0, "qkv", EC + oc, eng="act")
                    # V (normal layout) for this panel
                    for tck in range(4):
                        ps = ps_tile()
                        for ec in range(EC):
                            nc.tensor.matmul(ps[:],
                                             hp_[ec][:, tck * P:(tck + 1) * P],
                                             wqkv[:, ec, 2 * E:3 * E],
                                             start=(ec == 0), stop=(ec == EC - 1))
                        kc = pan * 4 + tck
                        if use_bias:
                            add_vbias(0, ps)
                        act(vf[:, kc, :, 0:HD],
                            ps[:].rearrange("p (h d) -> p h d", h=H), AF.Copy)
                mark("b0.attn")
                # attention (qt outer so out-proj can start per query tile)
                for (q0, qn_) in ((0, 512), (512, 512), (1024, 128)):
                    for h_ in range(H):
                        attn_head_qt(ktf, qt_t, vf, h_, q0, qn_,
                                     list(range(NKC)), [None] * NKC, ot_strips)

            # ---- pools for the post-block0 phases (opened after kvfull frees,
            # closed at end of emission so repeat>1 can reopen) ----
            _lstack = contextlib.ExitStack()
            lpool = lambda name, bufs, **kw: _lstack.enter_context(
                tc.tile_pool(name=name, bufs=bufs, **kw))
            p_kt = lpool("kt", 1)        # KT (banded) [P, EC, T] bf16
            p_v = lpool("v", 1)          # V_ext [P, NTC, H, HD+1] bf16
            p_g = lpool("g", 2 if not use_bias else 1)          # gelu out [P, FFC, NT] bf16
            p_wfc1 = lpool("wfc1", 1)
            p_wfc2 = lpool("wfc2", 1)

            # ---------- MLP (ln2 + fc1 + gelu + fc2 + residual) ----------
            def mlp(i, x_strips):
                h2 = layernorm(x_strips, T, True, "h")
                wf1 = p_wfc1.tile([P, EC, FF], b16, tag="wfc1", name=f"wfc1_{i}")
                for ec in range(EC):
                    nc.sync.dma_start(wf1[:, ec, :], w1[i][ec])
                wf2 = p_wfc2.tile([P, FFC, E], b16, tag="wfc2", name=f"wfc2_{i}")
                for fc in range(FFC):
                    nc.sync.dma_start(wf2[:, fc, :], w2[i][fc])
                x_new = [p_x.tile([P, T], f32, tag=f"x{s}", name=f"xm{i}_{s}")
                         for s in range(EC)]
                for nt in range(NQT):
                    o0 = nt * NT
                    g = p_g.tile([P, FFC, NT], b16, tag="g", name="g")
                    for fc in range(FFC):
                        ps = ps_tile()
                        for ec in range(EC):
                            nc.tensor.matmul(ps[:, :NT],
                                             wf1[:, ec, fc * P:(fc + 1) * P],
                                             h2[ec][:, o0:o0 + NT],
                                             start=(ec == 0), stop=(ec == EC - 1))
                        if use_bias:
                            nc.vector.tensor_scalar(ps[:, :NT], ps[:, :NT],
                                                    bslice(i, "fc1", fc), None, OP.add)
                        act(g[:, fc, :], ps[:, :NT], AF.Gelu_apprx_tanh)
                    for oc in range(EC):
                        ps = ps_tile()
                        for fc in range(FFC):
                            nc.tensor.matmul(ps[:, :NT],
                                             wf2[:, fc, oc * P:(oc + 1) * P],
                                             g[:, fc, :],
                                             start=(fc == 0), stop=(fc == FFC - 1))
                        if use_bias:
                            nc.vector.tensor_scalar(ps[:, :NT], ps[:, :NT],
                                                    bslice(i, "fc2", oc), None, OP.add)
                        nc.vector.tensor_add(x_new[oc][:, o0:o0 + NT],
                                             x_strips[oc][:, o0:o0 + NT], ps[:, :NT])
                return x_new

            mark("b0.proj+mlp")
            # block 0 out projection + residual + MLP
            x1 = [p_x.tile([P, T], f32, tag=f"x{s}", name=f"x1_{s}")
                  for s in range(EC)]
            def post_out0(oc, o, n, ps):
                if use_bias:
                    nc.vector.tensor_scalar(ps[:, :n], ps[:, :n],
                                            bslice(0, "out", oc), None, OP.add)
                nc.vector.tensor_add(x1[oc][:, o:o + n], xs[oc][:, o:o + n], ps[:, :n])
            gemm(wout, 0, EC, ot_strips, T, post_out0)
            x1 = mlp(0, x1)

            # ==================================================================
            # BLOCKS 1, 2: banded attention
            # ==================================================================
            mark("banded")
            x_cur = x1
            for i in (1, 2):
                lower = (i == 1)
                mark(f"b{i}.ln1qkv")
                wqkv = p_wqkv.tile([P, EC, 3 * E], b16, tag="wqkv", name=f"wqkv{i}")
                for ec in range(EC):
                    nc.sync.dma_start(wqkv[:, ec, :], wq[i][ec])
                wout = p_wout.tile([P, EC, E], b16, tag="wout", name=f"wout{i}")
                for ec in range(EC):
                    nc.sync.dma_start(wout[:, ec, :], wo[i][ec])
                h1 = layernorm(x_cur, T, True, "h")
                qt_t = p_qt.tile([P, EC, T], b16, tag="qt", name=f"qt{i}")
                kt_t = p_kt.tile([P, EC, T], b16, tag="kt", name=f"kt{i}")
                v_t = p_v.tile([P, NTC, H, HD + 1], b16, tag="v", name=f"v{i}")
                nc.vector.memset(v_t[:, :, :, HD:HD + 1], 1.0)
                otb = p_ot.tile([P, EC, T], b16, tag="ot", name=f"ot{i}")
                ot_strips = [otb[:, s, :] for s in range(EC)]
                mark(f"b{i}.attn")
                WINS = ([(0, 15), (0, 143), (128, 143), (256, 128)] if lower
                        else [(0, 143), (113, 143), (241, 143), (369, 15)])
                _gq = [0]

                def qkv_ps():
                    # only mm0/mm1 here: po/sc banks stay free for the
                    # interleaved attention chains
                    t = ("mm0", "mm1")[_gq[0] % 2]
                    _gq[0] += 1
                    return p_ps.tile([P, 512], f32, tag=t, name="gq")

                def emit_attn_qt(qt):
                    for h_ in range(H):
                        kcs, mks, wins = [], [], []
                        for c in range(4):
                            kc = 3 * qt + (c - 1 if lower else c)
                            if 0 <= kc < NTC:
                                kcs.append(kc)
                                mks.append(f"m_{'lo' if lower else 'up'}{c}")
                                wins.append(WINS[c])
                        attn_head_qt(kt_t, qt_t, v_t, h_, qt * NT, NT, kcs,
                                     mks, ot_strips, windows=wins)

                # emit qkv per token tile, with each attention query tile
                # interleaved as soon as its K/V columns exist (in-order
                # engine streams otherwise serialize attention behind the
                # whole projection)
                for nt in range(NQT):
                    o0 = nt * NT
                    for oc in range(EC):
                        ps = qkv_ps()
                        for ec in range(EC):
                            nc.tensor.matmul(
                                ps[:, :NT],
                                wqkv[:, ec, oc * P:(oc + 1) * P],
                                h1[ec][:, o0:o0 + NT],
                                start=(ec == 0), stop=(ec == EC - 1))
                        psum_to_sbuf(qt_t[:, oc, o0:o0 + NT], ps[:, :NT],
                                     i, "qkv", oc, eng="act")
                        ps = qkv_ps()
                        for ec in range(EC):
                            nc.tensor.matmul(
                                ps[:, :NT],
                                wqkv[:, ec, E + oc * P:E + (oc + 1) * P],
                                h1[ec][:, o0:o0 + NT],
                                start=(ec == 0), stop=(ec == EC - 1))
                        psum_to_sbuf(kt_t[:, oc, o0:o0 + NT], ps[:, :NT],
                                     i, "qkv", EC + oc, eng="act")
                    for tck in range(3 * nt, 3 * nt + 3):
                        ps = qkv_ps()
                        for ec in range(EC):
                            nc.tensor.matmul(ps[:],
                                             h1[ec][:, tck * P:(tck + 1) * P],
                                             wqkv[:, ec, 2 * E:3 * E],
                                             start=(ec == 0), stop=(ec == EC - 1))
                        if use_bias:
                            add_vbias(i, ps)
                        act(v_t[:, tck, :, 0:HD],
                            ps[:].rearrange("p (h d) -> p h d", h=H), AF.Copy)
                    aq = nt if lower else nt - 1
                    if aq >= 0:
                        emit_attn_qt(aq)
                if not lower:
                    emit_attn_qt(NQT - 1)
                mark(f"b{i}.projmlp")
                x_new = [p_x.tile([P, T], f32, tag=f"x{s}", name=f"xa{i}_{s}")
                         for s in range(EC)]
                def post_out(oc, o, n, ps, i=i, x_new=x_new, x_cur=x_cur):
                    if use_bias:
                        nc.vector.tensor_scalar(ps[:, :n], ps[:, :n],
                                                bslice(i, "out", oc), None, OP.add)
                    nc.vector.tensor_add(x_new[oc][:, o:o + n],
                                         x_cur[oc][:, o:o + n], ps[:, :n])
                gemm(wout, 0, EC, ot_strips, T, post_out)
                x_cur = mlp(i, x_new)

            mark("out")
            # output
            for s in range(EC):
                nc.sync.dma_start(yT[P * s:P * (s + 1), :], x_cur[s][:])
            _lstack.close()


        for _rep in range(repeat):
            _emit_once()

        # record build-order instruction -> phase map (before scheduling)
        _PHASE_OF.clear()
        names = [ins.name for bb in nc.main_func.blocks for ins in bb.instructions]
        bounds = [n for _, n in _PHASE_MARKS]
        labels = [l for l, _ in _PHASE_MARKS]
        import bisect as _bis
        for idx, nm in enumerate(names):
            j = _bis.bisect_right(bounds, idx) - 1
            _PHASE_OF[nm] = labels[j] if j >= 0 else "pre"

    nc.compile()
    return nc


# ------------------------------------------------------------------
# cached executor (compile once, run many)
# ------------------------------------------------------------------
class _Exec:
    def __init__(self, use_bias: bool):
        import jax
        import concourse.mybir as mybir
        from concourse import bass2jax
        from concourse.bass2jax import install_neuronx_cc_hook, _bass_exec_p
        from jax.sharding import Mesh, PartitionSpec
        from jax.experimental.shard_map import shard_map

        install_neuronx_cc_hook()
        nc = _build_nc(use_bias)
        self.nc = nc

        part_name = (nc.partition_id_tensor.name
                     if nc.partition_id_tensor is not None else None)
        in_names, out_names, out_avals = [], [], []
        self.zero_shapes = []
        for alloc in nc.m.functions[0].allocations:
            if not isinstance(alloc, mybir.MemoryLocationSet):
                continue
            name = alloc.memorylocations[0].name
            if alloc.kind == "ExternalInput":
                if name != part_name:
                    in_names.append(name)
            elif alloc.kind == "ExternalOutput":
                out_names.append(name)
                shape = tuple(alloc.tensor_shape)
                dtype = mybir.dt.np(alloc.dtype)
                out_avals.append(jax.core.ShapedArray(shape, dtype))
                self.zero_shapes.append((shape, dtype))
        n_params = len(in_names)
        all_in = in_names + out_names
        if part_name is not None:
            all_in = all_in + [part_name]
        self.in_names = in_names
        self.out_names = out_names
        n_outs = len(out_names)

        def _body(*args):
            operands = list(args)
            if part_name is not None:
                operands.append(bass2jax.partition_id_tensor())
            outs = _bass_exec_p.bind(
                *operands,
                out_avals=tuple(out_avals),
                in_names=tuple(all_in),
                out_names=tuple(out_names),
                lowering_input_output_aliases=(),
                sim_require_finite=True,
                sim_require_nnan=True,
                nc=nc,
            )
            return tuple(outs)
        self._body = _body

        devices = jax.devices()[:N_CORES]
        mesh = Mesh(np.asarray(devices), ("core",))
        in_specs = (PartitionSpec("core"),) * (n_params + n_outs)
        out_specs = (PartitionSpec("core"),) * n_outs
        donate = tuple(range(n_params, n_params + n_outs))
        self.fn = jax.jit(
            shard_map(_body, mesh=mesh, in_specs=in_specs,
                      out_specs=out_specs, check_rep=False),
            donate_argnums=donate, keep_unused=True)
        self.out_avals = out_avals

    def bench(self, in_maps, iters=10):
        """device-resident-input timing: returns per-iteration seconds"""
        import time
        import jax
        from jax.sharding import Mesh, PartitionSpec, NamedSharding
        if not hasattr(self, "_bench_fn"):
            from jax.experimental.shard_map import shard_map
            devices = jax.devices()[:N_CORES]
            mesh = Mesh(np.asarray(devices), ("core",))
            n_in = len(self.in_names) + len(self.zero_shapes)
            self._bench_fn = jax.jit(
                shard_map(self._body, mesh=mesh,
                          in_specs=(PartitionSpec("core"),) * n_in,
                          out_specs=(PartitionSpec("core"),) * len(self.out_names),
                          check_rep=False),
                keep_unused=True)
            self._bench_sharding = NamedSharding(mesh, PartitionSpec("core"))
        concat_in = [
            np.concatenate([np.asarray(in_maps[c][n]) for c in range(N_CORES)], axis=0)
            for n in self.in_names
        ] + [np.zeros((N_CORES * s[0], *s[1:]), d) for (s, d) in self.zero_shapes]
        import jax
        dev_in = [jax.device_put(a, self._bench_sharding) for a in concat_in]
        out = jax.block_until_ready(self._bench_fn(*dev_in))  # warm/compile
        t0 = time.time()
        for _ in range(iters):
            out = self._bench_fn(*dev_in)
        jax.block_until_ready(out)
        return (time.time() - t0) / iters

    def run(self, in_maps):
        """in_maps: list of 8 dicts name->np.ndarray. returns list of dicts"""
        concat_in = [
            np.concatenate([np.asarray(in_maps[c][n]) for c in range(N_CORES)], axis=0)
            for n in self.in_names
        ]
        concat_zeros = [np.zeros((N_CORES * s[0], *s[1:]), d)
                        for (s, d) in self.zero_shapes]
        outs = self.fn(*concat_in, *concat_zeros)
        import jax
        outs = jax.block_until_ready(outs)
        res = []
        for c in range(N_CORES):
            d = {}
            for idx, n in enumerate(self.out_names):
                shp = self.out_avals[idx].shape
                d[n] = np.asarray(outs[idx]).reshape(N_CORES, *shp)[c]
            res.append(d)
        return res


def _get_exec(use_bias: bool) -> "_Exec":
    key = bool(use_bias)
    if key not in _EXEC_CACHE:
        _EXEC_CACHE[key] = _Exec(key)
    return _EXEC_CACHE[key]


# ------------------------------------------------------------------
# host-side input prep
# ------------------------------------------------------------------
def _band_masks():
    ki = np.arange(P)[:, None]
    qi = np.arange(NT)[None, :]
    out = {}
    for c in range(4):
        # lower band, k-chunk kc = 3t + (c-1):  0 <= (i-k) <= W-1 with
        # i-k = qi - ki + 128*(1-c)
        d = qi - ki + P * (1 - c)
        out[f"m_lo{c}"] = ((d >= 0) & (d <= W_BAND - 1)).astype(bf16)
        # upper band, k-chunk kc = 3t + c:  0 <= (k-i) <= W-1 with
        # k-i = ki - qi + 128*c
        d = ki - qi + P * c
        out[f"m_up{c}"] = ((d >= 0) & (d <= W_BAND - 1)).astype(bf16)
    return out


def kernel(x, ln1_w, ln1_b, ln2_w, ln2_b, qkv_w, qkv_b, out_w, out_b,
           fc1_w, fc1_b, fc2_w, fc2_b):
    x = np.asarray(x, np.float32)
    ln1_w = np.asarray(ln1_w, np.float32); ln1_b = np.asarray(ln1_b, np.float32)
    ln2_w = np.asarray(ln2_w, np.float32); ln2_b = np.asarray(ln2_b, np.float32)
    qkv_w = np.asarray(qkv_w, np.float32); qkv_b = np.asarray(qkv_b, np.float32)
    out_w = np.asarray(out_w, np.float32); out_b = np.asarray(out_b, np.float32)
    fc1_w = np.asarray(fc1_w, np.float32); fc1_b = np.asarray(fc1_b, np.float32)
    fc2_w = np.asarray(fc2_w, np.float32); fc2_b = np.asarray(fc2_b, np.float32)

    # fold LN affine into following projection weights (exact):
    #   h = z*w + b  =>  h @ Wt.T = z @ (W*w).T + b @ W.T
    wq_f, wo_f, w1_f, w2_f = [], [], [], []
    biases = []
    for i in range(3):
        qw = qkv_w[i] * ln1_w[i][None, :]
        qb = qkv_b[i] + qkv_w[i] @ ln1_b[i]
        f1 = fc1_w[i] * ln2_w[i][None, :]
        f1b = fc1_b[i] + fc1_w[i] @ ln2_b[i]
        wq_f.append(qw); w1_f.append(f1)
        wo_f.append(out_w[i]); w2_f.append(fc2_w[i])
        # packed per-feature bias tile [P, 32] (q, k, out, fc1, fc2) + v row
        bias_pack = np.zeros((P, 32), np.float32)
        bias_pack[:, 0:8] = qb[:2 * E].reshape(8, P).T
        bias_pack[:, 8:12] = out_b[i].reshape(4, P).T
        bias_pack[:, 12:28] = f1b.reshape(16, P).T
        bias_pack[:, 28:32] = fc2_b[i].reshape(4, P).T
        biases.append((bias_pack, qb[2 * E:].reshape(1, E).astype(bf16)))
    use_bias = any(np.abs(b).max() > 0 or np.abs(np.asarray(v, np.float32)).max() > 0
                   for b, v in biases)

    ex = _get_exec(use_bias)

    masks = _band_masks()
    # weight tensors, transposed to [EC, P, out] layout
    weights = {}
    for i in range(3):
        weights[f"qkvT{i}"] = np.ascontiguousarray(
            wq_f[i].T.reshape(EC, P, 3 * E)).astype(bf16)
        weights[f"outT{i}"] = np.ascontiguousarray(
            wo_f[i].T.reshape(EC, P, E)).astype(bf16)
        weights[f"fc1T{i}"] = np.ascontiguousarray(
            w1_f[i].T.reshape(EC, P, FF)).astype(bf16)
        weights[f"fc2T{i}"] = np.ascontiguousarray(
            w2_f[i].T.reshape(FFC, P, E)).astype(bf16)
        if use_bias:
            weights[f"bias{i}"] = biases[i][0]
            weights[f"vbias{i}"] = biases[i][1]

    in_maps = []
    for core in range(N_CORES):
        b, c = divmod(core, CHUNKS)
        e0 = EXT_STARTS[c]
        xT_ext = np.ascontiguousarray(x[b, e0:e0 + T, :].T)
        xfT = np.ascontiguousarray(x[b].T).astype(bf16)
        m = {"xT": xT_ext, "xfT": xfT, **weights, **masks}
        in_maps.append(m)

    res = ex.run(in_maps)

    out = np.empty((B, S, E), np.float32)
    for core in range(N_CORES):
        b, c = divmod(core, CHUNKS)
        yT_ = res[core]["yT"]               # [E, T]
        off = OWN_OFF[c]
        out[b, OWN * c:OWN * (c + 1), :] = yT_[:, off:off + OWN].T
    return out



# revision 15
# speedup vs baseline: 1.2150x; 1.2150x over previous
"""Trainium2 Bass kernel for nn_AlternateAttention (3-block transformer:
global attention, lower-band attention, upper-band attention, each with MLP).

Sharding: 8 cores = 2 batches x 4 sequence chunks of 1024 tokens, each core
processing an extended window of 1152 tokens (64-token halo each side) so the
banded blocks need no inter-core communication. Block 0 (global attention)
needs full-sequence K/V; each core recomputes them from the replicated raw
input (LN1+KV projection over the full 4096 tokens of its batch).

Layout: activations live TRANSPOSED in SBUF ([feature, token]) so projections
are natural matmuls. All projection weights are cast to fp8e4 (scaled x64 so
N(0,0.02) weights stay in the normal range) and GEMMs run in DoubleRow perf
mode (two 128-deep k-tiles per instruction at 0.5 cycles/row).  The 1/64
de-scale is folded into the psum->sbuf consumer (activation scale or
scalar_tensor_tensor) -- for attention scores both Q and K carry x64 so the
softmax exp scale absorbs the 4096.  Attention probabilities and V are fp8
so AV also runs DoubleRow; exp reads score PAIRS from 2-bank [P,2,512] PSUM
tiles to halve the Activation-engine fixed overhead.
"""
import contextlib
import numpy as np
import ml_dtypes

# ---- problem constants (hardcoded per contract) ----
B, S, E, H, W_BAND, FF = 2, 4096, 512, 8, 16, 2048
HD = E // H                      # 64
N_CORES = 8
CHUNKS = 4                       # sequence chunks per batch
OWN = S // CHUNKS                # 1024
HALO = 64
T = OWN + 2 * HALO               # 1152 = 9*128
P = 128
NT = 384                         # token tile (3 per T)
NQT = T // NT                    # 3
NKC = S // P                     # 32 k-chunks for global attention
NTC = T // P                     # 9 token chunks of ext window
EC = E // P                      # 4 feature strips
FFC = FF // P                    # 16
VD = HD + 2                      # V tile row pitch: 66 so DoubleRow pair stride (H*VD) % 16 == 0

WS = 64.0                        # weight pre-scale into fp8
POLY_PER = 4                     # DVE-exp'd score pairs per (head, q-chunk)
IWS = 1.0 / WS
SC_EXP = 0.125 / (WS * WS)       # exp scale absorbing q/k weight scales

EXT_STARTS = [max(0, min(OWN * c - HALO, S - T)) for c in range(CHUNKS)]
OWN_OFF = [OWN * c - EXT_STARTS[c] for c in range(CHUNKS)]

_EXEC_CACHE = {}
_PHASE_MARKS = []
_PHASE_OF = {}

bf16 = ml_dtypes.bfloat16
f8e4 = ml_dtypes.float8_e4m3


# ------------------------------------------------------------------
# device program
# ------------------------------------------------------------------
def _patch_act_tables():
    # The bacc table-load placement maps exp -> 'exp_and_others' and
    # ln -> 'natural_log', thrashing the ACT table RAM (~1.3us per switch,
    # dozens of switches). Restrict the choice to the two sets that cover
    # everything this kernel uses so exp/ln never evict each other.
    import concourse.hw_specs as hw_specs
    import concourse.bacc as bacc_mod
    import concourse.bass_interp as bass_interp
    if getattr(hw_specs, "_aa_patched", False):
        return
    orig = hw_specs.get_activation_tables
    keep = {"natural_log_exp_and_others", "gelu_apprx_tanh_and_others"}

    def _gat(arch):
        tabs = orig(arch)
        return {k: (v if k in keep else set()) for k, v in tabs.items()}

    hw_specs.get_activation_tables = _gat
    bacc_mod.get_activation_tables = _gat
    bass_interp.get_activation_tables = _gat
    hw_specs._aa_patched = True


def _build_nc(use_bias: bool, repeat: int = 1):
    import concourse.bacc as bacc
    import concourse.mybir as mybir
    import concourse.tile as tile

    _patch_act_tables()

    f32 = mybir.dt.float32
    b16 = mybir.dt.bfloat16
    f8 = mybir.dt.float8e4
    AF = mybir.ActivationFunctionType
    OP = mybir.AluOpType
    DR = mybir.MatmulPerfMode.DoubleRow

    nc = bacc.Bacc("TRN2", target_bir_lowering=False, debug=False,
                   num_devices=N_CORES)
    _PHASE_MARKS.clear()

    def mark(label):
        n = sum(len(b.instructions) for b in nc.main_func.blocks)
        _PHASE_MARKS.append((label, n))

    # ---- dram tensors ----
    xT = nc.dram_tensor("xT", [E, T], b16, kind="ExternalInput")
    xfT = nc.dram_tensor("xfT", [E, S], b16, kind="ExternalInput")
    wq, wo, w1, w2, bias_d = [], [], [], [], []
    for i in range(3):
        wq.append(nc.dram_tensor(f"qkvT{i}", [EC, P, 3 * E], f8, kind="ExternalInput"))
        wo.append(nc.dram_tensor(f"outT{i}", [EC, P, E], f8, kind="ExternalInput"))
        w1.append(nc.dram_tensor(f"fc1T{i}", [EC, P, FF], f8, kind="ExternalInput"))
        w2.append(nc.dram_tensor(f"fc2T{i}", [FFC, P, E], f8, kind="ExternalInput"))
        if W_COMP_MLP:
            w1.append(nc.dram_tensor(f"fc1L{i}", [EC, P, FF], f8, kind="ExternalInput"))
            w2.append(nc.dram_tensor(f"fc2L{i}", [FFC, P, E], f8, kind="ExternalInput"))
        if use_bias:
            # packed per-feature biases for transposed-layout outputs:
            # [q(4xP) | k(4xP) | out(4xP) | fc1(16xP) | fc2(4xP)] -> [P, 32]
            bias_d.append(nc.dram_tensor(f"bias{i}", [P, 32], f32, kind="ExternalInput"))
            bias_d.append(nc.dram_tensor(f"vbias{i}", [1, E], b16, kind="ExternalInput"))
    # paired 0/1 bf16 stripe-mask tiles for banded attention: [P, 2, NT]
    mask_d = {}
    for nm in ("m2_lo01", "m2_lo23", "m2_lo12", "m2_up01", "m2_up23"):
        mask_d[nm] = nc.dram_tensor(nm, [P, 2, NT], b16, kind="ExternalInput")
    yT = nc.dram_tensor("yT", [E, T], f32, kind="ExternalOutput")

    from concourse.tile import add_dep_helper as _adh

    with tile.TileContext(nc) as tc, contextlib.ExitStack() as ctx:
        pool = lambda name, bufs, **kw: ctx.enter_context(
            tc.tile_pool(name=name, bufs=bufs, **kw))

        # Order ACT instructions across table-set boundaries (exp/ln vs gelu)
        # so the activation-table RAM isn't thrashed (~1.3us per reload).
        # Within a set no ordering is imposed.
        _act_sets = {AF.Gelu_apprx_tanh: 1}
        _prev_set = [None]
        _prev_insts = [[]]
        _cur_insts = [[]]

        def act(out, in_, func, **kw):
            inst = nc.scalar.activation(out, in_, func, **kw)
            if func == AF.Copy:
                return inst
            s = _act_sets.get(func, 0)
            if _prev_set[0] is None:
                _prev_set[0] = s
            if s != _prev_set[0]:
                _prev_insts[0] = _cur_insts[0][-64:]
                _cur_insts[0] = []
                _prev_set[0] = s
            for p in _prev_insts[0]:
                _adh(inst.ins, p.ins, sync=True, reason="act-table batch order")
            _cur_insts[0].append(inst)
            return inst

        # ---- pools live for the whole kernel ----
        p_x = pool("x", 2)           # residual strips f32, tags x0..x3
        p_h = pool("h", 2)           # LN output fp8 [P, EC, Tn]
        p_sq = pool("sq", 4)         # per-nt squares bf16
        p_qt = pool("qt", 1)         # QT [P, EC, T] fp8
        p_kt = pool("kt", 1)         # KT (banded) [P, EC, T] fp8
        p_ot = pool("ot", 1)         # attention out strips fp8
        p_wqkv = pool("wqkv", 2)     # qkv weights [P, EC, 3E] fp8
        p_wout = pool("wout", 2)
        p_probs = pool("probs", 4)   # exp outputs fp8 [P, 2, 512]
        p_tmp = pool("tmp", 2)       # temporaries
        p_sm = pool("sm", 2)         # small [1, n] stat vectors
        p_c = pool("const", 1)       # ones, masks, biases
        # PSUM: 8 banks = two 2-bank score slots + four single banks
        p_ps = pool("ps", 1, space="PSUM")
        # sA/sB: [P,2,512] paired attention scores (also misc gemm accum)
        # p0..p3: po accumulators / LN stats / broadcasts / gemm accum

        ones = p_c.tile([P, P], b16)
        nc.vector.memset(ones[:], 1.0)
        eps1 = p_c.tile([1, 1], f32)
        nc.vector.memset(eps1[:], 1e-5)
        masks = {}
        for nm, d in mask_d.items():
            mt = p_c.tile([P, 2, NT], b16, tag=f"mask_{nm}", name=f"mask_{nm}")
            nc.sync.dma_start(mt[:], d[:])
            masks[nm] = mt
        bias_t, vbias_t = [], []
        if use_bias:
            for i in range(3):
                bt = p_c.tile([P, 32], f32, tag=f"bias{i}", name=f"bias{i}")
                nc.sync.dma_start(bt[:], bias_d[2 * i][:])
                bias_t.append(bt)
                vt = p_c.tile([1, E], b16, tag=f"vbias{i}", name=f"vbias{i}")
                nc.sync.dma_start(vt[:], bias_d[2 * i + 1][:])
                vbias_t.append(vt)

        def bslice(i, group, oc):
            base = {"qkv": 0, "out": 8, "fc1": 12, "fc2": 28}[group]
            return bias_t[i][:, base + oc:base + oc + 1]

        def add_vbias(i, ps):
            # V projection output is in normal layout [token, feat]: bias
            # varies along the free dim -> broadcast with a K=1 matmul.
            vb = p_ps.tile([P, 512], f32, tag="p0", name="vb")
            nc.tensor.matmul(vb[:], ones[0:1, :], vbias_t[i][:],
                             start=True, stop=True)
            vbs = p_tmp.tile([P, 512], f32, tag="vbs", name="vbs")
            nc.vector.tensor_copy(vbs[:], vb[:])
            nc.vector.tensor_add(ps[:], ps[:], vbs[:])

        def _emit_once():
            # load residual strips
            xs = []
            for s in range(EC):
                t = p_x.tile([P, T], b16, tag=f"x{s}", name=f"xin{s}")
                nc.sync.dma_start(t[:], xT[P * s:P * (s + 1), :])
                xs.append(t)

            # ---------- layernorm ----------
            def layernorm(x_strips, Tn, htag,
                          stat_tags=("sA",), bc_tags=("sB", "sC"),
                          sq_eng=None):
                """x_strips: 4 bf16 strips [P, Tn] -> fp8 [P,EC,Tn].
                Stats via ones-matmul partition reductions (bf16), squares on
                GPSIMD (SBUF->SBUF), per-token mean/rstd broadcast via K=1
                matmuls, normalize on DVE in 2x bf16 mode, fp8 output."""
                sq_eng = sq_eng or nc.gpsimd
                h8 = p_h.tile([P, EC, Tn], f8, tag=htag, name=htag)
                step = 512 if Tn % 512 == 0 else NT
                nss = [(k * step, min(step, Tn - k * step))
                       for k in range((Tn + step - 1) // step)]
                for si, (o, n) in enumerate(nss):
                    sl = slice(o, o + n)
                    s12 = p_ps.tile([1, 1024], f32, tag=stat_tags[0], name="s12")
                    s1 = s12[:, 0:512]
                    s2 = s12[:, 512:1024]
                    for s in range(EC):
                        rhs_x = x_strips[s][:, sl]
                        nc.tensor.matmul(s1[:, :n], ones[:, 0:1], rhs_x,
                                         start=(s == 0), stop=(s == EC - 1))
                        sq_nt = p_sq.tile([P, 512], b16, tag="sqn", name="sqn")
                        sq_eng.tensor_mul(sq_nt[:, :n], rhs_x, rhs_x)
                        nc.tensor.matmul(s2[:, :n], ones[:, 0:1], sq_nt[:, :n],
                                         start=(s == 0), stop=(s == EC - 1))
                    with nc.allow_low_precision(reason="bf16 LN stats"):
                        m_b = p_sm.tile([1, 512], b16, tag="m_b", name="m_b")
                        nc.vector.tensor_scalar(m_b[:, :n], s1[:, :n], 1.0 / E,
                                                None, OP.mult)
                        stt = p_sm.tile([1, 1024], b16, tag="stt", name="stt")
                        sa, sb = stt[:, 0:n], stt[:, 512:512 + n]
                        nc.vector.tensor_scalar(sa, s2[:, :n], 1.0 / E, None,
                                                OP.mult)
                        nc.vector.tensor_mul(sb, m_b[:, :n], m_b[:, :n])
                        nc.vector.tensor_sub(sa, sa, sb)
                        act(sb, sa, AF.Ln, bias=eps1[:])
                        lnv = sb
                        r_b = p_sm.tile([1, 512], b16, tag="r_b", name="r_b")
                        act(r_b[:, :n], lnv, AF.Exp, scale=-0.5)
                    bc = p_ps.tile([P, 2, 512], f32,
                                   tag=bc_tags[si % len(bc_tags)], name="bc")
                    mB, rB = bc[:, 0, :], bc[:, 1, :]
                    nc.tensor.matmul(mB[:, :n], ones[0:1, :], m_b[:, :n],
                                     start=True, stop=True)
                    nc.tensor.matmul(rB[:, :n], ones[0:1, :], r_b[:, :n],
                                     start=True, stop=True)
                    mBs = p_tmp.tile([P, 512], b16, tag="mBs", name="mBs")
                    act(mBs[:, :n], mB[:, :n], AF.Copy)
                    rBs = p_tmp.tile([P, 512], b16, tag="rBs", name="rBs")
                    act(rBs[:, :n], rB[:, :n], AF.Copy)
                    with nc.allow_low_precision(reason="fp8 GEMM inputs"):
                        for s in range(EC):
                            t0 = p_tmp.tile([P, 512], b16, tag="lnt", name="lnt")
                            nc.vector.tensor_sub(t0[:, :n], x_strips[s][:, sl],
                                                 mBs[:, :n])
                            nc.vector.tensor_mul(h8[:, s, sl], t0[:, :n],
                                                 rBs[:, :n])
                return h8

            # ---------- PSUM rotors ----------
            def rotor(tags, shape, name, tagged=False):
                st = [0]

                def get():
                    t = tags[st[0] % len(tags)]
                    st[0] += 1
                    tl = p_ps.tile(shape, f32, tag=t, name=name)
                    return (tl, t) if tagged else tl
                return get

            sc2_g = rotor(("sA", "sB"), [P, 2, 512], "sc2")   # paired scores
            sc2_c = rotor(("sC",), [P, 2, 512], "sc2c")       # poly-exp pairs
            po_g = rotor(("p0", "p1"), [VD, 512], "po", tagged=True)
            po_b = rotor(("p0", "p1"), [VD, 512], "po", tagged=True)
            gem_r = rotor(("sA", "sB"), [P, 512], "gps")      # non-attn phases
            qkv_r = rotor(("sC",), [P, 512], "qps")           # banded qkv
            mlp_r = rotor(("sC", "p0", "p1"), [P, 512], "mps")
            kv_r = rotor(("p0", "p1"), [P, 512], "kvps")

            # ---------- DoubleRow GEMM (transposed layout) ----------
            def gemm8(w8, col0, n_oc, h8, Tn, post, ps_get=None, bias_i=None,
                      bias_grp=None):
                ps_get = ps_get or gem_r
                step = 512 if Tn % 512 == 0 else NT
                nss = [(k * step, min(step, Tn - k * step))
                       for k in range((Tn + step - 1) // step)]
                for (o, n) in nss:
                    for oc in range(n_oc):
                        ps = ps_get()
                        for sp in range(EC // 2):
                            nc.tensor.matmul(
                                ps[:, :n],
                                w8[:, 2 * sp:2 * sp + 2,
                                   col0 + oc * P:col0 + (oc + 1) * P],
                                h8[:, 2 * sp:2 * sp + 2, o:o + n],
                                start=(sp == 0), stop=(sp == EC // 2 - 1),
                                perf_mode=DR)
                        if use_bias and bias_grp is not None:
                            nc.vector.tensor_scalar(
                                ps[:, :n], ps[:, :n],
                                bslice(bias_i, bias_grp, oc), None, OP.add)
                        post(oc, o, n, ps)

            def psum_to_sbuf8(dst_ap, ps_ap, eng="dve", scale=None):
                """copy matmul accumulator to sbuf (fp8/bf16 out)"""
                with nc.allow_low_precision(reason="fp8 GEMM inputs"):
                    if eng == "act":
                        act(dst_ap, ps_ap, AF.Copy,
                            scale=1.0 if scale is None else scale)
                    elif scale is None:
                        nc.vector.tensor_copy(dst_ap, ps_ap)
                    else:
                        nc.vector.tensor_scalar(dst_ap, ps_ap, scale, None,
                                                OP.mult)

            # ---------- DVE exp via repeated squaring:
            # exp(s*x) ~ (1 + y + y^2/2)^8 with y = s*x/8  (rel err < 1% for
            # |s*x| <= 1.8).  Uses only tensor_scalar / tensor_tensor, which
            # run in 2x/4x DVE modes on bf16 -- offloads softmax exp from the
            # saturated Activation engine.

            def poly_exp(dst_ap, ps_ap):
                y = p_tmp.tile([P, 2, 512], b16, tag="pey", name="pey")
                u = p_tmp.tile([P, 2, 512], b16, tag="peu", name="peu")
                n2 = ps_ap.shape[-1]
                ya = y[:, :, :n2]
                ua = u[:, :, :n2]
                with nc.allow_low_precision(reason="poly exp"):
                    # t = y/sqrt(2) + 1/sqrt(2);  z = t^2 + 0.5 = 1 + y + y^2/2
                    nc.vector.tensor_scalar(ya, ps_ap, SC_EXP / 8 * 0.70710678,
                                            0.70710678, OP.mult, OP.add)
                    nc.vector.tensor_mul(ua, ya, ya)
                    nc.vector.tensor_scalar(ua, ua, 0.5, None, OP.add)
                    nc.vector.tensor_mul(ua, ua, ua)
                    nc.vector.tensor_mul(ua, ua, ua)
                    nc.vector.tensor_mul(dst_ap, ua, ua)

            # ---------- attention finish (shared) ----------
            def attn_finish(po_t, h_, qt0, qn, ot_strips):
                po, po_tag = po_t
                hp, hh = h_ // 2, h_ % 2
                ou = p_tmp.tile([VD, 512], b16, tag="ou", name="ou")
                with nc.allow_low_precision(reason="bf16 softmax normalizer"):
                    nc.vector.tensor_copy(ou[:, :qn], po[:, :qn])  # frees bank
                    linv = p_sm.tile([1, 512], b16, tag="linv", name="linv")
                    nc.vector.reciprocal(linv[:, :qn], ou[HD:HD + 1, :qn])
                lB = p_ps.tile([P, 512], f32, tag=po_tag, name="lB")
                nc.tensor.matmul(lB[:HD, :qn], ones[0:1, :HD], linv[:, :qn],
                                 start=True, stop=True)
                with nc.allow_low_precision(reason="fp8 GEMM inputs"):
                    nc.vector.tensor_mul(
                        ot_strips[hp][HD * hh:HD * (hh + 1), qt0:qt0 + qn],
                        ou[:HD, :qn], lB[:HD, :qn])

            # ==================================================================
            # BLOCK 0: global attention
            # ==================================================================
            wqkv = p_wqkv.tile([P, EC, 3 * E], f8, tag="wqkv", name="wqkv0")
            for ec in range(EC):
                nc.sync.dma_start(wqkv[:, ec, :], wq[0][ec])
            wout = p_wout.tile([P, EC, E], f8, tag="wout", name="wout0")
            for ec in range(EC):
                nc.sync.dma_start(wout[:, ec, :], wo[0][ec])

            mark("b0.ln1+q")
            h1 = layernorm(xs, T, "h")
            # Q projection (ext window)
            qt_t = p_qt.tile([P, EC, T], f8, tag="qt", name="qt0")
            gemm8(wqkv, 0, EC, h1, T,
                  lambda oc, o, n, ps: psum_to_sbuf8(qt_t[:, oc, o:o + n],
                                                     ps[:, :n], eng="act"),
                  bias_i=0, bias_grp="qkv")

            mark("b0.kv")
            ot0 = p_ot.tile([P, EC, T], f8, tag="ot", name="ot0")
            ot_strips = [ot0[:, s, :] for s in range(EC)]
            with tc.tile_pool(name="kvfull", bufs=1) as p_kv, \
                 tc.tile_pool(name="xpan", bufs=3 if not use_bias else 1) as p_xp:
                ktf = p_kv.tile([P, EC, S], f8, tag="ktf", name="ktf")
                vf = p_kv.tile([P, NKC, H, VD], f8, tag="vf", name="vf")
                nc.vector.memset(vf[:, :, :, HD:HD + 1], 1.0)
                nc.vector.memset(vf[:, :, :, HD + 1:VD], 0.0)
                for pan in range(S // 512):
                    xp = p_xp.tile([P, EC, 512], b16, tag="xp", name="xp")
                    for s in range(EC):
                        nc.sync.dma_start(xp[:, s, :],
                                          xfT[P * s:P * (s + 1), 512 * pan:512 * (pan + 1)])
                    hp8 = layernorm([xp[:, s, :] for s in range(EC)], 512,
                                    "hp")
                    # K^T columns for this panel
                    for oc in range(EC):
                        ps = kv_r()
                        for sp in range(EC // 2):
                            nc.tensor.matmul(
                                ps[:],
                                wqkv[:, 2 * sp:2 * sp + 2,
                                     E + oc * P:E + (oc + 1) * P],
                                hp8[:, 2 * sp:2 * sp + 2, :],
                                start=(sp == 0), stop=(sp == EC // 2 - 1),
                                perf_mode=DR)
                        if use_bias:
                            nc.vector.tensor_scalar(ps[:], ps[:],
                                                    bslice(0, "qkv", EC + oc),
                                                    None, OP.add)
                        psum_to_sbuf8(ktf[:, oc, 512 * pan:512 * (pan + 1)],
                                      ps[:], eng="act")
                    # V (normal layout) for this panel
                    for tck in range(4):
                        ps = kv_r()
                        for sp in range(EC // 2):
                            nc.tensor.matmul(
                                ps[:],
                                hp8[:, 2 * sp:2 * sp + 2,
                                    tck * P:(tck + 1) * P],
                                wqkv[:, 2 * sp:2 * sp + 2, 2 * E:3 * E],
                                start=(sp == 0), stop=(sp == EC // 2 - 1),
                                perf_mode=DR)
                        kc = pan * 4 + tck
                        if use_bias:
                            add_vbias(0, ps)
                        with nc.allow_low_precision(reason="fp8 V"):
                            act(vf[:, kc, :, 0:HD],
                                ps[:].rearrange("p (h d) -> p h d", h=H),
                                AF.Copy, scale=IWS)
                mark("b0.attn")
                # attention: paired scores -> one exp per 2 k-chunks ->
                # DoubleRow AV accumulate
                for (q0, qn_) in ((0, 512), (512, 512), (1024, 128)):
                    n_poly = POLY_PER if qn_ == 512 else POLY_PER // 2
                    stride = (NKC // 2) // max(n_poly, 1)
                    for h_ in range(H):
                        poly_cs = set(range(0, NKC // 2, stride)[:n_poly])
                        hp, hh = h_ // 2, h_ % 2
                        po_t = po_g()
                        po = po_t[0]

                        def score_pair(sp2, c):
                            for i in (0, 1):
                                kc = 2 * c + i
                                nc.tensor.matmul(
                                    sp2[:, i, :qn_],
                                    ktf[HD * hh:HD * (hh + 1), hp,
                                        kc * P:(kc + 1) * P],
                                    qt_t[HD * hh:HD * (hh + 1), hp,
                                         q0:q0 + qn_],
                                    start=True, stop=True)

                        # poly pairs: scores+DVE-exp emitted early, AV deferred
                        # until a few ACT pairs later so the in-order PE stream
                        # never waits on the DVE chain.
                        n_emitted = [0]
                        pend = []

                        def av(pr8, c, last=False):
                            nc.tensor.matmul(
                                po[:, :qn_],
                                vf[:, 2 * c:2 * c + 2, h_, :],
                                pr8[:, :, :qn_],
                                start=(n_emitted[0] == 0), stop=last,
                                perf_mode=DR)
                            n_emitted[0] += 1

                        def start_poly(c):
                            sp2 = sc2_c()
                            score_pair(sp2, c)
                            pr8 = p_probs.tile([P, 2, 512], f8, tag="pr",
                                               name="pr")
                            poly_exp(pr8[:, :, :qn_], sp2[:, :, :qn_])
                            pend.append((pr8, c))

                        act_cs = [c for c in range(NKC // 2)
                                  if c not in poly_cs]
                        total = NKC // 2
                        held = []   # score tiles awaiting exp+AV (lookahead 1)

                        def flush_act(last=False):
                            sp2h, ch = held.pop(0)
                            pr8 = p_probs.tile([P, 2, 512], f8, tag="pr",
                                               name="pr")
                            act(pr8[:, :, :qn_], sp2h[:, :, :qn_], AF.Exp,
                                scale=SC_EXP)
                            av(pr8, ch, last=last)

                        for j, c in enumerate(act_cs):
                            if j % stride == 0 and poly_cs:
                                # drain previous poly, launch next
                                if pend:
                                    pr8p, cp = pend.pop(0)
                                    av(pr8p, cp)
                                nxt = min(poly_cs)
                                poly_cs.discard(nxt)
                                start_poly(nxt)
                            sp2 = sc2_g()
                            score_pair(sp2, c)
                            held.append((sp2, c))
                            if len(held) > 1:
                                flush_act()
                        while held:
                            flush_act(last=(n_emitted[0] == total - 1
                                            and not pend))
                        while pend:
                            pr8p, cp = pend.pop(0)
                            av(pr8p, cp, last=(n_emitted[0] == total - 1))
                        attn_finish(po_t, h_, q0, qn_, ot_strips)

            # ---- pools for the post-block0 phases (opened after kvfull frees,
            # closed at end of emission so repeat>1 can reopen) ----
            _lstack = contextlib.ExitStack()
            lpool = lambda name, bufs, **kw: _lstack.enter_context(
                tc.tile_pool(name=name, bufs=bufs, **kw))
            p_v = lpool("v", 1)          # V_ext [P, NTC, H, VD] fp8
            p_g = lpool("g", 2)          # gelu out [P, FFC, NT] fp8
            p_wfc1 = lpool("wfc1", 2)
            p_wfc2 = lpool("wfc2", 2)

            # ---------- MLP (ln2 + fc1 + gelu + fc2 + residual) ----------
            def mlp(i, x_strips, final=False):
                h2 = layernorm(x_strips, T, "h")
                ii = 2 * i if W_COMP_MLP else i
                wf1 = p_wfc1.tile([P, EC, FF], f8, tag="wfc1", name=f"wfc1_{i}")
                for ec in range(EC):
                    nc.sync.dma_start(wf1[:, ec, :], w1[ii][ec])
                wf2 = p_wfc2.tile([P, FFC, E], f8, tag="wfc2", name=f"wfc2_{i}")
                for fc in range(FFC):
                    nc.sync.dma_start(wf2[:, fc, :], w2[ii][fc])
                if W_COMP_MLP:
                    wf1l = p_wfc1.tile([P, EC, FF], f8, tag="wfc1l",
                                       name=f"wfc1l_{i}")
                    for ec in range(EC):
                        nc.sync.dma_start(wf1l[:, ec, :], w1[ii + 1][ec])
                    wf2l = p_wfc2.tile([P, FFC, E], f8, tag="wfc2l",
                                       name=f"wfc2l_{i}")
                    for fc in range(FFC):
                        nc.sync.dma_start(wf2l[:, fc, :], w2[ii + 1][fc])
                xdt = f32 if final else b16
                x_new = [p_x.tile([P, T], xdt, tag=f"x{s}", name=f"xm{i}_{s}")
                         for s in range(EC)]
                for nt in range(NQT):
                    o0 = nt * NT
                    g = p_g.tile([P, FFC, NT], f8, tag="g", name="g")
                    for fcp in range(FFC // 2):
                        ps2 = sc2_g()
                        for j in (0, 1):
                            fc = 2 * fcp + j
                            n_pass = 4 if W_COMP_MLP else 2
                            for k in range(n_pass):
                                wsrc = wf1 if k < 2 else wf1l
                                sp = k % 2
                                nc.tensor.matmul(
                                    ps2[:, j, :NT],
                                    wsrc[:, 2 * sp:2 * sp + 2,
                                         fc * P:(fc + 1) * P],
                                    h2[:, 2 * sp:2 * sp + 2, o0:o0 + NT],
                                    start=(k == 0), stop=(k == n_pass - 1),
                                    perf_mode=DR)
                        with nc.allow_low_precision(reason="fp8 gelu"):
                            if use_bias:
                                for j in (0, 1):
                                    act(g[:, 2 * fcp + j, :], ps2[:, j, :NT],
                                        AF.Gelu_apprx_tanh, scale=IWS,
                                        bias=bslice(i, "fc1", 2 * fcp + j))
                            else:
                                act(g[:, 2 * fcp:2 * fcp + 2, :],
                                    ps2[:, :, :NT],
                                    AF.Gelu_apprx_tanh, scale=IWS)
                    for oc in range(EC):
                        ps = mlp_r()
                        n_pass = FFC if W_COMP_MLP else FFC // 2
                        for k in range(n_pass):
                            wsrc = wf2 if k < FFC // 2 else wf2l
                            fp_ = k % (FFC // 2)
                            nc.tensor.matmul(
                                ps[:, :NT],
                                wsrc[:, 2 * fp_:2 * fp_ + 2,
                                     oc * P:(oc + 1) * P],
                                g[:, 2 * fp_:2 * fp_ + 2, :],
                                start=(k == 0), stop=(k == n_pass - 1),
                                perf_mode=DR)
                        if use_bias:
                            nc.vector.tensor_scalar(ps[:, :NT], ps[:, :NT],
                                                    bslice(i, "fc2", oc),
                                                    None, OP.add)
                        with nc.allow_low_precision(reason="bf16 residual"):
                            nc.vector.scalar_tensor_tensor(
                                x_new[oc][:, o0:o0 + NT], ps[:, :NT], IWS,
                                x_strips[oc][:, o0:o0 + NT], OP.mult, OP.add)
                return x_new

            mark("b0.proj+mlp")
            # block 0 out projection + residual + MLP
            x1 = [p_x.tile([P, T], b16, tag=f"x{s}", name=f"x1_{s}")
                  for s in range(EC)]

            def post_out0(oc, o, n, ps):
                with nc.allow_low_precision(reason="bf16 residual"):
                    nc.vector.scalar_tensor_tensor(
                        x1[oc][:, o:o + n], ps[:, :n], IWS, xs[oc][:, o:o + n],
                        OP.mult, OP.add)
            gemm8(wout, 0, EC, ot0, T, post_out0, bias_i=0, bias_grp="out")
            x1 = mlp(0, x1)

            # ==================================================================
            # BLOCKS 1, 2: banded attention
            # ==================================================================
            mark("banded")
            x_cur = x1
            for i in (1, 2):
                lower = (i == 1)
                mark(f"b{i}.ln1qkv")
                wqkv = p_wqkv.tile([P, EC, 3 * E], f8, tag="wqkv",
                                   name=f"wqkv{i}")
                for ec in range(EC):
                    nc.sync.dma_start(wqkv[:, ec, :], wq[i][ec])
                wout = p_wout.tile([P, EC, E], f8, tag="wout", name=f"wout{i}")
                for ec in range(EC):
                    nc.sync.dma_start(wout[:, ec, :], wo[i][ec])
                h1 = layernorm(x_cur, T, "h")
                qt_t = p_qt.tile([P, EC, T], f8, tag="qt", name=f"qt{i}")
                kt_t = p_kt.tile([P, EC, T], f8, tag="kt", name=f"kt{i}")
                v_t = p_v.tile([P, NTC, H, VD], f8, tag="v", name=f"v{i}")
                nc.vector.memset(v_t[:, :, :, HD:HD + 1], 1.0)
                nc.vector.memset(v_t[:, :, :, HD + 1:VD], 0.0)
                otb = p_ot.tile([P, EC, T], f8, tag="ot", name=f"ot{i}")
                ot_strips = [otb[:, s, :] for s in range(EC)]
                mark(f"b{i}.attn")
                # per query tile: groups of (kc0, n_kc, mask, window) where a
                # 2-group is a DoubleRow pair over the union window.
                # c-window tables (relative k-chunk c -> query window):
                WINS = ([(0, 15), (0, 143), (128, 143), (256, 128)] if lower
                        else [(0, 143), (113, 143), (241, 143), (369, 15)])

                def union(c0, c1):
                    lo = min(WINS[c0][0], WINS[c1][0])
                    hi = max(WINS[c0][0] + WINS[c0][1],
                             WINS[c1][0] + WINS[c1][1])
                    return (lo, hi - lo)

                def groups_for(qt):
                    # valid relative chunks: kc = 3qt + (c-1 | c)
                    cs = [c for c in range(4)
                          if 0 <= 3 * qt + (c - 1 if lower else c) < NTC]
                    kc0 = lambda c: 3 * qt + (c - 1 if lower else c)
                    bnd = "lo" if lower else "up"
                    gs = []
                    if cs == [0, 1, 2, 3]:
                        gs.append((kc0(0), 2, masks[f"m2_{bnd}01"], union(0, 1),
                                   (0, 1)))
                        gs.append((kc0(2), 2, masks[f"m2_{bnd}23"], union(2, 3),
                                   (2, 3)))
                    elif cs == [1, 2, 3]:
                        gs.append((kc0(1), 2, masks["m2_lo12"], union(1, 2),
                                   (1, 2)))
                        gs.append((kc0(3), 1, masks["m2_lo23"], WINS[3],
                                   (3, None)))
                    elif cs == [0, 1, 2]:
                        gs.append((kc0(0), 2, masks["m2_up01"], union(0, 1),
                                   (0, 1)))
                        gs.append((kc0(2), 1, masks["m2_up23"], WINS[2],
                                   (2, None)))
                    else:
                        raise AssertionError(cs)
                    return gs

                def emit_attn_qt(qt):
                    for h_ in range(H):
                        hp, hh = h_ // 2, h_ % 2
                        po_t = po_b()
                        po = po_t[0]
                        covered = []
                        gs = groups_for(qt)
                        sps = []
                        for (kcs, nk, mask_t, (qo, qw), cpair) in gs:
                            sp2 = sc2_g()
                            for ii in range(nk):
                                nc.tensor.matmul(
                                    sp2[:, ii, qo:qo + qw],
                                    kt_t[HD * hh:HD * (hh + 1), hp,
                                         (kcs + ii) * P:(kcs + ii + 1) * P],
                                    qt_t[HD * hh:HD * (hh + 1), hp,
                                         qt * NT + qo:qt * NT + qo + qw],
                                    start=True, stop=True)
                            sps.append(sp2)
                        for gi, (kcs, nk, mask_t, (qo, qw), cpair) in enumerate(gs):
                            sp2 = sps[gi]
                            pr8 = p_probs.tile([P, 2, 512], f8, tag="pr",
                                               name="pr")
                            act(pr8[:, :nk, qo:qo + qw], sp2[:, :nk, qo:qo + qw],
                                AF.Exp, scale=SC_EXP)
                            prm = p_probs.tile([P, 2, 512], f8, tag="prm",
                                               name="prm")
                            with nc.allow_low_precision(reason="fp8 probs"):
                                if nk == 2:
                                    nc.gpsimd.tensor_mul(
                                        prm[:, :2, qo:qo + qw],
                                        pr8[:, :2, qo:qo + qw],
                                        mask_t[:, :, qo:qo + qw])
                                else:
                                    # single: mask half cpair[0]%2 of the pair
                                    mi = cpair[0] % 2
                                    nc.gpsimd.tensor_mul(
                                        prm[:, 0, qo:qo + qw],
                                        pr8[:, 0, qo:qo + qw],
                                        mask_t[:, mi, qo:qo + qw])
                            # AV into po, split into fresh/accum column parts
                            parts = []
                            pos = qo
                            for (clo, chi) in covered + [(qo + qw, qo + qw)]:
                                if pos >= qo + qw:
                                    break
                                if chi <= pos:
                                    continue
                                if clo > pos:
                                    parts.append((pos, min(clo, qo + qw), True))
                                if clo < qo + qw:
                                    lo = max(clo, pos)
                                    hi = min(chi, qo + qw)
                                    if lo < hi:
                                        parts.append((lo, hi, False))
                                pos = max(pos, chi)
                            for (lo, hi, is_new) in parts:
                                if nk == 2:
                                    nc.tensor.matmul(
                                        po[:, lo:hi],
                                        v_t[:, kcs:kcs + 2, h_, :],
                                        prm[:, :2, lo:hi],
                                        start=is_new, stop=False,
                                        perf_mode=DR, skip_group_check=True)
                                else:
                                    nc.tensor.matmul(
                                        po[:, lo:hi],
                                        v_t[:, kcs, h_, :],
                                        prm[:, 0, lo:hi],
                                        start=is_new, stop=False,
                                        skip_group_check=True)
                            covered = sorted(covered + [(qo, qo + qw)])
                            merged = []
                            for (lo, hi) in covered:
                                if merged and lo <= merged[-1][1]:
                                    merged[-1] = (merged[-1][0],
                                                  max(hi, merged[-1][1]))
                                else:
                                    merged.append((lo, hi))
                            covered = merged
                        attn_finish(po_t, h_, qt * NT, NT, ot_strips)

                # emit qkv per token tile, with each attention query tile
                # interleaved as soon as its K/V columns exist (in-order
                # engine streams otherwise serialize attention behind the
                # whole projection)
                for nt in range(NQT):
                    o0 = nt * NT
                    for oc in range(EC):
                        ps = qkv_r()
                        for sp in range(EC // 2):
                            nc.tensor.matmul(
                                ps[:, :NT],
                                wqkv[:, 2 * sp:2 * sp + 2, oc * P:(oc + 1) * P],
                                h1[:, 2 * sp:2 * sp + 2, o0:o0 + NT],
                                start=(sp == 0), stop=(sp == EC // 2 - 1),
                                perf_mode=DR)
                        if use_bias:
                            nc.vector.tensor_scalar(ps[:, :NT], ps[:, :NT],
                                                    bslice(i, "qkv", oc),
                                                    None, OP.add)
                        psum_to_sbuf8(qt_t[:, oc, o0:o0 + NT], ps[:, :NT],
                                      eng="act")
                        ps = qkv_r()
                        for sp in range(EC // 2):
                            nc.tensor.matmul(
                                ps[:, :NT],
                                wqkv[:, 2 * sp:2 * sp + 2,
                                     E + oc * P:E + (oc + 1) * P],
                                h1[:, 2 * sp:2 * sp + 2, o0:o0 + NT],
                                start=(sp == 0), stop=(sp == EC // 2 - 1),
                                perf_mode=DR)
                        if use_bias:
                            nc.vector.tensor_scalar(ps[:, :NT], ps[:, :NT],
                                                    bslice(i, "qkv", EC + oc),
                                                    None, OP.add)
                        psum_to_sbuf8(kt_t[:, oc, o0:o0 + NT], ps[:, :NT],
                                      eng="act")
                    for tck in range(3 * nt, 3 * nt + 3):
                        ps = qkv_r()
                        for sp in range(EC // 2):
                            nc.tensor.matmul(
                                ps[:],
                                h1[:, 2 * sp:2 * sp + 2,
                                   tck * P:(tck + 1) * P],
                                wqkv[:, 2 * sp:2 * sp + 2, 2 * E:3 * E],
                                start=(sp == 0), stop=(sp == EC // 2 - 1),
                                perf_mode=DR)
                        if use_bias:
                            add_vbias(i, ps)
                        with nc.allow_low_precision(reason="fp8 V"):
                            act(v_t[:, tck, :, 0:HD],
                                ps[:].rearrange("p (h d) -> p h d", h=H),
                                AF.Copy, scale=IWS)
                    aq = nt if lower else nt - 1
                    if aq >= 0:
                        emit_attn_qt(aq)
                if not lower:
                    emit_attn_qt(NQT - 1)
                mark(f"b{i}.projmlp")
                x_new = [p_x.tile([P, T], b16, tag=f"x{s}", name=f"xa{i}_{s}")
                         for s in range(EC)]

                def post_out(oc, o, n, ps, x_new=x_new, x_cur=x_cur):
                    with nc.allow_low_precision(reason="bf16 residual"):
                        nc.vector.scalar_tensor_tensor(
                            x_new[oc][:, o:o + n], ps[:, :n], IWS,
                            x_cur[oc][:, o:o + n], OP.mult, OP.add)
                gemm8(wout, 0, EC, otb, T, post_out, bias_i=i, bias_grp="out")
                x_cur = mlp(i, x_new, final=(i == 2))

            mark("out")
            # output
            for s in range(EC):
                nc.sync.dma_start(yT[P * s:P * (s + 1), :], x_cur[s][:])
            _lstack.close()

        for _rep in range(repeat):
            _emit_once()

        # record build-order instruction -> phase map (before scheduling)
        _PHASE_OF.clear()
        names = [ins.name for bb in nc.main_func.blocks for ins in bb.instructions]
        bounds = [n for _, n in _PHASE_MARKS]
        labels = [l for l, _ in _PHASE_MARKS]
        import bisect as _bis
        for idx, nm in enumerate(names):
            j = _bis.bisect_right(bounds, idx) - 1
            _PHASE_OF[nm] = labels[j] if j >= 0 else "pre"

    nc.compile()
    return nc


# ------------------------------------------------------------------
# cached executor (compile once, run many)
# ------------------------------------------------------------------
class _Exec:
    def __init__(self, use_bias: bool):
        import jax
        import concourse.mybir as mybir
        from concourse import bass2jax
        from concourse.bass2jax import install_neuronx_cc_hook, _bass_exec_p
        from jax.sharding import Mesh, PartitionSpec
        from jax.experimental.shard_map import shard_map

        install_neuronx_cc_hook()
        nc = _build_nc(use_bias)
        self.nc = nc

        part_name = (nc.partition_id_tensor.name
                     if nc.partition_id_tensor is not None else None)
        in_names, out_names, out_avals = [], [], []
        self.zero_shapes = []
        for alloc in nc.m.functions[0].allocations:
            if not isinstance(alloc, mybir.MemoryLocationSet):
                continue
            name = alloc.memorylocations[0].name
            if alloc.kind == "ExternalInput":
                if name != part_name:
                    in_names.append(name)
            elif alloc.kind == "ExternalOutput":
                out_names.append(name)
                shape = tuple(alloc.tensor_shape)
                dtype = mybir.dt.np(alloc.dtype)
                out_avals.append(jax.core.ShapedArray(shape, dtype))
                self.zero_shapes.append((shape, dtype))
        n_params = len(in_names)
        all_in = in_names + out_names
        if part_name is not None:
            all_in = all_in + [part_name]
        self.in_names = in_names
        self.out_names = out_names
        n_outs = len(out_names)

        def _body(*args):
            operands = list(args)
            if part_name is not None:
                operands.append(bass2jax.partition_id_tensor())
            outs = _bass_exec_p.bind(
                *operands,
                out_avals=tuple(out_avals),
                in_names=tuple(all_in),
                out_names=tuple(out_names),
                lowering_input_output_aliases=(),
                sim_require_finite=True,
                sim_require_nnan=True,
                nc=nc,
            )
            return tuple(outs)
        self._body = _body

        devices = jax.devices()[:N_CORES]
        mesh = Mesh(np.asarray(devices), ("core",))
        in_specs = (PartitionSpec("core"),) * (n_params + n_outs)
        out_specs = (PartitionSpec("core"),) * n_outs
        donate = tuple(range(n_params, n_params + n_outs))
        self.fn = jax.jit(
            shard_map(_body, mesh=mesh, in_specs=in_specs,
                      out_specs=out_specs, check_rep=False),
            donate_argnums=donate, keep_unused=True)
        self.out_avals = out_avals

    def bench(self, in_maps, iters=10):
        """device-resident-input timing: returns per-iteration seconds"""
        import time
        import jax
        from jax.sharding import Mesh, PartitionSpec, NamedSharding
        if not hasattr(self, "_bench_fn"):
            from jax.experimental.shard_map import shard_map
            devices = jax.devices()[:N_CORES]
            mesh = Mesh(np.asarray(devices), ("core",))
            n_in = len(self.in_names) + len(self.zero_shapes)
            self._bench_fn = jax.jit(
                shard_map(self._body, mesh=mesh,
                          in_specs=(PartitionSpec("core"),) * n_in,
                          out_specs=(PartitionSpec("core"),) * len(self.out_names),
                          check_rep=False),
                keep_unused=True)
            self._bench_sharding = NamedSharding(mesh, PartitionSpec("core"))
        concat_in = [
            np.concatenate([np.asarray(in_maps[c][n]) for c in range(N_CORES)], axis=0)
            for n in self.in_names
        ] + [np.zeros((N_CORES * s[0], *s[1:]), d) for (s, d) in self.zero_shapes]
        import jax
        dev_in = [jax.device_put(a, self._bench_sharding) for a in concat_in]
        out = jax.block_until_ready(self._bench_fn(*dev_in))  # warm/compile
        t0 = time.time()
        for _ in range(iters):
            out = self._bench_fn(*dev_in)
        jax.block_until_ready(out)
        return (time.time() - t0) / iters

    def run(self, in_maps):
        """in_maps: list of 8 dicts name->np.ndarray. returns list of dicts"""
        concat_in = [
            np.concatenate([np.asarray(in_maps[c][n]) for c in range(N_CORES)], axis=0)
            for n in self.in_names
        ]
        concat_zeros = [np.zeros((N_CORES * s[0], *s[1:]), d)
                        for (s, d) in self.zero_shapes]
        outs = self.fn(*concat_in, *concat_zeros)
        import jax
        outs = jax.block_until_ready(outs)
        res = []
        for c in range(N_CORES):
            d = {}
            for idx, n in enumerate(self.out_names):
                shp = self.out_avals[idx].shape
                d[n] = np.asarray(outs[idx]).reshape(N_CORES, *shp)[c]
            res.append(d)
        return res


def _get_exec(use_bias: bool) -> "_Exec":
    key = bool(use_bias)
    if key not in _EXEC_CACHE:
        _EXEC_CACHE[key] = _Exec(key)
    return _EXEC_CACHE[key]


# ------------------------------------------------------------------
# host-side input prep
# ------------------------------------------------------------------
def _band_masks():
    ki = np.arange(P)[:, None]
    qi = np.arange(NT)[None, :]
    m = {}
    for c in range(4):
        # lower band, k-chunk kc = 3t + (c-1):  0 <= (i-k) <= W-1 with
        # i-k = qi - ki + 128*(1-c)
        d = qi - ki + P * (1 - c)
        m[f"lo{c}"] = ((d >= 0) & (d <= W_BAND - 1)).astype(bf16)
        # upper band, k-chunk kc = 3t + c:  0 <= (k-i) <= W-1 with
        # k-i = ki - qi + 128*c
        d = ki - qi + P * c
        m[f"up{c}"] = ((d >= 0) & (d <= W_BAND - 1)).astype(bf16)
    out = {}
    for bnd in ("lo", "up"):
        for (nm, a, b) in (("01", 0, 1), ("23", 2, 3)):
            out[f"m2_{bnd}{nm}"] = np.ascontiguousarray(
                np.stack([m[f"{bnd}{a}"], m[f"{bnd}{b}"]], axis=1))
    out["m2_lo12"] = np.ascontiguousarray(
        np.stack([m["lo1"], m["lo2"]], axis=1))
    return out


def kernel(x, ln1_w, ln1_b, ln2_w, ln2_b, qkv_w, qkv_b, out_w, out_b,
           fc1_w, fc1_b, fc2_w, fc2_b):
    x = np.asarray(x, np.float32)
    ln1_w = np.asarray(ln1_w, np.float32); ln1_b = np.asarray(ln1_b, np.float32)
    ln2_w = np.asarray(ln2_w, np.float32); ln2_b = np.asarray(ln2_b, np.float32)
    qkv_w = np.asarray(qkv_w, np.float32); qkv_b = np.asarray(qkv_b, np.float32)
    out_w = np.asarray(out_w, np.float32); out_b = np.asarray(out_b, np.float32)
    fc1_w = np.asarray(fc1_w, np.float32); fc1_b = np.asarray(fc1_b, np.float32)
    fc2_w = np.asarray(fc2_w, np.float32); fc2_b = np.asarray(fc2_b, np.float32)

    # fold LN affine into following projection weights (exact):
    #   h = z*w + b  =>  h @ Wt.T = z @ (W*w).T + b @ W.T
    wq_f, wo_f, w1_f, w2_f = [], [], [], []
    biases = []
    for i in range(3):
        qw = qkv_w[i] * ln1_w[i][None, :]
        qb = qkv_b[i] + qkv_w[i] @ ln1_b[i]
        f1 = fc1_w[i] * ln2_w[i][None, :]
        f1b = fc1_b[i] + fc1_w[i] @ ln2_b[i]
        wq_f.append(qw); w1_f.append(f1)
        wo_f.append(out_w[i]); w2_f.append(fc2_w[i])
        # packed per-feature bias tile [P, 32] (q, k, out, fc1, fc2) + v row.
        # q/k/out/fc2 biases ride the x64-scaled psum; fc1 bias is consumed
        # by the gelu activation in true units.
        bias_pack = np.zeros((P, 32), np.float32)
        bias_pack[:, 0:8] = WS * qb[:2 * E].reshape(8, P).T
        bias_pack[:, 8:12] = WS * out_b[i].reshape(4, P).T
        bias_pack[:, 12:28] = f1b.reshape(16, P).T
        bias_pack[:, 28:32] = WS * fc2_b[i].reshape(4, P).T
        biases.append((bias_pack, WS * qb[2 * E:].reshape(1, E)))
    use_bias = any(np.abs(b).max() > 0 or np.abs(np.asarray(v, np.float32)).max() > 0
                   for b, v in biases)

    ex = _get_exec(use_bias)

    masks = _band_masks()
    # weight tensors: x64 pre-scale into fp8e4, [chunk, P, out] layout
    weights = {}
    for i in range(3):
        weights[f"qkvT{i}"] = np.ascontiguousarray(
            (WS * wq_f[i].T).reshape(EC, P, 3 * E)).astype(f8e4)
        weights[f"outT{i}"] = np.ascontiguousarray(
            (WS * wo_f[i].T).reshape(EC, P, E)).astype(f8e4)
        w1s = np.ascontiguousarray((WS * w1_f[i].T).reshape(EC, P, FF))
        w2s = np.ascontiguousarray((WS * w2_f[i].T).reshape(FFC, P, E))
        weights[f"fc1T{i}"] = w1s.astype(f8e4)
        weights[f"fc2T{i}"] = w2s.astype(f8e4)
        if W_COMP_MLP:
            weights[f"fc1L{i}"] = (
                w1s - weights[f"fc1T{i}"].astype(np.float32)).astype(f8e4)
            weights[f"fc2L{i}"] = (
                w2s - weights[f"fc2T{i}"].astype(np.float32)).astype(f8e4)
        if use_bias:
            weights[f"bias{i}"] = biases[i][0]
            weights[f"vbias{i}"] = biases[i][1].astype(bf16)

    in_maps = []
    for core in range(N_CORES):
        b, c = divmod(core, CHUNKS)
        e0 = EXT_STARTS[c]
        xT_ext = np.ascontiguousarray(x[b, e0:e0 + T, :].T).astype(bf16)
        xfT = np.ascontiguousarray(x[b].T).astype(bf16)
        m = {"xT": xT_ext, "xfT": xfT, **weights, **masks}
        in_maps.append(m)

    res = ex.run(in_maps)

    out = np.empty((B, S, E), np.float32)
    for core in range(N_CORES):
        b, c = divmod(core, CHUNKS)
        yT_ = res[core]["yT"]               # [E, T]
        off = OWN_OFF[c]
        out[b, OWN * c:OWN * (c + 1), :] = yT_[:, off:off + OWN].T
    return out


# revision 21
# speedup vs baseline: 1.2496x; 1.0285x over previous
"""Trainium2 Bass kernel for nn_AlternateAttention (3-block transformer:
global attention, lower-band attention, upper-band attention, each with MLP).

Sharding: 8 cores = 2 batches x 4 sequence chunks of 1024 tokens, each core
processing an extended window of 1152 tokens (64-token halo each side) so the
banded blocks need no inter-core communication. Block 0 (global attention)
needs full-sequence K/V; each core recomputes them from the replicated raw
input (LN1+KV projection over the full 4096 tokens of its batch).

Layout: activations live TRANSPOSED in SBUF ([feature, token]) so projections
are natural matmuls. All projection weights are cast to fp8e4 (scaled x64 so
N(0,0.02) weights stay in the normal range) and GEMMs run in DoubleRow perf
mode (two 128-deep k-tiles per instruction at 0.5 cycles/row).  The 1/64
de-scale is folded into the psum->sbuf consumer (activation scale or
scalar_tensor_tensor) -- for attention scores both Q and K carry x64 so the
softmax exp scale absorbs the 4096.  Attention probabilities and V are fp8
so AV also runs DoubleRow; exp reads score PAIRS from 2-bank [P,2,512] PSUM
tiles to halve the Activation-engine fixed overhead.
"""
import contextlib
import numpy as np
import ml_dtypes

# ---- problem constants (hardcoded per contract) ----
B, S, E, H, W_BAND, FF = 2, 4096, 512, 8, 16, 2048
HD = E // H                      # 64
N_CORES = 8
CHUNKS = 4                       # sequence chunks per batch
OWN = S // CHUNKS                # 1024
HALO = 64
T = OWN + 2 * HALO               # 1152 = 9*128
P = 128
NT = 384                         # token tile (3 per T)
NQT = T // NT                    # 3
NKC = S // P                     # 32 k-chunks for global attention
NTC = T // P                     # 9 token chunks of ext window
EC = E // P                      # 4 feature strips
FFC = FF // P                    # 16
VD = HD + 2                      # V tile row pitch: 66 so DoubleRow pair stride (H*VD) % 16 == 0

WS = 64.0                        # weight pre-scale into fp8
POLY_PER = 4                     # DVE-exp'd score pairs per (head, q-chunk)
IWS = 1.0 / WS
SC_EXP = 0.125 / (WS * WS)       # exp scale absorbing q/k weight scales

EXT_STARTS = [max(0, min(OWN * c - HALO, S - T)) for c in range(CHUNKS)]
OWN_OFF = [OWN * c - EXT_STARTS[c] for c in range(CHUNKS)]

_EXEC_CACHE = {}
_PHASE_MARKS = []
_PHASE_OF = {}

bf16 = ml_dtypes.bfloat16
f8e4 = ml_dtypes.float8_e4m3


# ------------------------------------------------------------------
# device program
# ------------------------------------------------------------------
def _patch_act_tables():
    # The bacc table-load placement maps exp -> 'exp_and_others' and
    # ln -> 'natural_log', thrashing the ACT table RAM (~1.3us per switch,
    # dozens of switches). Restrict the choice to the two sets that cover
    # everything this kernel uses so exp/ln never evict each other.
    import concourse.hw_specs as hw_specs
    import concourse.bacc as bacc_mod
    import concourse.bass_interp as bass_interp
    if getattr(hw_specs, "_aa_patched", False):
        return
    orig = hw_specs.get_activation_tables
    keep = {"natural_log_exp_and_others", "gelu_apprx_tanh_and_others"}

    def _gat(arch):
        tabs = orig(arch)
        return {k: (v if k in keep else set()) for k, v in tabs.items()}

    hw_specs.get_activation_tables = _gat
    bacc_mod.get_activation_tables = _gat
    bass_interp.get_activation_tables = _gat
    hw_specs._aa_patched = True


def _build_nc(use_bias: bool, repeat: int = 1):
    import concourse.bacc as bacc
    import concourse.mybir as mybir
    import concourse.tile as tile

    _patch_act_tables()

    f32 = mybir.dt.float32
    b16 = mybir.dt.bfloat16
    f8 = mybir.dt.float8e4
    AF = mybir.ActivationFunctionType
    OP = mybir.AluOpType
    DR = mybir.MatmulPerfMode.DoubleRow

    nc = bacc.Bacc("TRN2", target_bir_lowering=False, debug=False,
                   num_devices=N_CORES)
    _PHASE_MARKS.clear()

    def mark(label):
        n = sum(len(b.instructions) for b in nc.main_func.blocks)
        _PHASE_MARKS.append((label, n))

    # ---- dram tensors ----
    xT = nc.dram_tensor("xT", [E, T], b16, kind="ExternalInput")
    xfT = nc.dram_tensor("xfT", [E, S], b16, kind="ExternalInput")
    wq, wo, w1, w2, bias_d = [], [], [], [], []
    for i in range(3):
        wq.append(nc.dram_tensor(f"qkvT{i}", [P, EC * 3 * E], f8, kind="ExternalInput"))
        wo.append(nc.dram_tensor(f"outT{i}", [P, EC * E], f8, kind="ExternalInput"))
        w1.append(nc.dram_tensor(f"fc1T{i}", [P, EC * FF], f8, kind="ExternalInput"))
        w2.append(nc.dram_tensor(f"fc2T{i}", [P, FFC * E], f8, kind="ExternalInput"))
        if W_COMP_MLP:
            w1.append(nc.dram_tensor(f"fc1L{i}", [P, EC * FF], f8, kind="ExternalInput"))
            w2.append(nc.dram_tensor(f"fc2L{i}", [P, FFC * E], f8, kind="ExternalInput"))
        if use_bias:
            # packed per-feature biases for transposed-layout outputs:
            # [q(4xP) | k(4xP) | out(4xP) | fc1(16xP) | fc2(4xP)] -> [P, 32]
            bias_d.append(nc.dram_tensor(f"bias{i}", [P, 32], f32, kind="ExternalInput"))
            bias_d.append(nc.dram_tensor(f"vbias{i}", [1, E], b16, kind="ExternalInput"))
    # paired 0/1 bf16 stripe-mask tiles for banded attention: [P, 2, NT]
    mask_d = {}
    for nm in ("m2_lo01", "m2_lo23", "m2_lo12", "m2_up01", "m2_up23"):
        mask_d[nm] = nc.dram_tensor(nm, [P, 2, NT], b16, kind="ExternalInput")
    yT = nc.dram_tensor("yT", [E, T], f32, kind="ExternalOutput")

    from concourse.tile import add_dep_helper as _adh

    with tile.TileContext(nc) as tc, contextlib.ExitStack() as ctx:
        pool = lambda name, bufs, **kw: ctx.enter_context(
            tc.tile_pool(name=name, bufs=bufs, **kw))

        # Order ACT instructions across table-set boundaries (exp/ln vs gelu)
        # so the activation-table RAM isn't thrashed (~1.3us per reload).
        # Within a set no ordering is imposed.
        _act_sets = {AF.Gelu_apprx_tanh: 1}
        _prev_set = [None]
        _prev_insts = [[]]
        _cur_insts = [[]]

        def act(out, in_, func, **kw):
            inst = nc.scalar.activation(out, in_, func, **kw)
            if func == AF.Copy:
                return inst
            s = _act_sets.get(func, 0)
            if _prev_set[0] is None:
                _prev_set[0] = s
            if s != _prev_set[0]:
                _prev_insts[0] = _cur_insts[0][-64:]
                _cur_insts[0] = []
                _prev_set[0] = s
            for p in _prev_insts[0]:
                _adh(inst.ins, p.ins, sync=True, reason="act-table batch order")
            _cur_insts[0].append(inst)
            return inst

        # ---- pools live for the whole kernel ----
        p_x = pool("x", 2)           # residual strips f32, tags x0..x3
        p_h = pool("h", 2)           # LN output fp8 [P, EC, Tn]
        p_sq = pool("sq", 4)         # per-nt squares bf16
        p_qt = pool("qt", 1)         # QT [P, EC, T] fp8
        p_kt = pool("kt", 1)         # KT (banded) [P, EC, T] fp8
        p_ot = pool("ot", 1)         # attention out strips fp8
        p_wqkv = pool("wqkv", 2)     # qkv weights [P, EC, 3E] fp8
        p_wout = pool("wout", 2)
        p_probs = pool("probs", 4)   # exp outputs fp8 [P, 2, 512]
        p_tmp = pool("tmp", 2)       # temporaries
        p_sm = pool("sm", 2)         # small [1, n] stat vectors
        p_c = pool("const", 1)       # ones, masks, biases
        # PSUM: 8 banks = two 2-bank score slots + four single banks
        p_ps = pool("ps", 1, space="PSUM")
        # sA/sB: [P,2,512] paired attention scores (also misc gemm accum)
        # p0..p3: po accumulators / LN stats / broadcasts / gemm accum

        ones = p_c.tile([P, P], b16)
        nc.vector.memset(ones[:], 1.0)
        eps1 = p_c.tile([1, 1], f32)
        nc.vector.memset(eps1[:], 1e-5)
        masks = {}
        for nm, d in mask_d.items():
            mt = p_c.tile([P, 2, NT], b16, tag=f"mask_{nm}", name=f"mask_{nm}")
            nc.sync.dma_start(mt[:], d[:])
            masks[nm] = mt
        bias_t, vbias_t = [], []
        if use_bias:
            for i in range(3):
                bt = p_c.tile([P, 32], f32, tag=f"bias{i}", name=f"bias{i}")
                nc.sync.dma_start(bt[:], bias_d[2 * i][:])
                bias_t.append(bt)
                vt = p_c.tile([1, E], b16, tag=f"vbias{i}", name=f"vbias{i}")
                nc.sync.dma_start(vt[:], bias_d[2 * i + 1][:])
                vbias_t.append(vt)

        def bslice(i, group, oc):
            base = {"qkv": 0, "out": 8, "fc1": 12, "fc2": 28}[group]
            return bias_t[i][:, base + oc:base + oc + 1]

        def add_vbias(i, ps):
            # V projection output is in normal layout [token, feat]: bias
            # varies along the free dim -> broadcast with a K=1 matmul.
            vb = p_ps.tile([P, 512], f32, tag="p0", name="vb")
            nc.tensor.matmul(vb[:], ones[0:1, :], vbias_t[i][:],
                             start=True, stop=True)
            vbs = p_tmp.tile([P, 512], f32, tag="vbs", name="vbs")
            nc.vector.tensor_copy(vbs[:], vb[:])
            nc.vector.tensor_add(ps[:], ps[:], vbs[:])

        def _emit_once():
            # load residual strips
            xs = []
            for s in range(EC):
                t = p_x.tile([P, T], b16, tag=f"x{s}", name=f"xin{s}")
                nc.sync.dma_start(t[:], xT[P * s:P * (s + 1), :])
                xs.append(t)

            # ---------- layernorm ----------
            def layernorm(x_strips, Tn, htag,
                          stat_tags=("sA",), bc_tags=("sB", "sC"),
                          sq_eng=None):
                """x_strips: 4 bf16 strips [P, Tn] -> fp8 [P,EC,Tn].
                Stats via ones-matmul partition reductions (bf16), squares on
                GPSIMD (SBUF->SBUF), per-token mean/rstd broadcast via K=1
                matmuls, normalize on DVE in 2x bf16 mode, fp8 output."""
                sq_eng = sq_eng or nc.gpsimd
                h8 = p_h.tile([P, EC, Tn], f8, tag=htag, name=htag)
                step = 512 if Tn % 512 == 0 else NT
                nss = [(k * step, min(step, Tn - k * step))
                       for k in range((Tn + step - 1) // step)]
                for si, (o, n) in enumerate(nss):
                    sl = slice(o, o + n)
                    s12 = p_ps.tile([1, 1024], f32, tag=stat_tags[0], name="s12")
                    s1 = s12[:, 0:512]
                    s2 = s12[:, 512:1024]
                    for s in range(EC):
                        rhs_x = x_strips[s][:, sl]
                        nc.tensor.matmul(s1[:, :n], ones[:, 0:1], rhs_x,
                                         start=(s == 0), stop=(s == EC - 1))
                        sq_nt = p_sq.tile([P, 512], b16, tag="sqn", name="sqn")
                        sq_eng.tensor_mul(sq_nt[:, :n], rhs_x, rhs_x)
                        nc.tensor.matmul(s2[:, :n], ones[:, 0:1], sq_nt[:, :n],
                                         start=(s == 0), stop=(s == EC - 1))
                    with nc.allow_low_precision(reason="bf16 LN stats"):
                        m_b = p_sm.tile([1, 512], b16, tag="m_b", name="m_b")
                        nc.vector.tensor_scalar(m_b[:, :n], s1[:, :n], 1.0 / E,
                                                None, OP.mult)
                        stt = p_sm.tile([1, 1024], b16, tag="stt", name="stt")
                        sa, sb = stt[:, 0:n], stt[:, 512:512 + n]
                        nc.vector.tensor_scalar(sa, s2[:, :n], 1.0 / E, None,
                                                OP.mult)
                        nc.vector.tensor_mul(sb, m_b[:, :n], m_b[:, :n])
                        nc.vector.tensor_sub(sa, sa, sb)
                        act(sb, sa, AF.Ln, bias=eps1[:])
                        lnv = sb
                        r_b = p_sm.tile([1, 512], b16, tag="r_b", name="r_b")
                        act(r_b[:, :n], lnv, AF.Exp, scale=-0.5)
                    bc = p_ps.tile([P, 2, 512], f32,
                                   tag=bc_tags[si % len(bc_tags)], name="bc")
                    mB, rB = bc[:, 0, :], bc[:, 1, :]
                    nc.tensor.matmul(mB[:, :n], ones[0:1, :], m_b[:, :n],
                                     start=True, stop=True)
                    nc.tensor.matmul(rB[:, :n], ones[0:1, :], r_b[:, :n],
                                     start=True, stop=True)
                    mBs = p_tmp.tile([P, 512], b16, tag="mBs", name="mBs")
                    act(mBs[:, :n], mB[:, :n], AF.Copy)
                    rBs = p_tmp.tile([P, 512], b16, tag="rBs", name="rBs")
                    act(rBs[:, :n], rB[:, :n], AF.Copy)
                    with nc.allow_low_precision(reason="fp8 GEMM inputs"):
                        for s in range(EC):
                            t0 = p_tmp.tile([P, 512], b16, tag="lnt", name="lnt")
                            nc.vector.tensor_sub(t0[:, :n], x_strips[s][:, sl],
                                                 mBs[:, :n])
                            nc.vector.tensor_mul(h8[:, s, sl], t0[:, :n],
                                                 rBs[:, :n])
                return h8

            # ---------- PSUM rotors ----------
            def rotor(tags, shape, name, tagged=False):
                st = [0]

                def get():
                    t = tags[st[0] % len(tags)]
                    st[0] += 1
                    tl = p_ps.tile(shape, f32, tag=t, name=name)
                    return (tl, t) if tagged else tl
                return get

            sc2_g = rotor(("sA", "sB"), [P, 2, 512], "sc2")   # paired scores
            sc2_c = rotor(("sC",), [P, 2, 512], "sc2c")       # poly-exp pairs
            po_g = rotor(("p0", "p1"), [VD, 512], "po", tagged=True)
            po_b = rotor(("p0", "p1"), [VD, 512], "po", tagged=True)
            gem_r = rotor(("sA", "sB"), [P, 512], "gps")      # non-attn phases
            qkv_r = rotor(("sC",), [P, 512], "qps")           # banded qkv
            mlp_r = rotor(("sC", "p0", "p1"), [P, 512], "mps")
            kv_r = rotor(("p0", "p1"), [P, 512], "kvps")

            # ---------- DoubleRow GEMM (transposed layout) ----------
            def gemm8(w8, col0, n_oc, h8, Tn, post, ps_get=None, bias_i=None,
                      bias_grp=None):
                ps_get = ps_get or gem_r
                step = 512 if Tn % 512 == 0 else NT
                nss = [(k * step, min(step, Tn - k * step))
                       for k in range((Tn + step - 1) // step)]
                for (o, n) in nss:
                    for oc in range(n_oc):
                        ps = ps_get()
                        for sp in range(EC // 2):
                            nc.tensor.matmul(
                                ps[:, :n],
                                w8[:, 2 * sp:2 * sp + 2,
                                   col0 + oc * P:col0 + (oc + 1) * P],
                                h8[:, 2 * sp:2 * sp + 2, o:o + n],
                                start=(sp == 0), stop=(sp == EC // 2 - 1),
                                perf_mode=DR)
                        if use_bias and bias_grp is not None:
                            nc.vector.tensor_scalar(
                                ps[:, :n], ps[:, :n],
                                bslice(bias_i, bias_grp, oc), None, OP.add)
                        post(oc, o, n, ps)

            def psum_to_sbuf8(dst_ap, ps_ap, eng="dve", scale=None):
                """copy matmul accumulator to sbuf (fp8/bf16 out)"""
                with nc.allow_low_precision(reason="fp8 GEMM inputs"):
                    if eng == "act":
                        act(dst_ap, ps_ap, AF.Copy,
                            scale=1.0 if scale is None else scale)
                    elif scale is None:
                        nc.vector.tensor_copy(dst_ap, ps_ap)
                    else:
                        nc.vector.tensor_scalar(dst_ap, ps_ap, scale, None,
                                                OP.mult)

            # ---------- DVE exp via repeated squaring:
            # exp(s*x) ~ (1 + y + y^2/2)^8 with y = s*x/8  (rel err < 1% for
            # |s*x| <= 1.8).  Uses only tensor_scalar / tensor_tensor, which
            # run in 2x/4x DVE modes on bf16 -- offloads softmax exp from the
            # saturated Activation engine.

            def poly_exp(dst_ap, ps_ap):
                y = p_tmp.tile([P, 2, 512], b16, tag="pey", name="pey")
                u = p_tmp.tile([P, 2, 512], b16, tag="peu", name="peu")
                n2 = ps_ap.shape[-1]
                ya = y[:, :, :n2]
                ua = u[:, :, :n2]
                with nc.allow_low_precision(reason="poly exp"):
                    # t = y/sqrt(2) + 1/sqrt(2);  z = t^2 + 0.5 = 1 + y + y^2/2
                    nc.vector.tensor_scalar(ya, ps_ap, SC_EXP / 8 * 0.70710678,
                                            0.70710678, OP.mult, OP.add)
                    nc.vector.tensor_mul(ua, ya, ya)
                    nc.vector.tensor_scalar(ua, ua, 0.5, None, OP.add)
                    nc.vector.tensor_mul(ua, ua, ua)
                    nc.vector.tensor_mul(ua, ua, ua)
                    nc.vector.tensor_mul(dst_ap, ua, ua)

            # ---------- attention finish (shared) ----------
            def attn_finish(po_t, h_, qt0, qn, ot_strips):
                po, po_tag = po_t
                hp, hh = h_ // 2, h_ % 2
                ou = p_tmp.tile([VD, 512], b16, tag="ou", name="ou")
                with nc.allow_low_precision(reason="bf16 softmax normalizer"):
                    nc.vector.tensor_copy(ou[:, :qn], po[:, :qn])  # frees bank
                    linv = p_sm.tile([1, 512], b16, tag="linv", name="linv")
                    nc.vector.reciprocal(linv[:, :qn], ou[HD:HD + 1, :qn])
                lB = p_ps.tile([P, 512], f32, tag=po_tag, name="lB")
                nc.tensor.matmul(lB[:HD, :qn], ones[0:1, :HD], linv[:, :qn],
                                 start=True, stop=True)
                with nc.allow_low_precision(reason="fp8 GEMM inputs"):
                    nc.vector.tensor_mul(
                        ot_strips[hp][HD * hh:HD * (hh + 1), qt0:qt0 + qn],
                        ou[:HD, :qn], lB[:HD, :qn])

            # ==================================================================
            # BLOCK 0: global attention
            # ==================================================================
            wqkv = p_wqkv.tile([P, EC, 3 * E], f8, tag="wqkv", name="wqkv0")
            nc.sync.dma_start(wqkv[:], wq[0][:])
            wout = p_wout.tile([P, EC, E], f8, tag="wout", name="wout0")
            nc.sync.dma_start(wout[:], wo[0][:])

            mark("b0.ln1+q")
            h1 = layernorm(xs, T, "h")
            # Q projection (ext window)
            qt_t = p_qt.tile([P, EC, T], f8, tag="qt", name="qt0")
            gemm8(wqkv, 0, EC, h1, T,
                  lambda oc, o, n, ps: psum_to_sbuf8(qt_t[:, oc, o:o + n],
                                                     ps[:, :n], eng="act"),
                  bias_i=0, bias_grp="qkv")

            mark("b0.kv")
            ot0 = p_ot.tile([P, EC, T], f8, tag="ot", name="ot0")
            ot_strips = [ot0[:, s, :] for s in range(EC)]
            with tc.tile_pool(name="kvfull", bufs=1) as p_kv, \
                 tc.tile_pool(name="xpan", bufs=2 if not use_bias else 1) as p_xp:
                ktf = p_kv.tile([P, EC, S], f8, tag="ktf", name="ktf")
                vf = p_kv.tile([P, NKC, H, VD], f8, tag="vf", name="vf")
                nc.vector.memset(vf[:, :, :, HD:HD + 1], 1.0)
                nc.vector.memset(vf[:, :, :, HD + 1:VD], 0.0)
                PAN = 1024
                for pan in range(S // PAN):
                    xp = p_xp.tile([P, EC, PAN], b16, tag="xp", name="xp")
                    for s in range(EC):
                        nc.sync.dma_start(xp[:, s, :],
                                          xfT[P * s:P * (s + 1), PAN * pan:PAN * (pan + 1)])
                    hp8 = layernorm([xp[:, s, :] for s in range(EC)], PAN,
                                    "hp")
                    # K^T columns for this panel
                    for half in range(PAN // 512):
                        hsl = slice(512 * half, 512 * (half + 1))
                        for oc in range(EC):
                            ps = kv_r()
                            for sp in range(EC // 2):
                                nc.tensor.matmul(
                                    ps[:],
                                    wqkv[:, 2 * sp:2 * sp + 2,
                                         E + oc * P:E + (oc + 1) * P],
                                    hp8[:, 2 * sp:2 * sp + 2, hsl],
                                    start=(sp == 0), stop=(sp == EC // 2 - 1),
                                    perf_mode=DR)
                            if use_bias:
                                nc.vector.tensor_scalar(ps[:], ps[:],
                                                        bslice(0, "qkv", EC + oc),
                                                        None, OP.add)
                            psum_to_sbuf8(
                                ktf[:, oc,
                                    PAN * pan + 512 * half:PAN * pan + 512 * (half + 1)],
                                ps[:], eng="act")
                    # V (normal layout) for this panel
                    for tck in range(PAN // P):
                        ps = kv_r()
                        for sp in range(EC // 2):
                            nc.tensor.matmul(
                                ps[:],
                                hp8[:, 2 * sp:2 * sp + 2,
                                    tck * P:(tck + 1) * P],
                                wqkv[:, 2 * sp:2 * sp + 2, 2 * E:3 * E],
                                start=(sp == 0), stop=(sp == EC // 2 - 1),
                                perf_mode=DR)
                        kc = pan * (PAN // P) + tck
                        if use_bias:
                            add_vbias(0, ps)
                        with nc.allow_low_precision(reason="fp8 V"):
                            act(vf[:, kc, :, 0:HD],
                                ps[:].rearrange("p (h d) -> p h d", h=H),
                                AF.Copy, scale=IWS)
                mark("b0.attn")
                # attention: paired scores -> one exp per 2 k-chunks ->
                # DoubleRow AV accumulate
                for (q0, qn_) in ((0, 512), (512, 512), (1024, 128)):
                    n_poly = POLY_PER if qn_ == 512 else POLY_PER // 2
                    stride = (NKC // 2) // max(n_poly, 1)
                    for h_ in range(H):
                        poly_cs = set(range(0, NKC // 2, stride)[:n_poly])
                        hp, hh = h_ // 2, h_ % 2
                        po_t = po_g()
                        po = po_t[0]

                        def score_pair(sp2, c):
                            for i in (0, 1):
                                kc = 2 * c + i
                                nc.tensor.matmul(
                                    sp2[:, i, :qn_],
                                    ktf[HD * hh:HD * (hh + 1), hp,
                                        kc * P:(kc + 1) * P],
                                    qt_t[HD * hh:HD * (hh + 1), hp,
                                         q0:q0 + qn_],
                                    start=True, stop=True)

                        # poly pairs: scores+DVE-exp emitted early, AV deferred
                        # until a few ACT pairs later so the in-order PE stream
                        # never waits on the DVE chain.
                        n_emitted = [0]
                        pend = []

                        def av(pr8, c, last=False):
                            nc.tensor.matmul(
                                po[:, :qn_],
                                vf[:, 2 * c:2 * c + 2, h_, :],
                                pr8[:, :, :qn_],
                                start=(n_emitted[0] == 0), stop=last,
                                perf_mode=DR)
                            n_emitted[0] += 1

                        def start_poly(c):
                            sp2 = sc2_c()
                            score_pair(sp2, c)
                            pr8 = p_probs.tile([P, 2, 512], f8, tag="pr",
                                               name="pr")
                            poly_exp(pr8[:, :, :qn_], sp2[:, :, :qn_])
                            pend.append((pr8, c))

                        act_cs = [c for c in range(NKC // 2)
                                  if c not in poly_cs]
                        total = NKC // 2
                        held = []   # score tiles awaiting exp+AV (lookahead 1)

                        def flush_act(last=False):
                            sp2h, ch = held.pop(0)
                            pr8 = p_probs.tile([P, 2, 512], f8, tag="pr",
                                               name="pr")
                            act(pr8[:, :, :qn_], sp2h[:, :, :qn_], AF.Exp,
                                scale=SC_EXP)
                            av(pr8, ch, last=last)

                        for j, c in enumerate(act_cs):
                            if j % stride == 0 and poly_cs:
                                # drain previous poly, launch next
                                if pend:
                                    pr8p, cp = pend.pop(0)
                                    av(pr8p, cp)
                                nxt = min(poly_cs)
                                poly_cs.discard(nxt)
                                start_poly(nxt)
                            sp2 = sc2_g()
                            score_pair(sp2, c)
                            held.append((sp2, c))
                            if len(held) > 1:
                                flush_act()
                        while held:
                            flush_act(last=(n_emitted[0] == total - 1
                                            and not pend))
                        while pend:
                            pr8p, cp = pend.pop(0)
                            av(pr8p, cp, last=(n_emitted[0] == total - 1))
                        attn_finish(po_t, h_, q0, qn_, ot_strips)

            # ---- pools for the post-block0 phases (opened after kvfull frees,
            # closed at end of emission so repeat>1 can reopen) ----
            _lstack = contextlib.ExitStack()
            lpool = lambda name, bufs, **kw: _lstack.enter_context(
                tc.tile_pool(name=name, bufs=bufs, **kw))
            p_v = lpool("v", 1)          # V_ext [P, NTC, H, VD] fp8
            p_g = lpool("g", 2)          # gelu out [P, FFC, NT] fp8
            p_wfc1 = lpool("wfc1", 2)
            p_wfc2 = lpool("wfc2", 2)

            # ---------- MLP (ln2 + fc1 + gelu + fc2 + residual) ----------
            def mlp(i, x_strips, final=False):
                h2 = layernorm(x_strips, T, "h")
                ii = 2 * i if W_COMP_MLP else i
                wf1 = p_wfc1.tile([P, EC, FF], f8, tag="wfc1", name=f"wfc1_{i}")
                nc.sync.dma_start(wf1[:], w1[ii][:])
                wf2 = p_wfc2.tile([P, FFC, E], f8, tag="wfc2", name=f"wfc2_{i}")
                nc.sync.dma_start(wf2[:], w2[ii][:])
                if W_COMP_MLP:
                    wf1l = p_wfc1.tile([P, EC, FF], f8, tag="wfc1l",
                                       name=f"wfc1l_{i}", bufs=1)
                    nc.sync.dma_start(wf1l[:], w1[ii + 1][:])
                    wf2l = p_wfc2.tile([P, FFC, E], f8, tag="wfc2l",
                                       name=f"wfc2l_{i}", bufs=1)
                    nc.sync.dma_start(wf2l[:], w2[ii + 1][:])
                xdt = f32 if final else b16
                x_new = [p_x.tile([P, T], xdt, tag=f"x{s}", name=f"xm{i}_{s}")
                         for s in range(EC)]
                for nt in range(NQT):
                    o0 = nt * NT
                    g = p_g.tile([P, FFC, NT], f8, tag="g", name="g")
                    for fcp in range(FFC // 2):
                        ps2 = sc2_g()
                        for j in (0, 1):
                            fc = 2 * fcp + j
                            n_pass = 4 if W_COMP_MLP else 2
                            for k in range(n_pass):
                                wsrc = wf1 if k < 2 else wf1l
                                sp = k % 2
                                nc.tensor.matmul(
                                    ps2[:, j, :NT],
                                    wsrc[:, 2 * sp:2 * sp + 2,
                                         fc * P:(fc + 1) * P],
                                    h2[:, 2 * sp:2 * sp + 2, o0:o0 + NT],
                                    start=(k == 0), stop=(k == n_pass - 1),
                                    perf_mode=DR)
                        with nc.allow_low_precision(reason="fp8 gelu"):
                            if use_bias:
                                for j in (0, 1):
                                    act(g[:, 2 * fcp + j, :], ps2[:, j, :NT],
                                        AF.Gelu_apprx_tanh, scale=IWS,
                                        bias=bslice(i, "fc1", 2 * fcp + j))
                            else:
                                act(g[:, 2 * fcp:2 * fcp + 2, :],
                                    ps2[:, :, :NT],
                                    AF.Gelu_apprx_tanh, scale=IWS)
                    for oc in range(EC):
                        ps = mlp_r()
                        n_pass = FFC if W_COMP_MLP else FFC // 2
                        for k in range(n_pass):
                            wsrc = wf2 if k < FFC // 2 else wf2l
                            fp_ = k % (FFC // 2)
                            nc.tensor.matmul(
                                ps[:, :NT],
                                wsrc[:, 2 * fp_:2 * fp_ + 2,
                                     oc * P:(oc + 1) * P],
                                g[:, 2 * fp_:2 * fp_ + 2, :],
                                start=(k == 0), stop=(k == n_pass - 1),
                                perf_mode=DR)
                        if use_bias:
                            nc.vector.tensor_scalar(ps[:, :NT], ps[:, :NT],
                                                    bslice(i, "fc2", oc),
                                                    None, OP.add)
                        with nc.allow_low_precision(reason="bf16 residual"):
                            nc.vector.scalar_tensor_tensor(
                                x_new[oc][:, o0:o0 + NT], ps[:, :NT], IWS,
                                x_strips[oc][:, o0:o0 + NT], OP.mult, OP.add)
                return x_new

            mark("b0.proj+mlp")
            # block 0 out projection + residual + MLP
            x1 = [p_x.tile([P, T], b16, tag=f"x{s}", name=f"x1_{s}")
                  for s in range(EC)]

            def post_out0(oc, o, n, ps):
                with nc.allow_low_precision(reason="bf16 residual"):
                    nc.vector.scalar_tensor_tensor(
                        x1[oc][:, o:o + n], ps[:, :n], IWS, xs[oc][:, o:o + n],
                        OP.mult, OP.add)
            gemm8(wout, 0, EC, ot0, T, post_out0, bias_i=0, bias_grp="out")
            x1 = mlp(0, x1)

            # ==================================================================
            # BLOCKS 1, 2: banded attention
            # ==================================================================
            mark("banded")
            x_cur = x1
            for i in (1, 2):
                lower = (i == 1)
                mark(f"b{i}.ln1qkv")
                wqkv = p_wqkv.tile([P, EC, 3 * E], f8, tag="wqkv",
                                   name=f"wqkv{i}")
                nc.sync.dma_start(wqkv[:], wq[i][:])
                wout = p_wout.tile([P, EC, E], f8, tag="wout", name=f"wout{i}")
                nc.sync.dma_start(wout[:], wo[i][:])
                h1 = layernorm(x_cur, T, "h")
                qt_t = p_qt.tile([P, EC, T], f8, tag="qt", name=f"qt{i}")
                kt_t = p_kt.tile([P, EC, T], f8, tag="kt", name=f"kt{i}")
                v_t = p_v.tile([P, NTC, H, VD], f8, tag="v", name=f"v{i}")
                nc.vector.memset(v_t[:, :, :, HD:HD + 1], 1.0)
                nc.vector.memset(v_t[:, :, :, HD + 1:VD], 0.0)
                otb = p_ot.tile([P, EC, T], f8, tag="ot", name=f"ot{i}")
                ot_strips = [otb[:, s, :] for s in range(EC)]
                mark(f"b{i}.attn")
                # per query tile: groups of (kc0, n_kc, mask, window) where a
                # 2-group is a DoubleRow pair over the union window.
                # c-window tables (relative k-chunk c -> query window):
                WINS = ([(0, 15), (0, 143), (128, 143), (256, 128)] if lower
                        else [(0, 143), (113, 143), (241, 143), (369, 15)])

                def union(c0, c1):
                    lo = min(WINS[c0][0], WINS[c1][0])
                    hi = max(WINS[c0][0] + WINS[c0][1],
                             WINS[c1][0] + WINS[c1][1])
                    return (lo, hi - lo)

                def groups_for(qt):
                    # valid relative chunks: kc = 3qt + (c-1 | c)
                    cs = [c for c in range(4)
                          if 0 <= 3 * qt + (c - 1 if lower else c) < NTC]
                    kc0 = lambda c: 3 * qt + (c - 1 if lower else c)
                    bnd = "lo" if lower else "up"
                    gs = []
                    if cs == [0, 1, 2, 3]:
                        gs.append((kc0(0), 2, masks[f"m2_{bnd}01"], union(0, 1),
                                   (0, 1)))
                        gs.append((kc0(2), 2, masks[f"m2_{bnd}23"], union(2, 3),
                                   (2, 3)))
                    elif cs == [1, 2, 3]:
                        gs.append((kc0(1), 2, masks["m2_lo12"], union(1, 2),
                                   (1, 2)))
                        gs.append((kc0(3), 1, masks["m2_lo23"], WINS[3],
                                   (3, None)))
                    elif cs == [0, 1, 2]:
                        gs.append((kc0(0), 2, masks["m2_up01"], union(0, 1),
                                   (0, 1)))
                        gs.append((kc0(2), 1, masks["m2_up23"], WINS[2],
                                   (2, None)))
                    else:
                        raise AssertionError(cs)
                    return gs

                def emit_attn_qt(qt):
                    for h_ in range(H):
                        hp, hh = h_ // 2, h_ % 2
                        po_t = po_b()
                        po = po_t[0]
                        covered = []
                        gs = groups_for(qt)
                        sps = []
                        for (kcs, nk, mask_t, (qo, qw), cpair) in gs:
                            sp2 = sc2_g()
                            for ii in range(nk):
                                nc.tensor.matmul(
                                    sp2[:, ii, qo:qo + qw],
                                    kt_t[HD * hh:HD * (hh + 1), hp,
                                         (kcs + ii) * P:(kcs + ii + 1) * P],
                                    qt_t[HD * hh:HD * (hh + 1), hp,
                                         qt * NT + qo:qt * NT + qo + qw],
                                    start=True, stop=True)
                            sps.append(sp2)
                        for gi, (kcs, nk, mask_t, (qo, qw), cpair) in enumerate(gs):
                            sp2 = sps[gi]
                            pr8 = p_probs.tile([P, 2, 512], f8, tag="pr",
                                               name="pr")
                            act(pr8[:, :nk, qo:qo + qw], sp2[:, :nk, qo:qo + qw],
                                AF.Exp, scale=SC_EXP)
                            prm = p_probs.tile([P, 2, 512], f8, tag="prm",
                                               name="prm")
                            with nc.allow_low_precision(reason="fp8 probs"):
                                if nk == 2:
                                    nc.gpsimd.tensor_mul(
                                        prm[:, :2, qo:qo + qw],
                                        pr8[:, :2, qo:qo + qw],
                                        mask_t[:, :, qo:qo + qw])
                                else:
                                    # single: mask half cpair[0]%2 of the pair
                                    mi = cpair[0] % 2
                                    nc.gpsimd.tensor_mul(
                                        prm[:, 0, qo:qo + qw],
                                        pr8[:, 0, qo:qo + qw],
                                        mask_t[:, mi, qo:qo + qw])
                            # AV into po, split into fresh/accum column parts
                            parts = []
                            pos = qo
                            for (clo, chi) in covered + [(qo + qw, qo + qw)]:
                                if pos >= qo + qw:
                                    break
                                if chi <= pos:
                                    continue
                                if clo > pos:
                                    parts.append((pos, min(clo, qo + qw), True))
                                if clo < qo + qw:
                                    lo = max(clo, pos)
                                    hi = min(chi, qo + qw)
                                    if lo < hi:
                                        parts.append((lo, hi, False))
                                pos = max(pos, chi)
                            for (lo, hi, is_new) in parts:
                                if nk == 2:
                                    nc.tensor.matmul(
                                        po[:, lo:hi],
                                        v_t[:, kcs:kcs + 2, h_, :],
                                        prm[:, :2, lo:hi],
                                        start=is_new, stop=False,
                                        perf_mode=DR, skip_group_check=True)
                                else:
                                    nc.tensor.matmul(
                                        po[:, lo:hi],
                                        v_t[:, kcs, h_, :],
                                        prm[:, 0, lo:hi],
                                        start=is_new, stop=False,
                                        skip_group_check=True)
                            covered = sorted(covered + [(qo, qo + qw)])
                            merged = []
                            for (lo, hi) in covered:
                                if merged and lo <= merged[-1][1]:
                                    merged[-1] = (merged[-1][0],
                                                  max(hi, merged[-1][1]))
                                else:
                                    merged.append((lo, hi))
                            covered = merged
                        attn_finish(po_t, h_, qt * NT, NT, ot_strips)

                # emit qkv per token tile, with each attention query tile
                # interleaved as soon as its K/V columns exist (in-order
                # engine streams otherwise serialize attention behind the
                # whole projection)
                for nt in range(NQT):
                    o0 = nt * NT
                    for oc in range(EC):
                        ps = qkv_r()
                        for sp in range(EC // 2):
                            nc.tensor.matmul(
                                ps[:, :NT],
                                wqkv[:, 2 * sp:2 * sp + 2, oc * P:(oc + 1) * P],
                                h1[:, 2 * sp:2 * sp + 2, o0:o0 + NT],
                                start=(sp == 0), stop=(sp == EC // 2 - 1),
                                perf_mode=DR)
                        if use_bias:
                            nc.vector.tensor_scalar(ps[:, :NT], ps[:, :NT],
                                                    bslice(i, "qkv", oc),
                                                    None, OP.add)
                        psum_to_sbuf8(qt_t[:, oc, o0:o0 + NT], ps[:, :NT],
                                      eng="act")
                        ps = qkv_r()
                        for sp in range(EC // 2):
                            nc.tensor.matmul(
                                ps[:, :NT],
                                wqkv[:, 2 * sp:2 * sp + 2,
                                     E + oc * P:E + (oc + 1) * P],
                                h1[:, 2 * sp:2 * sp + 2, o0:o0 + NT],
                                start=(sp == 0), stop=(sp == EC // 2 - 1),
                                perf_mode=DR)
                        if use_bias:
                            nc.vector.tensor_scalar(ps[:, :NT], ps[:, :NT],
                                                    bslice(i, "qkv", EC + oc),
                                                    None, OP.add)
                        psum_to_sbuf8(kt_t[:, oc, o0:o0 + NT], ps[:, :NT],
                                      eng="act")
                    for tck in range(3 * nt, 3 * nt + 3):
                        ps = qkv_r()
                        for sp in range(EC // 2):
                            nc.tensor.matmul(
                                ps[:],
                                h1[:, 2 * sp:2 * sp + 2,
                                   tck * P:(tck + 1) * P],
                                wqkv[:, 2 * sp:2 * sp + 2, 2 * E:3 * E],
                                start=(sp == 0), stop=(sp == EC // 2 - 1),
                                perf_mode=DR)
                        if use_bias:
                            add_vbias(i, ps)
                        with nc.allow_low_precision(reason="fp8 V"):
                            act(v_t[:, tck, :, 0:HD],
                                ps[:].rearrange("p (h d) -> p h d", h=H),
                                AF.Copy, scale=IWS)
                    aq = nt if lower else nt - 1
                    if aq >= 0:
                        emit_attn_qt(aq)
                if not lower:
                    emit_attn_qt(NQT - 1)
                mark(f"b{i}.projmlp")
                x_new = [p_x.tile([P, T], b16, tag=f"x{s}", name=f"xa{i}_{s}")
                         for s in range(EC)]

                def post_out(oc, o, n, ps, x_new=x_new, x_cur=x_cur):
                    with nc.allow_low_precision(reason="bf16 residual"):
                        nc.vector.scalar_tensor_tensor(
                            x_new[oc][:, o:o + n], ps[:, :n], IWS,
                            x_cur[oc][:, o:o + n], OP.mult, OP.add)
                gemm8(wout, 0, EC, otb, T, post_out, bias_i=i, bias_grp="out")
                x_cur = mlp(i, x_new, final=(i == 2))

            mark("out")
            # output
            for s in range(EC):
                nc.sync.dma_start(yT[P * s:P * (s + 1), :], x_cur[s][:])
            _lstack.close()

        for _rep in range(repeat):
            _emit_once()

        # record build-order instruction -> phase map (before scheduling)
        _PHASE_OF.clear()
        names = [ins.name for bb in nc.main_func.blocks for ins in bb.instructions]
        bounds = [n for _, n in _PHASE_MARKS]
        labels = [l for l, _ in _PHASE_MARKS]
        import bisect as _bis
        for idx, nm in enumerate(names):
            j = _bis.bisect_right(bounds, idx) - 1
            _PHASE_OF[nm] = labels[j] if j >= 0 else "pre"

    nc.compile()
    return nc


# ------------------------------------------------------------------
# cached executor (compile once, run many)
# ------------------------------------------------------------------
class _Exec:
    def __init__(self, use_bias: bool):
        import jax
        import concourse.mybir as mybir
        from concourse import bass2jax
        from concourse.bass2jax import install_neuronx_cc_hook, _bass_exec_p
        from jax.sharding import Mesh, PartitionSpec
        from jax.experimental.shard_map import shard_map

        install_neuronx_cc_hook()
        nc = _build_nc(use_bias)
        self.nc = nc

        part_name = (nc.partition_id_tensor.name
                     if nc.partition_id_tensor is not None else None)
        in_names, out_names, out_avals = [], [], []
        self.zero_shapes = []
        for alloc in nc.m.functions[0].allocations:
            if not isinstance(alloc, mybir.MemoryLocationSet):
                continue
            name = alloc.memorylocations[0].name
            if alloc.kind == "ExternalInput":
                if name != part_name:
                    in_names.append(name)
            elif alloc.kind == "ExternalOutput":
                out_names.append(name)
                shape = tuple(alloc.tensor_shape)
                dtype = mybir.dt.np(alloc.dtype)
                out_avals.append(jax.core.ShapedArray(shape, dtype))
                self.zero_shapes.append((shape, dtype))
        n_params = len(in_names)
        all_in = in_names + out_names
        if part_name is not None:
            all_in = all_in + [part_name]
        self.in_names = in_names
        self.out_names = out_names
        n_outs = len(out_names)

        def _body(*args):
            operands = list(args)
            if part_name is not None:
                operands.append(bass2jax.partition_id_tensor())
            outs = _bass_exec_p.bind(
                *operands,
                out_avals=tuple(out_avals),
                in_names=tuple(all_in),
                out_names=tuple(out_names),
                lowering_input_output_aliases=(),
                sim_require_finite=True,
                sim_require_nnan=True,
                nc=nc,
            )
            return tuple(outs)
        self._body = _body

        devices = jax.devices()[:N_CORES]
        mesh = Mesh(np.asarray(devices), ("core",))
        in_specs = (PartitionSpec("core"),) * (n_params + n_outs)
        out_specs = (PartitionSpec("core"),) * n_outs
        donate = tuple(range(n_params, n_params + n_outs))
        self.fn = jax.jit(
            shard_map(_body, mesh=mesh, in_specs=in_specs,
                      out_specs=out_specs, check_rep=False),
            donate_argnums=donate, keep_unused=True)
        self.out_avals = out_avals

    def bench(self, in_maps, iters=10):
        """device-resident-input timing: returns per-iteration seconds"""
        import time
        import jax
        from jax.sharding import Mesh, PartitionSpec, NamedSharding
        if not hasattr(self, "_bench_fn"):
            from jax.experimental.shard_map import shard_map
            devices = jax.devices()[:N_CORES]
            mesh = Mesh(np.asarray(devices), ("core",))
            n_in = len(self.in_names) + len(self.zero_shapes)
            self._bench_fn = jax.jit(
                shard_map(self._body, mesh=mesh,
                          in_specs=(PartitionSpec("core"),) * n_in,
                          out_specs=(PartitionSpec("core"),) * len(self.out_names),
                          check_rep=False),
                keep_unused=True)
            self._bench_sharding = NamedSharding(mesh, PartitionSpec("core"))
        concat_in = [
            np.concatenate([np.asarray(in_maps[c][n]) for c in range(N_CORES)], axis=0)
            for n in self.in_names
        ] + [np.zeros((N_CORES * s[0], *s[1:]), d) for (s, d) in self.zero_shapes]
        import jax
        dev_in = [jax.device_put(a, self._bench_sharding) for a in concat_in]
        out = jax.block_until_ready(self._bench_fn(*dev_in))  # warm/compile
        t0 = time.time()
        for _ in range(iters):
            out = self._bench_fn(*dev_in)
        jax.block_until_ready(out)
        return (time.time() - t0) / iters

    def run(self, in_maps):
        """in_maps: list of 8 dicts name->np.ndarray. returns list of dicts"""
        concat_in = [
            np.concatenate([np.asarray(in_maps[c][n]) for c in range(N_CORES)], axis=0)
            for n in self.in_names
        ]
        concat_zeros = [np.zeros((N_CORES * s[0], *s[1:]), d)
                        for (s, d) in self.zero_shapes]
        outs = self.fn(*concat_in, *concat_zeros)
        import jax
        outs = jax.block_until_ready(outs)
        res = []
        for c in range(N_CORES):
            d = {}
            for idx, n in enumerate(self.out_names):
                shp = self.out_avals[idx].shape
                d[n] = np.asarray(outs[idx]).reshape(N_CORES, *shp)[c]
            res.append(d)
        return res


def _get_exec(use_bias: bool) -> "_Exec":
    key = bool(use_bias)
    if key not in _EXEC_CACHE:
        _EXEC_CACHE[key] = _Exec(key)
    return _EXEC_CACHE[key]


# ------------------------------------------------------------------
# host-side input prep
# ------------------------------------------------------------------
def _band_masks():
    ki = np.arange(P)[:, None]
    qi = np.arange(NT)[None, :]
    m = {}
    for c in range(4):
        # lower band, k-chunk kc = 3t + (c-1):  0 <= (i-k) <= W-1 with
        # i-k = qi - ki + 128*(1-c)
        d = qi - ki + P * (1 - c)
        m[f"lo{c}"] = ((d >= 0) & (d <= W_BAND - 1)).astype(bf16)
        # upper band, k-chunk kc = 3t + c:  0 <= (k-i) <= W-1 with
        # k-i = ki - qi + 128*c
        d = ki - qi + P * c
        m[f"up{c}"] = ((d >= 0) & (d <= W_BAND - 1)).astype(bf16)
    out = {}
    for bnd in ("lo", "up"):
        for (nm, a, b) in (("01", 0, 1), ("23", 2, 3)):
            out[f"m2_{bnd}{nm}"] = np.ascontiguousarray(
                np.stack([m[f"{bnd}{a}"], m[f"{bnd}{b}"]], axis=1))
    out["m2_lo12"] = np.ascontiguousarray(
        np.stack([m["lo1"], m["lo2"]], axis=1))
    return out


def kernel(x, ln1_w, ln1_b, ln2_w, ln2_b, qkv_w, qkv_b, out_w, out_b,
           fc1_w, fc1_b, fc2_w, fc2_b):
    x = np.asarray(x, np.float32)
    ln1_w = np.asarray(ln1_w, np.float32); ln1_b = np.asarray(ln1_b, np.float32)
    ln2_w = np.asarray(ln2_w, np.float32); ln2_b = np.asarray(ln2_b, np.float32)
    qkv_w = np.asarray(qkv_w, np.float32); qkv_b = np.asarray(qkv_b, np.float32)
    out_w = np.asarray(out_w, np.float32); out_b = np.asarray(out_b, np.float32)
    fc1_w = np.asarray(fc1_w, np.float32); fc1_b = np.asarray(fc1_b, np.float32)
    fc2_w = np.asarray(fc2_w, np.float32); fc2_b = np.asarray(fc2_b, np.float32)

    # fold LN affine into following projection weights (exact):
    #   h = z*w + b  =>  h @ Wt.T = z @ (W*w).T + b @ W.T
    wq_f, wo_f, w1_f, w2_f = [], [], [], []
    biases = []
    for i in range(3):
        qw = qkv_w[i] * ln1_w[i][None, :]
        qb = qkv_b[i] + qkv_w[i] @ ln1_b[i]
        f1 = fc1_w[i] * ln2_w[i][None, :]
        f1b = fc1_b[i] + fc1_w[i] @ ln2_b[i]
        wq_f.append(qw); w1_f.append(f1)
        wo_f.append(out_w[i]); w2_f.append(fc2_w[i])
        # packed per-feature bias tile [P, 32] (q, k, out, fc1, fc2) + v row.
        # q/k/out/fc2 biases ride the x64-scaled psum; fc1 bias is consumed
        # by the gelu activation in true units.
        bias_pack = np.zeros((P, 32), np.float32)
        bias_pack[:, 0:8] = WS * qb[:2 * E].reshape(8, P).T
        bias_pack[:, 8:12] = WS * out_b[i].reshape(4, P).T
        bias_pack[:, 12:28] = f1b.reshape(16, P).T
        bias_pack[:, 28:32] = WS * fc2_b[i].reshape(4, P).T
        biases.append((bias_pack, WS * qb[2 * E:].reshape(1, E)))
    use_bias = any(np.abs(b).max() > 0 or np.abs(np.asarray(v, np.float32)).max() > 0
                   for b, v in biases)

    ex = _get_exec(use_bias)

    masks = _band_masks()
    # weight tensors: x64 pre-scale into fp8e4, [chunk, P, out] layout
    weights = {}
    for i in range(3):
        pm = lambda a, C: np.ascontiguousarray(
            a.reshape(C, P, -1).transpose(1, 0, 2).reshape(P, -1))
        weights[f"qkvT{i}"] = pm(WS * wq_f[i].T, EC).astype(f8e4)
        weights[f"outT{i}"] = pm(WS * wo_f[i].T, EC).astype(f8e4)
        w1s = pm(WS * w1_f[i].T, EC)
        w2s = pm(WS * w2_f[i].T, FFC)
        weights[f"fc1T{i}"] = w1s.astype(f8e4)
        weights[f"fc2T{i}"] = w2s.astype(f8e4)
        if W_COMP_MLP:
            weights[f"fc1L{i}"] = (
                w1s - weights[f"fc1T{i}"].astype(np.float32)).astype(f8e4)
            weights[f"fc2L{i}"] = (
                w2s - weights[f"fc2T{i}"].astype(np.float32)).astype(f8e4)
        if use_bias:
            weights[f"bias{i}"] = biases[i][0]
            weights[f"vbias{i}"] = biases[i][1].astype(bf16)

    in_maps = []
    for core in range(N_CORES):
        b, c = divmod(core, CHUNKS)
        e0 = EXT_STARTS[c]
        xT_ext = np.ascontiguousarray(x[b, e0:e0 + T, :].T).astype(bf16)
        xfT = np.ascontiguousarray(x[b].T).astype(bf16)
        m = {"xT": xT_ext, "xfT": xfT, **weights, **masks}
        in_maps.append(m)

    res = ex.run(in_maps)

    out = np.empty((B, S, E), np.float32)
    for core in range(N_CORES):
        b, c = divmod(core, CHUNKS)
        yT_ = res[core]["yT"]               # [E, T]
        off = OWN_OFF[c]
        out[b, OWN * c:OWN * (c + 1), :] = yT_[:, off:off + OWN].T
    return out


# revision 30
# speedup vs baseline: 1.2675x; 1.0143x over previous
"""Trainium2 Bass kernel for nn_AlternateAttention (3-block transformer:
global attention, lower-band attention, upper-band attention, each with MLP).

Sharding: 8 cores = 2 batches x 4 sequence chunks of 1024 tokens, each core
processing an extended window of 1152 tokens (64-token halo each side) so the
banded blocks need no inter-core communication. Block 0 (global attention)
needs full-sequence K/V; each core recomputes them from the replicated raw
input (LN1+KV projection over the full 4096 tokens of its batch).

Layout: activations live TRANSPOSED in SBUF ([feature, token]) so projections
are natural matmuls. All projection weights are cast to fp8e4 (scaled x64 so
N(0,0.02) weights stay in the normal range) and GEMMs run in DoubleRow perf
mode (two 128-deep k-tiles per instruction at 0.5 cycles/row).  The 1/64
de-scale is folded into the psum->sbuf consumer (activation scale or
scalar_tensor_tensor) -- for attention scores both Q and K carry x64 so the
softmax exp scale absorbs the 4096.  Attention probabilities and V are fp8
so AV also runs DoubleRow; exp reads score PAIRS from 2-bank [P,2,512] PSUM
tiles to halve the Activation-engine fixed overhead.
"""
import contextlib
import numpy as np
import ml_dtypes

# ---- problem constants (hardcoded per contract) ----
B, S, E, H, W_BAND, FF = 2, 4096, 512, 8, 16, 2048
HD = E // H                      # 64
N_CORES = 8
CHUNKS = 4                       # sequence chunks per batch
OWN = S // CHUNKS                # 1024
HALO = 64
T = OWN + 2 * HALO               # 1152 = 9*128
P = 128
NT = 384                         # token tile (3 per T)
NQT = T // NT                    # 3
NKC = S // P                     # 32 k-chunks for global attention
NTC = T // P                     # 9 token chunks of ext window
EC = E // P                      # 4 feature strips
FFC = FF // P                    # 16
VD = HD + 2                      # V tile row pitch: 66 so DoubleRow pair stride (H*VD) % 16 == 0

WS = 64.0                        # weight pre-scale into fp8
POLY_PER = 4                     # DVE-exp'd score pairs per (head, q-chunk)
IWS = 1.0 / WS
SC_EXP = 0.125 / (WS * WS)       # exp scale absorbing q/k weight scales

EXT_STARTS = [max(0, min(OWN * c - HALO, S - T)) for c in range(CHUNKS)]
OWN_OFF = [OWN * c - EXT_STARTS[c] for c in range(CHUNKS)]

_EXEC_CACHE = {}
_PHASE_MARKS = []
_PHASE_OF = {}

bf16 = ml_dtypes.bfloat16
f8e4 = ml_dtypes.float8_e4m3


# ------------------------------------------------------------------
# device program
# ------------------------------------------------------------------
def _patch_act_tables():
    # The bacc table-load placement maps exp -> 'exp_and_others' and
    # ln -> 'natural_log', thrashing the ACT table RAM (~1.3us per switch,
    # dozens of switches). Restrict the choice to the two sets that cover
    # everything this kernel uses so exp/ln never evict each other.
    import concourse.hw_specs as hw_specs
    import concourse.bacc as bacc_mod
    import concourse.bass_interp as bass_interp
    if getattr(hw_specs, "_aa_patched", False):
        return
    orig = hw_specs.get_activation_tables
    keep = {"natural_log_exp_and_others", "gelu_apprx_tanh_and_others"}

    def _gat(arch):
        tabs = orig(arch)
        return {k: (v if k in keep else set()) for k, v in tabs.items()}

    hw_specs.get_activation_tables = _gat
    bacc_mod.get_activation_tables = _gat
    bass_interp.get_activation_tables = _gat
    hw_specs._aa_patched = True


def _build_nc(use_bias: bool, repeat: int = 1):
    import concourse.bacc as bacc
    import concourse.mybir as mybir
    import concourse.tile as tile

    _patch_act_tables()

    f32 = mybir.dt.float32
    b16 = mybir.dt.bfloat16
    f8 = mybir.dt.float8e4
    AF = mybir.ActivationFunctionType
    OP = mybir.AluOpType
    DR = mybir.MatmulPerfMode.DoubleRow

    nc = bacc.Bacc("TRN2", target_bir_lowering=False, debug=False,
                   num_devices=N_CORES)
    _PHASE_MARKS.clear()

    def mark(label):
        n = sum(len(b.instructions) for b in nc.main_func.blocks)
        _PHASE_MARKS.append((label, n))

    # ---- dram tensors ----
    xT = nc.dram_tensor("xT", [E, T], b16, kind="ExternalInput")
    xfT = nc.dram_tensor("xfT", [E, S], b16, kind="ExternalInput")
    wq, wo, w1, w2, bias_d = [], [], [], [], []
    for i in range(3):
        wq.append(nc.dram_tensor(f"qkvT{i}", [P, EC * 3 * E], f8, kind="ExternalInput"))
        wo.append(nc.dram_tensor(f"outT{i}", [P, EC * E], f8, kind="ExternalInput"))
        w1.append(nc.dram_tensor(f"fc1T{i}", [P, EC * FF], f8, kind="ExternalInput"))
        w2.append(nc.dram_tensor(f"fc2T{i}", [P, FFC * E], f8, kind="ExternalInput"))
        if W_COMP_MLP:
            w1.append(nc.dram_tensor(f"fc1L{i}", [P, EC * FF], f8, kind="ExternalInput"))
            w2.append(nc.dram_tensor(f"fc2L{i}", [P, FFC * E], f8, kind="ExternalInput"))
        if use_bias:
            # packed per-feature biases for transposed-layout outputs:
            # [q(4xP) | k(4xP) | out(4xP) | fc1(16xP) | fc2(4xP)] -> [P, 32]
            bias_d.append(nc.dram_tensor(f"bias{i}", [P, 32], f32, kind="ExternalInput"))
            bias_d.append(nc.dram_tensor(f"vbias{i}", [1, E], b16, kind="ExternalInput"))
    # paired 0/1 bf16 stripe-mask tiles for banded attention: [P, 2, NT]
    mask_d = {}
    for nm in ("m2_lo01", "m2_lo23", "m2_lo12", "m2_up01", "m2_up23"):
        mask_d[nm] = nc.dram_tensor(nm, [P, 2, NT], b16, kind="ExternalInput")
    yT = nc.dram_tensor("yT", [E, T], f32, kind="ExternalOutput")

    from concourse.tile import add_dep_helper as _adh

    with tile.TileContext(nc) as tc, contextlib.ExitStack() as ctx:
        pool = lambda name, bufs, **kw: ctx.enter_context(
            tc.tile_pool(name=name, bufs=bufs, **kw))

        # Order ACT instructions across table-set boundaries (exp/ln vs gelu)
        # so the activation-table RAM isn't thrashed (~1.3us per reload).
        # Within a set no ordering is imposed.
        _act_sets = {AF.Gelu_apprx_tanh: 1}
        _prev_set = [None]
        _prev_insts = [[]]
        _cur_insts = [[]]

        def act(out, in_, func, **kw):
            inst = nc.scalar.activation(out, in_, func, **kw)
            if func == AF.Copy:
                return inst
            s = _act_sets.get(func, 0)
            if _prev_set[0] is None:
                _prev_set[0] = s
            if s != _prev_set[0]:
                _prev_insts[0] = _cur_insts[0][-64:]
                _cur_insts[0] = []
                _prev_set[0] = s
            for p in _prev_insts[0]:
                _adh(inst.ins, p.ins, sync=True, reason="act-table batch order")
            _cur_insts[0].append(inst)
            return inst

        # ---- pools live for the whole kernel ----
        p_x = pool("x", 2)           # residual strips f32, tags x0..x3
        p_h = pool("h", 2)           # LN output fp8 [P, EC, Tn]
        p_sq = pool("sq", 4)         # per-nt squares bf16
        p_qt = pool("qt", 1)         # QT [P, EC, T] fp8
        p_kt = pool("kt", 1)         # KT (banded) [P, EC, T] fp8
        p_ot = pool("ot", 1)         # attention out strips fp8
        p_wqkv = pool("wqkv", 2)     # qkv weights [P, EC, 3E] fp8
        p_wout = pool("wout", 2)
        p_probs = pool("probs", 8)   # exp outputs fp8 [P, 2, 512]
        p_tmp = pool("tmp", 2)       # temporaries
        p_sm = pool("sm", 2)         # small [1, n] stat vectors
        p_c = pool("const", 1)       # ones, masks, biases
        # PSUM: 8 banks = two 2-bank score slots + four single banks
        p_ps = pool("ps", 1, space="PSUM")
        # sA/sB: [P,2,512] paired attention scores (also misc gemm accum)
        # p0..p3: po accumulators / LN stats / broadcasts / gemm accum

        ones = p_c.tile([P, P], b16)
        nc.vector.memset(ones[:], 1.0)
        eps1 = p_c.tile([1, 1], f32)
        nc.vector.memset(eps1[:], 1e-5)
        masks = {}
        for nm, d in mask_d.items():
            mt = p_c.tile([P, 2, NT], b16, tag=f"mask_{nm}", name=f"mask_{nm}")
            nc.sync.dma_start(mt[:], d[:])
            masks[nm] = mt
        bias_t, vbias_t = [], []
        if use_bias:
            for i in range(3):
                bt = p_c.tile([P, 32], f32, tag=f"bias{i}", name=f"bias{i}")
                nc.sync.dma_start(bt[:], bias_d[2 * i][:])
                bias_t.append(bt)
                vt = p_c.tile([1, E], b16, tag=f"vbias{i}", name=f"vbias{i}")
                nc.sync.dma_start(vt[:], bias_d[2 * i + 1][:])
                vbias_t.append(vt)

        def bslice(i, group, oc):
            base = {"qkv": 0, "out": 8, "fc1": 12, "fc2": 28}[group]
            return bias_t[i][:, base + oc:base + oc + 1]

        def add_vbias(i, ps):
            # V projection output is in normal layout [token, feat]: bias
            # varies along the free dim -> broadcast with a K=1 matmul.
            vb = p_ps.tile([P, 512], f32, tag="p0", name="vb")
            nc.tensor.matmul(vb[:], ones[0:1, :], vbias_t[i][:],
                             start=True, stop=True)
            vbs = p_tmp.tile([P, 512], f32, tag="vbs", name="vbs")
            nc.vector.tensor_copy(vbs[:], vb[:])
            nc.vector.tensor_add(ps[:], ps[:], vbs[:])

        def _emit_once():
            # load residual strips
            xs = []
            for s in range(EC):
                t = p_x.tile([P, T], b16, tag=f"x{s}", name=f"xin{s}")
                nc.sync.dma_start(t[:], xT[P * s:P * (s + 1), :])
                xs.append(t)

            # ---------- layernorm ----------
            def layernorm(x_strips, Tn, htag,
                          stat_tags=("sA",), bc_tags=("sB", "sC"),
                          sq_eng=None):
                """x_strips: 4 bf16 strips [P, Tn] -> fp8 [P,EC,Tn].
                Stats via ones-matmul partition reductions (bf16), squares on
                GPSIMD (SBUF->SBUF), per-token mean/rstd broadcast via K=1
                matmuls, normalize on DVE in 2x bf16 mode, fp8 output."""
                sq_eng = sq_eng or nc.gpsimd
                h8 = p_h.tile([P, EC, Tn], f8, tag=htag, name=htag)
                step = 512 if Tn % 512 == 0 else NT
                nss = [(k * step, min(step, Tn - k * step))
                       for k in range((Tn + step - 1) // step)]
                for si, (o, n) in enumerate(nss):
                    sl = slice(o, o + n)
                    s12 = p_ps.tile([1, 1024], f32, tag=stat_tags[0], name="s12")
                    s1 = s12[:, 0:512]
                    s2 = s12[:, 512:1024]
                    for s in range(EC):
                        rhs_x = x_strips[s][:, sl]
                        nc.tensor.matmul(s1[:, :n], ones[:, 0:1], rhs_x,
                                         start=(s == 0), stop=(s == EC - 1))
                        sq_nt = p_sq.tile([P, 512], b16, tag="sqn", name="sqn")
                        sq_eng.tensor_mul(sq_nt[:, :n], rhs_x, rhs_x)
                        nc.tensor.matmul(s2[:, :n], ones[:, 0:1], sq_nt[:, :n],
                                         start=(s == 0), stop=(s == EC - 1))
                    with nc.allow_low_precision(reason="bf16 LN stats"):
                        m_b = p_sm.tile([1, 512], b16, tag="m_b", name="m_b")
                        nc.vector.tensor_scalar(m_b[:, :n], s1[:, :n], 1.0 / E,
                                                None, OP.mult)
                        stt = p_sm.tile([1, 1024], b16, tag="stt", name="stt")
                        sa, sb = stt[:, 0:n], stt[:, 512:512 + n]
                        nc.vector.tensor_scalar(sa, s2[:, :n], 1.0 / E, None,
                                                OP.mult)
                        nc.vector.tensor_mul(sb, m_b[:, :n], m_b[:, :n])
                        nc.vector.tensor_sub(sa, sa, sb)
                        act(sb, sa, AF.Ln, bias=eps1[:])
                        lnv = sb
                        r_b = p_sm.tile([1, 512], b16, tag="r_b", name="r_b")
                        act(r_b[:, :n], lnv, AF.Exp, scale=-0.5)
                    bc = p_ps.tile([P, 2, 512], f32,
                                   tag=bc_tags[si % len(bc_tags)], name="bc")
                    mB, rB = bc[:, 0, :], bc[:, 1, :]
                    nc.tensor.matmul(mB[:, :n], ones[0:1, :], m_b[:, :n],
                                     start=True, stop=True)
                    nc.tensor.matmul(rB[:, :n], ones[0:1, :], r_b[:, :n],
                                     start=True, stop=True)
                    mBs = p_tmp.tile([P, 512], b16, tag="mBs", name="mBs")
                    act(mBs[:, :n], mB[:, :n], AF.Copy)
                    rBs = p_tmp.tile([P, 512], b16, tag="rBs", name="rBs")
                    act(rBs[:, :n], rB[:, :n], AF.Copy)
                    with nc.allow_low_precision(reason="fp8 GEMM inputs"):
                        for s in range(EC):
                            t0 = p_tmp.tile([P, 512], b16, tag="lnt", name="lnt")
                            nc.vector.tensor_sub(t0[:, :n], x_strips[s][:, sl],
                                                 mBs[:, :n])
                            nc.vector.tensor_mul(h8[:, s, sl], t0[:, :n],
                                                 rBs[:, :n])
                return h8

            # ---------- PSUM rotors ----------
            def rotor(tags, shape, name, tagged=False):
                st = [0]

                def get():
                    t = tags[st[0] % len(tags)]
                    st[0] += 1
                    tl = p_ps.tile(shape, f32, tag=t, name=name)
                    return (tl, t) if tagged else tl
                return get

            sc2_g = rotor(("sA", "sB"), [P, 2, 512], "sc2")   # paired scores
            sc2_c = rotor(("sC",), [P, 2, 512], "sc2c")       # poly-exp pairs
            po_g = rotor(("p0", "p1"), [VD, 512], "po", tagged=True)
            po_b = rotor(("p0", "p1"), [VD, 512], "po", tagged=True)
            gem_r = rotor(("sA", "sB"), [P, 512], "gps")      # non-attn phases
            qkv_r = rotor(("sC",), [P, 512], "qps")           # banded qkv
            mlp_r = rotor(("sC", "p0", "p1"), [P, 512], "mps")
            kv_r = rotor(("p0", "p1"), [P, 512], "kvps")

            # ---------- DoubleRow GEMM (transposed layout) ----------
            def gemm8(w8, col0, n_oc, h8, Tn, post, ps_get=None, bias_i=None,
                      bias_grp=None):
                ps_get = ps_get or gem_r
                step = 512 if Tn % 512 == 0 else NT
                nss = [(k * step, min(step, Tn - k * step))
                       for k in range((Tn + step - 1) // step)]
                for (o, n) in nss:
                    for oc in range(n_oc):
                        ps = ps_get()
                        for sp in range(EC // 2):
                            nc.tensor.matmul(
                                ps[:, :n],
                                w8[:, 2 * sp:2 * sp + 2,
                                   col0 + oc * P:col0 + (oc + 1) * P],
                                h8[:, 2 * sp:2 * sp + 2, o:o + n],
                                start=(sp == 0), stop=(sp == EC // 2 - 1),
                                perf_mode=DR)
                        if use_bias and bias_grp is not None:
                            nc.vector.tensor_scalar(
                                ps[:, :n], ps[:, :n],
                                bslice(bias_i, bias_grp, oc), None, OP.add)
                        post(oc, o, n, ps)

            def psum_to_sbuf8(dst_ap, ps_ap, eng="dve", scale=None):
                """copy matmul accumulator to sbuf (fp8/bf16 out)"""
                with nc.allow_low_precision(reason="fp8 GEMM inputs"):
                    if eng == "act":
                        act(dst_ap, ps_ap, AF.Copy,
                            scale=1.0 if scale is None else scale)
                    elif scale is None:
                        nc.vector.tensor_copy(dst_ap, ps_ap)
                    else:
                        nc.vector.tensor_scalar(dst_ap, ps_ap, scale, None,
                                                OP.mult)

            # ---------- DVE exp via repeated squaring:
            # exp(s*x) ~ (1 + y + y^2/2)^8 with y = s*x/8  (rel err < 1% for
            # |s*x| <= 1.8).  Uses only tensor_scalar / tensor_tensor, which
            # run in 2x/4x DVE modes on bf16 -- offloads softmax exp from the
            # saturated Activation engine.

            def poly_exp(dst_ap, ps_ap):
                y = p_tmp.tile([P, 2, 512], b16, tag="pey", name="pey")
                u = p_tmp.tile([P, 2, 512], b16, tag="peu", name="peu")
                n2 = ps_ap.shape[-1]
                ya = y[:, :, :n2]
                ua = u[:, :, :n2]
                with nc.allow_low_precision(reason="poly exp"):
                    # t = y/sqrt(2) + 1/sqrt(2);  z = t^2 + 0.5 = 1 + y + y^2/2
                    nc.vector.tensor_scalar(ya, ps_ap, SC_EXP / 8 * 0.70710678,
                                            0.70710678, OP.mult, OP.add)
                    nc.vector.tensor_mul(ua, ya, ya)
                    nc.vector.tensor_scalar(ua, ua, 0.5, None, OP.add)
                    nc.vector.tensor_mul(ua, ua, ua)
                    nc.vector.tensor_mul(ua, ua, ua)
                    nc.vector.tensor_mul(dst_ap, ua, ua)

            # ---------- attention finish (shared) ----------
            def attn_finish(po_t, h_, qt0, qn, ot_strips):
                po, po_tag = po_t
                hp, hh = h_ // 2, h_ % 2
                ou = p_tmp.tile([VD, 512], b16, tag="ou", name="ou", bufs=4)
                with nc.allow_low_precision(reason="bf16 softmax normalizer"):
                    nc.vector.tensor_copy(ou[:, :qn], po[:, :qn])  # frees bank
                    linv = p_sm.tile([1, 512], b16, tag="linv", name="linv", bufs=4)
                    nc.vector.reciprocal(linv[:, :qn], ou[HD:HD + 1, :qn])
                lB = p_ps.tile([P, 512], f32, tag=po_tag, name="lB")
                nc.tensor.matmul(lB[:HD, :qn], ones[0:1, :HD], linv[:, :qn],
                                 start=True, stop=True)
                with nc.allow_low_precision(reason="fp8 GEMM inputs"):
                    nc.vector.tensor_mul(
                        ot_strips[hp][HD * hh:HD * (hh + 1), qt0:qt0 + qn],
                        ou[:HD, :qn], lB[:HD, :qn])

            # ==================================================================
            # BLOCK 0: global attention
            # ==================================================================
            wqkv = p_wqkv.tile([P, EC, 3 * E], f8, tag="wqkv", name="wqkv0")
            nc.sync.dma_start(wqkv[:], wq[0][:])
            wout = p_wout.tile([P, EC, E], f8, tag="wout", name="wout0")
            nc.sync.dma_start(wout[:], wo[0][:])

            mark("b0.ln1+q")
            h1 = layernorm(xs, T, "h")
            # Q projection (ext window)
            qt_t = p_qt.tile([P, EC, T], f8, tag="qt", name="qt0")
            gemm8(wqkv, 0, EC, h1, T,
                  lambda oc, o, n, ps: psum_to_sbuf8(qt_t[:, oc, o:o + n],
                                                     ps[:, :n], eng="act"),
                  bias_i=0, bias_grp="qkv")

            mark("b0.kv")
            ot0 = p_ot.tile([P, EC, T], f8, tag="ot", name="ot0")
            ot_strips = [ot0[:, s, :] for s in range(EC)]
            with tc.tile_pool(name="kvfull", bufs=1) as p_kv, \
                 tc.tile_pool(name="xpan", bufs=2 if not use_bias else 1) as p_xp:
                ktf = p_kv.tile([P, EC, S], f8, tag="ktf", name="ktf")
                vf = p_kv.tile([P, NKC, H, VD], f8, tag="vf", name="vf")
                nc.vector.memset(vf[:, :, :, HD:HD + 1], 1.0)
                nc.vector.memset(vf[:, :, :, HD + 1:VD], 0.0)
                PAN = 1024
                for pan in range(S // PAN):
                    xp = p_xp.tile([P, EC, PAN], b16, tag="xp", name="xp")
                    for s in range(EC):
                        nc.sync.dma_start(xp[:, s, :],
                                          xfT[P * s:P * (s + 1), PAN * pan:PAN * (pan + 1)])
                    hp8 = layernorm([xp[:, s, :] for s in range(EC)], PAN,
                                    "hp")
                    # K^T columns for this panel
                    for half in range(PAN // 512):
                        hsl = slice(512 * half, 512 * (half + 1))
                        for oc in range(EC):
                            ps = kv_r()
                            for sp in range(EC // 2):
                                nc.tensor.matmul(
                                    ps[:],
                                    wqkv[:, 2 * sp:2 * sp + 2,
                                         E + oc * P:E + (oc + 1) * P],
                                    hp8[:, 2 * sp:2 * sp + 2, hsl],
                                    start=(sp == 0), stop=(sp == EC // 2 - 1),
                                    perf_mode=DR)
                            if use_bias:
                                nc.vector.tensor_scalar(ps[:], ps[:],
                                                        bslice(0, "qkv", EC + oc),
                                                        None, OP.add)
                            psum_to_sbuf8(
                                ktf[:, oc,
                                    PAN * pan + 512 * half:PAN * pan + 512 * (half + 1)],
                                ps[:], eng="act")
                    # V (normal layout) for this panel
                    for tck in range(PAN // P):
                        ps = kv_r()
                        for sp in range(EC // 2):
                            nc.tensor.matmul(
                                ps[:],
                                hp8[:, 2 * sp:2 * sp + 2,
                                    tck * P:(tck + 1) * P],
                                wqkv[:, 2 * sp:2 * sp + 2, 2 * E:3 * E],
                                start=(sp == 0), stop=(sp == EC // 2 - 1),
                                perf_mode=DR)
                        kc = pan * (PAN // P) + tck
                        if use_bias:
                            add_vbias(0, ps)
                        with nc.allow_low_precision(reason="fp8 V"):
                            act(vf[:, kc, :, 0:HD],
                                ps[:].rearrange("p (h d) -> p h d", h=H),
                                AF.Copy, scale=IWS)
                mark("b0.attn")
                # attention: paired scores -> one exp per 2 k-chunks ->
                # DoubleRow AV accumulate
                for (q0, qn_) in ((0, 512), (512, 512), (1024, 128)):
                    n_poly = POLY_PER if qn_ == 512 else POLY_PER // 2
                    stride = (NKC // 2) // max(n_poly, 1)
                    for h_ in range(H):
                        poly_cs = set(range(0, NKC // 2, stride)[:n_poly])
                        hp, hh = h_ // 2, h_ % 2
                        po_t = po_g()
                        po = po_t[0]

                        def score_pair(sp2, c):
                            for i in (0, 1):
                                kc = 2 * c + i
                                nc.tensor.matmul(
                                    sp2[:, i, :qn_],
                                    ktf[HD * hh:HD * (hh + 1), hp,
                                        kc * P:(kc + 1) * P],
                                    qt_t[HD * hh:HD * (hh + 1), hp,
                                         q0:q0 + qn_],
                                    start=True, stop=True)

                        # poly pairs: scores+DVE-exp emitted early, AV deferred
                        # until a few ACT pairs later so the in-order PE stream
                        # never waits on the DVE chain.
                        n_emitted = [0]
                        pend = []

                        def av(pr8, c, last=False):
                            nc.tensor.matmul(
                                po[:, :qn_],
                                vf[:, 2 * c:2 * c + 2, h_, :],
                                pr8[:, :, :qn_],
                                start=(n_emitted[0] == 0), stop=last,
                                perf_mode=DR)
                            n_emitted[0] += 1

                        def start_poly(c):
                            sp2 = sc2_c()
                            score_pair(sp2, c)
                            pr8 = p_probs.tile([P, 2, 512], f8, tag="pr",
                                               name="pr")
                            poly_exp(pr8[:, :, :qn_], sp2[:, :, :qn_])
                            pend.append((pr8, c))

                        act_cs = [c for c in range(NKC // 2)
                                  if c not in poly_cs]
                        total = NKC // 2
                        held = []   # score tiles awaiting exp+AV (lookahead 1)

                        def flush_act(last=False):
                            sp2h, ch = held.pop(0)
                            pr8 = p_probs.tile([P, 2, 512], f8, tag="pr",
                                               name="pr")
                            act(pr8[:, :, :qn_], sp2h[:, :, :qn_], AF.Exp,
                                scale=SC_EXP)
                            av(pr8, ch, last=last)

                        for j, c in enumerate(act_cs):
                            if j % stride == 0 and poly_cs:
                                # drain previous poly, launch next
                                if pend:
                                    pr8p, cp = pend.pop(0)
                                    av(pr8p, cp)
                                nxt = min(poly_cs)
                                poly_cs.discard(nxt)
                                start_poly(nxt)
                            sp2 = sc2_g()
                            score_pair(sp2, c)
                            held.append((sp2, c))
                            if len(held) > 1:
                                flush_act()
                        while held:
                            flush_act(last=(n_emitted[0] == total - 1
                                            and not pend))
                        while pend:
                            pr8p, cp = pend.pop(0)
                            av(pr8p, cp, last=(n_emitted[0] == total - 1))
                        attn_finish(po_t, h_, q0, qn_, ot_strips)

            # ---- pools for the post-block0 phases (opened after kvfull frees,
            # closed at end of emission so repeat>1 can reopen) ----
            _lstack = contextlib.ExitStack()
            lpool = lambda name, bufs, **kw: _lstack.enter_context(
                tc.tile_pool(name=name, bufs=bufs, **kw))
            p_v = lpool("v", 1)          # V_ext [P, NTC, H, VD] fp8
            p_g = lpool("g", 2)          # gelu out [P, FFC, NT] fp8
            p_wfc1 = lpool("wfc1", 2)
            p_wfc2 = lpool("wfc2", 2)

            # ---------- MLP (ln2 + fc1 + gelu + fc2 + residual) ----------
            def mlp(i, x_strips, final=False):
                h2 = layernorm(x_strips, T, "h")
                ii = 2 * i if W_COMP_MLP else i
                wf1 = p_wfc1.tile([P, EC, FF], f8, tag="wfc1", name=f"wfc1_{i}")
                nc.sync.dma_start(wf1[:], w1[ii][:])
                wf2 = p_wfc2.tile([P, FFC, E], f8, tag="wfc2", name=f"wfc2_{i}")
                nc.sync.dma_start(wf2[:], w2[ii][:])
                if W_COMP_MLP:
                    wf1l = p_wfc1.tile([P, EC, FF], f8, tag="wfc1l",
                                       name=f"wfc1l_{i}", bufs=1)
                    nc.sync.dma_start(wf1l[:], w1[ii + 1][:])
                    wf2l = p_wfc2.tile([P, FFC, E], f8, tag="wfc2l",
                                       name=f"wfc2l_{i}", bufs=1)
                    nc.sync.dma_start(wf2l[:], w2[ii + 1][:])
                xdt = f32 if final else b16
                x_new = [p_x.tile([P, T], xdt, tag=f"x{s}", name=f"xm{i}_{s}")
                         for s in range(EC)]
                for nt in range(NQT):
                    o0 = nt * NT
                    g = p_g.tile([P, FFC, NT], f8, tag="g", name="g")
                    for fcp in range(FFC // 2):
                        ps2 = sc2_g()
                        for j in (0, 1):
                            fc = 2 * fcp + j
                            n_pass = 4 if W_COMP_MLP else 2
                            for k in range(n_pass):
                                wsrc = wf1 if k < 2 else wf1l
                                sp = k % 2
                                nc.tensor.matmul(
                                    ps2[:, j, :NT],
                                    wsrc[:, 2 * sp:2 * sp + 2,
                                         fc * P:(fc + 1) * P],
                                    h2[:, 2 * sp:2 * sp + 2, o0:o0 + NT],
                                    start=(k == 0), stop=(k == n_pass - 1),
                                    perf_mode=DR)
                        with nc.allow_low_precision(reason="fp8 gelu"):
                            if use_bias:
                                for j in (0, 1):
                                    act(g[:, 2 * fcp + j, :], ps2[:, j, :NT],
                                        AF.Gelu_apprx_tanh, scale=IWS,
                                        bias=bslice(i, "fc1", 2 * fcp + j))
                            else:
                                act(g[:, 2 * fcp:2 * fcp + 2, :],
                                    ps2[:, :, :NT],
                                    AF.Gelu_apprx_tanh, scale=IWS)
                    for oc in range(EC):
                        ps = mlp_r()
                        n_pass = FFC if W_COMP_MLP else FFC // 2
                        for k in range(n_pass):
                            wsrc = wf2 if k < FFC // 2 else wf2l
                            fp_ = k % (FFC // 2)
                            nc.tensor.matmul(
                                ps[:, :NT],
                                wsrc[:, 2 * fp_:2 * fp_ + 2,
                                     oc * P:(oc + 1) * P],
                                g[:, 2 * fp_:2 * fp_ + 2, :],
                                start=(k == 0), stop=(k == n_pass - 1),
                                perf_mode=DR)
                        if use_bias:
                            nc.vector.tensor_scalar(ps[:, :NT], ps[:, :NT],
                                                    bslice(i, "fc2", oc),
                                                    None, OP.add)
                        with nc.allow_low_precision(reason="bf16 residual"):
                            nc.vector.scalar_tensor_tensor(
                                x_new[oc][:, o0:o0 + NT], ps[:, :NT], IWS,
                                x_strips[oc][:, o0:o0 + NT], OP.mult, OP.add)
                return x_new

            mark("b0.proj+mlp")
            # block 0 out projection + residual + MLP
            x1 = [p_x.tile([P, T], b16, tag=f"x{s}", name=f"x1_{s}")
                  for s in range(EC)]

            def post_out0(oc, o, n, ps):
                with nc.allow_low_precision(reason="bf16 residual"):
                    nc.vector.scalar_tensor_tensor(
                        x1[oc][:, o:o + n], ps[:, :n], IWS, xs[oc][:, o:o + n],
                        OP.mult, OP.add)
            gemm8(wout, 0, EC, ot0, T, post_out0, bias_i=0, bias_grp="out")
            x1 = mlp(0, x1)

            # ==================================================================
            # BLOCKS 1, 2: banded attention
            # ==================================================================
            mark("banded")
            x_cur = x1
            for i in (1, 2):
                lower = (i == 1)
                mark(f"b{i}.ln1qkv")
                wqkv = p_wqkv.tile([P, EC, 3 * E], f8, tag="wqkv",
                                   name=f"wqkv{i}")
                nc.sync.dma_start(wqkv[:], wq[i][:])
                wout = p_wout.tile([P, EC, E], f8, tag="wout", name=f"wout{i}")
                nc.sync.dma_start(wout[:], wo[i][:])
                h1 = layernorm(x_cur, T, "h")
                qt_t = p_qt.tile([P, EC, T], f8, tag="qt", name=f"qt{i}")
                kt_t = p_kt.tile([P, EC, T], f8, tag="kt", name=f"kt{i}")
                v_t = p_v.tile([P, NTC, H, VD], f8, tag="v", name=f"v{i}")
                nc.vector.memset(v_t[:, :, :, HD:HD + 1], 1.0)
                nc.vector.memset(v_t[:, :, :, HD + 1:VD], 0.0)
                otb = p_ot.tile([P, EC, T], f8, tag="ot", name=f"ot{i}")
                ot_strips = [otb[:, s, :] for s in range(EC)]
                mark(f"b{i}.attn")
                # per query tile: groups of (kc0, n_kc, mask, window) where a
                # 2-group is a DoubleRow pair over the union window.
                # c-window tables (relative k-chunk c -> query window):
                WINS = ([(0, 15), (0, 143), (128, 143), (256, 128)] if lower
                        else [(0, 143), (113, 143), (241, 143), (369, 15)])

                def union(c0, c1):
                    lo = min(WINS[c0][0], WINS[c1][0])
                    hi = max(WINS[c0][0] + WINS[c0][1],
                             WINS[c1][0] + WINS[c1][1])
                    return (lo, hi - lo)

                def groups_for(qt):
                    # valid relative chunks: kc = 3qt + (c-1 | c)
                    cs = [c for c in range(4)
                          if 0 <= 3 * qt + (c - 1 if lower else c) < NTC]
                    kc0 = lambda c: 3 * qt + (c - 1 if lower else c)
                    bnd = "lo" if lower else "up"
                    gs = []
                    if cs == [0, 1, 2, 3]:
                        gs.append((kc0(0), 2, masks[f"m2_{bnd}01"], union(0, 1),
                                   (0, 1)))
                        gs.append((kc0(2), 2, masks[f"m2_{bnd}23"], union(2, 3),
                                   (2, 3)))
                    elif cs == [1, 2, 3]:
                        gs.append((kc0(1), 2, masks["m2_lo12"], union(1, 2),
                                   (1, 2)))
                        gs.append((kc0(3), 1, masks["m2_lo23"], WINS[3],
                                   (3, None)))
                    elif cs == [0, 1, 2]:
                        gs.append((kc0(0), 2, masks["m2_up01"], union(0, 1),
                                   (0, 1)))
                        gs.append((kc0(2), 1, masks["m2_up23"], WINS[2],
                                   (2, None)))
                    else:
                        raise AssertionError(cs)
                    return gs

                def emit_attn_qt(qt):
                    for h_ in range(H):
                        hp, hh = h_ // 2, h_ % 2
                        po_t = po_b()
                        po = po_t[0]
                        covered = []
                        gs = groups_for(qt)
                        sps = []
                        for (kcs, nk, mask_t, (qo, qw), cpair) in gs:
                            sp2 = sc2_g()
                            for ii in range(nk):
                                nc.tensor.matmul(
                                    sp2[:, ii, qo:qo + qw],
                                    kt_t[HD * hh:HD * (hh + 1), hp,
                                         (kcs + ii) * P:(kcs + ii + 1) * P],
                                    qt_t[HD * hh:HD * (hh + 1), hp,
                                         qt * NT + qo:qt * NT + qo + qw],
                                    start=True, stop=True)
                            sps.append(sp2)
                        for gi, (kcs, nk, mask_t, (qo, qw), cpair) in enumerate(gs):
                            sp2 = sps[gi]
                            pr8 = p_probs.tile([P, 2, 512], f8, tag="pr",
                                               name="pr")
                            act(pr8[:, :nk, qo:qo + qw], sp2[:, :nk, qo:qo + qw],
                                AF.Exp, scale=SC_EXP)
                            prm = p_probs.tile([P, 2, 512], f8, tag="prm",
                                               name="prm")
                            with nc.allow_low_precision(reason="fp8 probs"):
                                if nk == 2:
                                    nc.gpsimd.tensor_mul(
                                        prm[:, :2, qo:qo + qw],
                                        pr8[:, :2, qo:qo + qw],
                                        mask_t[:, :, qo:qo + qw])
                                else:
                                    # single: mask half cpair[0]%2 of the pair
                                    mi = cpair[0] % 2
                                    nc.gpsimd.tensor_mul(
                                        prm[:, 0, qo:qo + qw],
                                        pr8[:, 0, qo:qo + qw],
                                        mask_t[:, mi, qo:qo + qw])
                            # AV into po, split into fresh/accum column parts
                            parts = []
                            pos = qo
                            for (clo, chi) in covered + [(qo + qw, qo + qw)]:
                                if pos >= qo + qw:
                                    break
                                if chi <= pos:
                                    continue
                                if clo > pos:
                                    parts.append((pos, min(clo, qo + qw), True))
                                if clo < qo + qw:
                                    lo = max(clo, pos)
                                    hi = min(chi, qo + qw)
                                    if lo < hi:
                                        parts.append((lo, hi, False))
                                pos = max(pos, chi)
                            for (lo, hi, is_new) in parts:
                                if nk == 2:
                                    nc.tensor.matmul(
                                        po[:, lo:hi],
                                        v_t[:, kcs:kcs + 2, h_, :],
                                        prm[:, :2, lo:hi],
                                        start=is_new, stop=False,
                                        perf_mode=DR, skip_group_check=True)
                                else:
                                    nc.tensor.matmul(
                                        po[:, lo:hi],
                                        v_t[:, kcs, h_, :],
                                        prm[:, 0, lo:hi],
                                        start=is_new, stop=False,
                                        skip_group_check=True)
                            covered = sorted(covered + [(qo, qo + qw)])
                            merged = []
                            for (lo, hi) in covered:
                                if merged and lo <= merged[-1][1]:
                                    merged[-1] = (merged[-1][0],
                                                  max(hi, merged[-1][1]))
                                else:
                                    merged.append((lo, hi))
                            covered = merged
                        attn_finish(po_t, h_, qt * NT, NT, ot_strips)

                # emit qkv per token tile, with each attention query tile
                # interleaved as soon as its K/V columns exist (in-order
                # engine streams otherwise serialize attention behind the
                # whole projection)
                for nt in range(NQT):
                    o0 = nt * NT
                    for oc in range(EC):
                        ps = qkv_r()
                        for sp in range(EC // 2):
                            nc.tensor.matmul(
                                ps[:, :NT],
                                wqkv[:, 2 * sp:2 * sp + 2, oc * P:(oc + 1) * P],
                                h1[:, 2 * sp:2 * sp + 2, o0:o0 + NT],
                                start=(sp == 0), stop=(sp == EC // 2 - 1),
                                perf_mode=DR)
                        if use_bias:
                            nc.vector.tensor_scalar(ps[:, :NT], ps[:, :NT],
                                                    bslice(i, "qkv", oc),
                                                    None, OP.add)
                        psum_to_sbuf8(qt_t[:, oc, o0:o0 + NT], ps[:, :NT],
                                      eng="act")
                        ps = qkv_r()
                        for sp in range(EC // 2):
                            nc.tensor.matmul(
                                ps[:, :NT],
                                wqkv[:, 2 * sp:2 * sp + 2,
                                     E + oc * P:E + (oc + 1) * P],
                                h1[:, 2 * sp:2 * sp + 2, o0:o0 + NT],
                                start=(sp == 0), stop=(sp == EC // 2 - 1),
                                perf_mode=DR)
                        if use_bias:
                            nc.vector.tensor_scalar(ps[:, :NT], ps[:, :NT],
                                                    bslice(i, "qkv", EC + oc),
                                                    None, OP.add)
                        psum_to_sbuf8(kt_t[:, oc, o0:o0 + NT], ps[:, :NT],
                                      eng="act")
                    for tck in range(3 * nt, 3 * nt + 3):
                        ps = qkv_r()
                        for sp in range(EC // 2):
                            nc.tensor.matmul(
                                ps[:],
                                h1[:, 2 * sp:2 * sp + 2,
                                   tck * P:(tck + 1) * P],
                                wqkv[:, 2 * sp:2 * sp + 2, 2 * E:3 * E],
                                start=(sp == 0), stop=(sp == EC // 2 - 1),
                                perf_mode=DR)
                        if use_bias:
                            add_vbias(i, ps)
                        with nc.allow_low_precision(reason="fp8 V"):
                            act(v_t[:, tck, :, 0:HD],
                                ps[:].rearrange("p (h d) -> p h d", h=H),
                                AF.Copy, scale=IWS)
                    aq = nt if lower else nt - 1
                    if aq >= 0:
                        emit_attn_qt(aq)
                if not lower:
                    emit_attn_qt(NQT - 1)
                mark(f"b{i}.projmlp")
                x_new = [p_x.tile([P, T], b16, tag=f"x{s}", name=f"xa{i}_{s}")
                         for s in range(EC)]

                def post_out(oc, o, n, ps, x_new=x_new, x_cur=x_cur):
                    with nc.allow_low_precision(reason="bf16 residual"):
                        nc.vector.scalar_tensor_tensor(
                            x_new[oc][:, o:o + n], ps[:, :n], IWS,
                            x_cur[oc][:, o:o + n], OP.mult, OP.add)
                gemm8(wout, 0, EC, otb, T, post_out, bias_i=i, bias_grp="out")
                x_cur = mlp(i, x_new, final=(i == 2))

            mark("out")
            # output
            for s in range(EC):
                nc.sync.dma_start(yT[P * s:P * (s + 1), :], x_cur[s][:])
            _lstack.close()

        for _rep in range(repeat):
            _emit_once()

        # record build-order instruction -> phase map (before scheduling)
        _PHASE_OF.clear()
        names = [ins.name for bb in nc.main_func.blocks for ins in bb.instructions]
        bounds = [n for _, n in _PHASE_MARKS]
        labels = [l for l, _ in _PHASE_MARKS]
        import bisect as _bis
        for idx, nm in enumerate(names):
            j = _bis.bisect_right(bounds, idx) - 1
            _PHASE_OF[nm] = labels[j] if j >= 0 else "pre"

    nc.compile()
    return nc


# ------------------------------------------------------------------
# cached executor (compile once, run many)
# ------------------------------------------------------------------
class _Exec:
    def __init__(self, use_bias: bool):
        import jax
        import concourse.mybir as mybir
        from concourse import bass2jax
        from concourse.bass2jax import install_neuronx_cc_hook, _bass_exec_p
        from jax.sharding import Mesh, PartitionSpec
        from jax.experimental.shard_map import shard_map

        install_neuronx_cc_hook()
        nc = _build_nc(use_bias)
        self.nc = nc

        part_name = (nc.partition_id_tensor.name
                     if nc.partition_id_tensor is not None else None)
        in_names, out_names, out_avals = [], [], []
        self.zero_shapes = []
        for alloc in nc.m.functions[0].allocations:
            if not isinstance(alloc, mybir.MemoryLocationSet):
                continue
            name = alloc.memorylocations[0].name
            if alloc.kind == "ExternalInput":
                if name != part_name:
                    in_names.append(name)
            elif alloc.kind == "ExternalOutput":
                out_names.append(name)
                shape = tuple(alloc.tensor_shape)
                dtype = mybir.dt.np(alloc.dtype)
                out_avals.append(jax.core.ShapedArray(shape, dtype))
                self.zero_shapes.append((shape, dtype))
        n_params = len(in_names)
        all_in = in_names + out_names
        if part_name is not None:
            all_in = all_in + [part_name]
        self.in_names = in_names
        self.out_names = out_names
        n_outs = len(out_names)

        def _body(*args):
            operands = list(args)
            if part_name is not None:
                operands.append(bass2jax.partition_id_tensor())
            outs = _bass_exec_p.bind(
                *operands,
                out_avals=tuple(out_avals),
                in_names=tuple(all_in),
                out_names=tuple(out_names),
                lowering_input_output_aliases=(),
                sim_require_finite=True,
                sim_require_nnan=True,
                nc=nc,
            )
            return tuple(outs)
        self._body = _body

        devices = jax.devices()[:N_CORES]
        mesh = Mesh(np.asarray(devices), ("core",))
        in_specs = (PartitionSpec("core"),) * (n_params + n_outs)
        out_specs = (PartitionSpec("core"),) * n_outs
        donate = tuple(range(n_params, n_params + n_outs))
        self.fn = jax.jit(
            shard_map(_body, mesh=mesh, in_specs=in_specs,
                      out_specs=out_specs, check_rep=False),
            donate_argnums=donate, keep_unused=True)
        self.out_avals = out_avals

    def bench(self, in_maps, iters=10):
        """device-resident-input timing: returns per-iteration seconds"""
        import time
        import jax
        from jax.sharding import Mesh, PartitionSpec, NamedSharding
        if not hasattr(self, "_bench_fn"):
            from jax.experimental.shard_map import shard_map
            devices = jax.devices()[:N_CORES]
            mesh = Mesh(np.asarray(devices), ("core",))
            n_in = len(self.in_names) + len(self.zero_shapes)
            self._bench_fn = jax.jit(
                shard_map(self._body, mesh=mesh,
                          in_specs=(PartitionSpec("core"),) * n_in,
                          out_specs=(PartitionSpec("core"),) * len(self.out_names),
                          check_rep=False),
                keep_unused=True)
            self._bench_sharding = NamedSharding(mesh, PartitionSpec("core"))
        concat_in = [
            np.concatenate([np.asarray(in_maps[c][n]) for c in range(N_CORES)], axis=0)
            for n in self.in_names
        ] + [np.zeros((N_CORES * s[0], *s[1:]), d) for (s, d) in self.zero_shapes]
        import jax
        dev_in = [jax.device_put(a, self._bench_sharding) for a in concat_in]
        out = jax.block_until_ready(self._bench_fn(*dev_in))  # warm/compile
        t0 = time.time()
        for _ in range(iters):
            out = self._bench_fn(*dev_in)
        jax.block_until_ready(out)
        return (time.time() - t0) / iters

    def run(self, in_maps):
        """in_maps: list of 8 dicts name->np.ndarray. returns list of dicts"""
        concat_in = [
            np.concatenate([np.asarray(in_maps[c][n]) for c in range(N_CORES)], axis=0)
            for n in self.in_names
        ]
        concat_zeros = [np.zeros((N_CORES * s[0], *s[1:]), d)
                        for (s, d) in self.zero_shapes]
        outs = self.fn(*concat_in, *concat_zeros)
        import jax
        outs = jax.block_until_ready(outs)
        res = []
        for c in range(N_CORES):
            d = {}
            for idx, n in enumerate(self.out_names):
                shp = self.out_avals[idx].shape
                d[n] = np.asarray(outs[idx]).reshape(N_CORES, *shp)[c]
            res.append(d)
        return res


def _get_exec(use_bias: bool) -> "_Exec":
    key = bool(use_bias)
    if key not in _EXEC_CACHE:
        _EXEC_CACHE[key] = _Exec(key)
    return _EXEC_CACHE[key]


# ------------------------------------------------------------------
# host-side input prep
# ------------------------------------------------------------------
def _band_masks():
    ki = np.arange(P)[:, None]
    qi = np.arange(NT)[None, :]
    m = {}
    for c in range(4):
        # lower band, k-chunk kc = 3t + (c-1):  0 <= (i-k) <= W-1 with
        # i-k = qi - ki + 128*(1-c)
        d = qi - ki + P * (1 - c)
        m[f"lo{c}"] = ((d >= 0) & (d <= W_BAND - 1)).astype(bf16)
        # upper band, k-chunk kc = 3t + c:  0 <= (k-i) <= W-1 with
        # k-i = ki - qi + 128*c
        d = ki - qi + P * c
        m[f"up{c}"] = ((d >= 0) & (d <= W_BAND - 1)).astype(bf16)
    out = {}
    for bnd in ("lo", "up"):
        for (nm, a, b) in (("01", 0, 1), ("23", 2, 3)):
            out[f"m2_{bnd}{nm}"] = np.ascontiguousarray(
                np.stack([m[f"{bnd}{a}"], m[f"{bnd}{b}"]], axis=1))
    out["m2_lo12"] = np.ascontiguousarray(
        np.stack([m["lo1"], m["lo2"]], axis=1))
    return out


def kernel(x, ln1_w, ln1_b, ln2_w, ln2_b, qkv_w, qkv_b, out_w, out_b,
           fc1_w, fc1_b, fc2_w, fc2_b):
    x = np.asarray(x, np.float32)
    ln1_w = np.asarray(ln1_w, np.float32); ln1_b = np.asarray(ln1_b, np.float32)
    ln2_w = np.asarray(ln2_w, np.float32); ln2_b = np.asarray(ln2_b, np.float32)
    qkv_w = np.asarray(qkv_w, np.float32); qkv_b = np.asarray(qkv_b, np.float32)
    out_w = np.asarray(out_w, np.float32); out_b = np.asarray(out_b, np.float32)
    fc1_w = np.asarray(fc1_w, np.float32); fc1_b = np.asarray(fc1_b, np.float32)
    fc2_w = np.asarray(fc2_w, np.float32); fc2_b = np.asarray(fc2_b, np.float32)

    # fold LN affine into following projection weights (exact):
    #   h = z*w + b  =>  h @ Wt.T = z @ (W*w).T + b @ W.T
    wq_f, wo_f, w1_f, w2_f = [], [], [], []
    biases = []
    for i in range(3):
        qw = qkv_w[i] * ln1_w[i][None, :]
        qb = qkv_b[i] + qkv_w[i] @ ln1_b[i]
        f1 = fc1_w[i] * ln2_w[i][None, :]
        f1b = fc1_b[i] + fc1_w[i] @ ln2_b[i]
        wq_f.append(qw); w1_f.append(f1)
        wo_f.append(out_w[i]); w2_f.append(fc2_w[i])
        # packed per-feature bias tile [P, 32] (q, k, out, fc1, fc2) + v row.
        # q/k/out/fc2 biases ride the x64-scaled psum; fc1 bias is consumed
        # by the gelu activation in true units.
        bias_pack = np.zeros((P, 32), np.float32)
        bias_pack[:, 0:8] = WS * qb[:2 * E].reshape(8, P).T
        bias_pack[:, 8:12] = WS * out_b[i].reshape(4, P).T
        bias_pack[:, 12:28] = f1b.reshape(16, P).T
        bias_pack[:, 28:32] = WS * fc2_b[i].reshape(4, P).T
        biases.append((bias_pack, WS * qb[2 * E:].reshape(1, E)))
    use_bias = any(np.abs(b).max() > 0 or np.abs(np.asarray(v, np.float32)).max() > 0
                   for b, v in biases)

    ex = _get_exec(use_bias)

    masks = _band_masks()
    # weight tensors: x64 pre-scale into fp8e4, [chunk, P, out] layout
    weights = {}
    for i in range(3):
        pm = lambda a, C: np.ascontiguousarray(
            a.reshape(C, P, -1).transpose(1, 0, 2).reshape(P, -1))
        weights[f"qkvT{i}"] = pm(WS * wq_f[i].T, EC).astype(f8e4)
        weights[f"outT{i}"] = pm(WS * wo_f[i].T, EC).astype(f8e4)
        w1s = pm(WS * w1_f[i].T, EC)
        w2s = pm(WS * w2_f[i].T, FFC)
        weights[f"fc1T{i}"] = w1s.astype(f8e4)
        weights[f"fc2T{i}"] = w2s.astype(f8e4)
        if W_COMP_MLP:
            weights[f"fc1L{i}"] = (
                w1s - weights[f"fc1T{i}"].astype(np.float32)).astype(f8e4)
            weights[f"fc2L{i}"] = (
                w2s - weights[f"fc2T{i}"].astype(np.float32)).astype(f8e4)
        if use_bias:
            weights[f"bias{i}"] = biases[i][0]
            weights[f"vbias{i}"] = biases[i][1].astype(bf16)

    in_maps = []
    for core in range(N_CORES):
        b, c = divmod(core, CHUNKS)
        e0 = EXT_STARTS[c]
        xT_ext = np.ascontiguousarray(x[b, e0:e0 + T, :].T).astype(bf16)
        xfT = np.ascontiguousarray(x[b].T).astype(bf16)
        m = {"xT": xT_ext, "xfT": xfT, **weights, **masks}
        in_maps.append(m)

    res = ex.run(in_maps)

    out = np.empty((B, S, E), np.float32)
    for core in range(N_CORES):
        b, c = divmod(core, CHUNKS)
        yT_ = res[core]["yT"]               # [E, T]
        off = OWN_OFF[c]
        out[b, OWN * c:OWN * (c + 1), :] = yT_[:, off:off + OWN].T
    return out


# revision 39
# speedup vs baseline: 1.2865x; 1.0150x over previous
"""Trainium2 Bass kernel for nn_AlternateAttention (3-block transformer:
global attention, lower-band attention, upper-band attention, each with MLP).

Sharding: 8 cores = 2 batches x 4 sequence chunks of 1024 tokens, each core
processing an extended window of 1152 tokens (64-token halo each side) so the
banded blocks need no inter-core communication. Block 0 (global attention)
needs full-sequence K/V; each core recomputes them from the replicated raw
input (LN1+KV projection over the full 4096 tokens of its batch).

Layout: activations live TRANSPOSED in SBUF ([feature, token]) so projections
are natural matmuls. All projection weights are cast to fp8e4 (scaled x64 so
N(0,0.02) weights stay in the normal range) and GEMMs run in DoubleRow perf
mode (two 128-deep k-tiles per instruction at 0.5 cycles/row).  The 1/64
de-scale is folded into the psum->sbuf consumer (activation scale or
scalar_tensor_tensor) -- for attention scores both Q and K carry x64 so the
softmax exp scale absorbs the 4096.  Attention probabilities and V are fp8
so AV also runs DoubleRow; exp reads score PAIRS from 2-bank [P,2,512] PSUM
tiles to halve the Activation-engine fixed overhead.
"""
import contextlib
import numpy as np
import ml_dtypes

# ---- problem constants (hardcoded per contract) ----
B, S, E, H, W_BAND, FF = 2, 4096, 512, 8, 16, 2048
HD = E // H                      # 64
N_CORES = 8
CHUNKS = 4                       # sequence chunks per batch
OWN = S // CHUNKS                # 1024
HALO = 64
T = OWN + 2 * HALO               # 1152 = 9*128
P = 128
NT = 384                         # token tile (3 per T)
NQT = T // NT                    # 3
NKC = S // P                     # 32 k-chunks for global attention
NTC = T // P                     # 9 token chunks of ext window
EC = E // P                      # 4 feature strips
FFC = FF // P                    # 16
VD = HD + 2                      # V tile row pitch: 66 so DoubleRow pair stride (H*VD) % 16 == 0

WS = 64.0                        # weight pre-scale into fp8
POLY_PER = 4                     # DVE-exp'd score pairs per (head, q-chunk)
IWS = 1.0 / WS
SC_EXP = 0.125 / (WS * WS)       # exp scale absorbing q/k weight scales

EXT_STARTS = [max(0, min(OWN * c - HALO, S - T)) for c in range(CHUNKS)]
OWN_OFF = [OWN * c - EXT_STARTS[c] for c in range(CHUNKS)]

_EXEC_CACHE = {}
_PHASE_MARKS = []
_PHASE_OF = {}

bf16 = ml_dtypes.bfloat16
f8e4 = ml_dtypes.float8_e4m3


# ------------------------------------------------------------------
# device program
# ------------------------------------------------------------------
def _patch_act_tables():
    # The bacc table-load placement maps exp -> 'exp_and_others' and
    # ln -> 'natural_log', thrashing the ACT table RAM (~1.3us per switch,
    # dozens of switches). Restrict the choice to the two sets that cover
    # everything this kernel uses so exp/ln never evict each other.
    import concourse.hw_specs as hw_specs
    import concourse.bacc as bacc_mod
    import concourse.bass_interp as bass_interp
    if getattr(hw_specs, "_aa_patched", False):
        return
    orig = hw_specs.get_activation_tables
    keep = {"natural_log_exp_and_others", "gelu_apprx_tanh_and_others"}

    def _gat(arch):
        tabs = orig(arch)
        return {k: (v if k in keep else set()) for k, v in tabs.items()}

    hw_specs.get_activation_tables = _gat
    bacc_mod.get_activation_tables = _gat
    bass_interp.get_activation_tables = _gat
    hw_specs._aa_patched = True


def _build_nc(use_bias: bool, repeat: int = 1):
    import concourse.bacc as bacc
    import concourse.mybir as mybir
    import concourse.tile as tile

    _patch_act_tables()

    f32 = mybir.dt.float32
    b16 = mybir.dt.bfloat16
    f8 = mybir.dt.float8e4
    AF = mybir.ActivationFunctionType
    OP = mybir.AluOpType
    DR = mybir.MatmulPerfMode.DoubleRow

    nc = bacc.Bacc("TRN2", target_bir_lowering=False, debug=False,
                   num_devices=N_CORES)
    _PHASE_MARKS.clear()

    def mark(label):
        n = sum(len(b.instructions) for b in nc.main_func.blocks)
        _PHASE_MARKS.append((label, n))

    # ---- dram tensors ----
    xT = nc.dram_tensor("xT", [E, T], b16, kind="ExternalInput")
    xfT = nc.dram_tensor("xfT", [E, S], b16, kind="ExternalInput")
    wq, wo, w1, w2, bias_d = [], [], [], [], []
    for i in range(3):
        wq.append(nc.dram_tensor(f"qkvT{i}", [P, EC * 3 * E], f8, kind="ExternalInput"))
        wo.append(nc.dram_tensor(f"outT{i}", [P, EC * E], f8, kind="ExternalInput"))
        w1.append(nc.dram_tensor(f"fc1T{i}", [P, EC * FF], f8, kind="ExternalInput"))
        w2.append(nc.dram_tensor(f"fc2T{i}", [P, FFC * E], f8, kind="ExternalInput"))
        if W_COMP_MLP:
            w1.append(nc.dram_tensor(f"fc1L{i}", [P, EC * FF], f8, kind="ExternalInput"))
            w2.append(nc.dram_tensor(f"fc2L{i}", [P, FFC * E], f8, kind="ExternalInput"))
        if use_bias:
            # packed per-feature biases for transposed-layout outputs:
            # [q(4xP) | k(4xP) | out(4xP) | fc1(16xP) | fc2(4xP)] -> [P, 32]
            bias_d.append(nc.dram_tensor(f"bias{i}", [P, 32], f32, kind="ExternalInput"))
            bias_d.append(nc.dram_tensor(f"vbias{i}", [1, E], b16, kind="ExternalInput"))
    # paired 0/1 bf16 stripe-mask tiles for banded attention: [P, 2, NT]
    mask_d = {}
    for nm in ("m2_lo01", "m2_lo23", "m2_lo12", "m2_up01", "m2_up23"):
        mask_d[nm] = nc.dram_tensor(nm, [P, 2, NT], b16, kind="ExternalInput")
    yT = nc.dram_tensor("yT", [E, T], f32, kind="ExternalOutput")

    from concourse.tile import add_dep_helper as _adh

    with tile.TileContext(nc) as tc, contextlib.ExitStack() as ctx:
        pool = lambda name, bufs, **kw: ctx.enter_context(
            tc.tile_pool(name=name, bufs=bufs, **kw))

        # Order ACT instructions across table-set boundaries (exp/ln vs gelu)
        # so the activation-table RAM isn't thrashed (~1.3us per reload).
        # Within a set no ordering is imposed.
        _act_sets = {AF.Gelu_apprx_tanh: 1}
        _prev_set = [None]
        _prev_insts = [[]]
        _cur_insts = [[]]

        def act(out, in_, func, **kw):
            inst = nc.scalar.activation(out, in_, func, **kw)
            if func == AF.Copy:
                return inst
            s = _act_sets.get(func, 0)
            if _prev_set[0] is None:
                _prev_set[0] = s
            if s != _prev_set[0]:
                _prev_insts[0] = _cur_insts[0][-64:]
                _cur_insts[0] = []
                _prev_set[0] = s
            for p in _prev_insts[0]:
                _adh(inst.ins, p.ins, sync=True, reason="act-table batch order")
            _cur_insts[0].append(inst)
            return inst

        # ---- pools live for the whole kernel ----
        p_x = pool("x", 2)           # residual strips f32, tags x0..x3
        p_h = pool("h", 2)           # LN output fp8 [P, EC, Tn]
        p_sq = pool("sq", 4)         # per-nt squares bf16
        p_qt = pool("qt", 1)         # QT [P, EC, T] fp8
        p_kt = pool("kt", 1)         # KT (banded) [P, EC, T] fp8
        p_ot = pool("ot", 1)         # attention out strips fp8
        p_wqkv = pool("wqkv", 2)     # qkv weights [P, EC, 3E] fp8
        p_wout = pool("wout", 2)
        p_probs = pool("probs", 8)   # exp outputs fp8 [P, 2, 512]
        p_tmp = pool("tmp", 2)       # temporaries
        p_sm = pool("sm", 2)         # small [1, n] stat vectors
        p_c = pool("const", 1)       # ones, masks, biases
        # PSUM: 8 banks = two 2-bank score slots + four single banks
        p_ps = pool("ps", 1, space="PSUM")
        # sA/sB: [P,2,512] paired attention scores (also misc gemm accum)
        # p0..p3: po accumulators / LN stats / broadcasts / gemm accum

        ones = p_c.tile([P, P], b16)
        nc.vector.memset(ones[:], 1.0)
        eps1 = p_c.tile([1, 1], f32)
        nc.vector.memset(eps1[:], 1e-5)
        masks = {}
        for nm, d in mask_d.items():
            mt = p_c.tile([P, 2, NT], b16, tag=f"mask_{nm}", name=f"mask_{nm}")
            nc.sync.dma_start(mt[:], d[:])
            masks[nm] = mt
        bias_t, vbias_t = [], []
        if use_bias:
            for i in range(3):
                bt = p_c.tile([P, 32], f32, tag=f"bias{i}", name=f"bias{i}")
                nc.sync.dma_start(bt[:], bias_d[2 * i][:])
                bias_t.append(bt)
                vt = p_c.tile([1, E], b16, tag=f"vbias{i}", name=f"vbias{i}")
                nc.sync.dma_start(vt[:], bias_d[2 * i + 1][:])
                vbias_t.append(vt)

        def bslice(i, group, oc):
            base = {"qkv": 0, "out": 8, "fc1": 12, "fc2": 28}[group]
            return bias_t[i][:, base + oc:base + oc + 1]

        def add_vbias(i, ps):
            # V projection output is in normal layout [token, feat]: bias
            # varies along the free dim -> broadcast with a K=1 matmul.
            vb = p_ps.tile([P, 512], f32, tag="p0", name="vb")
            nc.tensor.matmul(vb[:], ones[0:1, :], vbias_t[i][:],
                             start=True, stop=True)
            vbs = p_tmp.tile([P, 512], f32, tag="vbs", name="vbs")
            nc.vector.tensor_copy(vbs[:], vb[:])
            nc.vector.tensor_add(ps[:], ps[:], vbs[:])

        def _emit_once():
            # load residual strips
            xs = []
            for s in range(EC):
                t = p_x.tile([P, T], b16, tag=f"x{s}", name=f"xin{s}")
                nc.sync.dma_start(t[:], xT[P * s:P * (s + 1), :])
                xs.append(t)

            # ---------- layernorm ----------
            def layernorm(x_strips, Tn, htag,
                          stat_tags=("sA",), bc_tags=("sB", "sC"),
                          sq_eng=None):
                """x_strips: 4 bf16 strips [P, Tn] -> fp8 [P,EC,Tn].
                Stats via ones-matmul partition reductions (bf16), squares on
                GPSIMD (SBUF->SBUF), per-token mean/rstd broadcast via K=1
                matmuls, normalize on DVE in 2x bf16 mode, fp8 output."""
                sq_eng = sq_eng or nc.gpsimd
                h8 = p_h.tile([P, EC, Tn], f8, tag=htag, name=htag)
                step = 512 if Tn % 512 == 0 else NT
                nss = [(k * step, min(step, Tn - k * step))
                       for k in range((Tn + step - 1) // step)]
                for si, (o, n) in enumerate(nss):
                    sl = slice(o, o + n)
                    s12 = p_ps.tile([1, 1024], f32, tag=stat_tags[0], name="s12")
                    s1 = s12[:, 0:512]
                    s2 = s12[:, 512:1024]
                    for s in range(EC):
                        rhs_x = x_strips[s][:, sl]
                        nc.tensor.matmul(s1[:, :n], ones[:, 0:1], rhs_x,
                                         start=(s == 0), stop=(s == EC - 1))
                        sq_nt = p_sq.tile([P, 512], b16, tag="sqn", name="sqn")
                        sq_eng.tensor_mul(sq_nt[:, :n], rhs_x, rhs_x)
                        nc.tensor.matmul(s2[:, :n], ones[:, 0:1], sq_nt[:, :n],
                                         start=(s == 0), stop=(s == EC - 1))
                    with nc.allow_low_precision(reason="bf16 LN stats"):
                        m_b = p_sm.tile([1, 512], b16, tag="m_b", name="m_b")
                        nc.vector.tensor_scalar(m_b[:, :n], s1[:, :n], 1.0 / E,
                                                None, OP.mult)
                        stt = p_sm.tile([1, 1024], b16, tag="stt", name="stt")
                        sa, sb = stt[:, 0:n], stt[:, 512:512 + n]
                        nc.vector.tensor_scalar(sa, s2[:, :n], 1.0 / E, None,
                                                OP.mult)
                        nc.vector.tensor_mul(sb, m_b[:, :n], m_b[:, :n])
                        nc.vector.tensor_sub(sa, sa, sb)
                        act(sb, sa, AF.Ln, bias=eps1[:])
                        lnv = sb
                        r_b = p_sm.tile([1, 512], b16, tag="r_b", name="r_b")
                        act(r_b[:, :n], lnv, AF.Exp, scale=-0.5)
                    bc = p_ps.tile([P, 2, 512], f32,
                                   tag=bc_tags[si % len(bc_tags)], name="bc")
                    mB, rB = bc[:, 0, :], bc[:, 1, :]
                    nc.tensor.matmul(mB[:, :n], ones[0:1, :], m_b[:, :n],
                                     start=True, stop=True)
                    nc.tensor.matmul(rB[:, :n], ones[0:1, :], r_b[:, :n],
                                     start=True, stop=True)
                    mBs = p_tmp.tile([P, 512], b16, tag="mBs", name="mBs")
                    act(mBs[:, :n], mB[:, :n], AF.Copy)
                    rBs = p_tmp.tile([P, 512], b16, tag="rBs", name="rBs")
                    act(rBs[:, :n], rB[:, :n], AF.Copy)
                    with nc.allow_low_precision(reason="fp8 GEMM inputs"):
                        for s in range(EC):
                            t0 = p_tmp.tile([P, 512], b16, tag="lnt", name="lnt")
                            nc.vector.tensor_sub(t0[:, :n], x_strips[s][:, sl],
                                                 mBs[:, :n])
                            nc.vector.tensor_mul(h8[:, s, sl], t0[:, :n],
                                                 rBs[:, :n])
                return h8

            # ---------- PSUM rotors ----------
            def rotor(tags, shape, name, tagged=False):
                st = [0]

                def get():
                    t = tags[st[0] % len(tags)]
                    st[0] += 1
                    tl = p_ps.tile(shape, f32, tag=t, name=name)
                    return (tl, t) if tagged else tl
                return get

            sc2_g = rotor(("sA", "sB"), [P, 2, 512], "sc2")   # paired scores
            sc2_c = rotor(("sC",), [P, 2, 512], "sc2c")       # poly-exp pairs
            po_g = rotor(("p0", "p1"), [VD, 512], "po", tagged=True)
            oproj_r = rotor(("p0", "p1"), [P, 512], "ops")
            po_b = rotor(("p0", "p1"), [VD, 512], "po", tagged=True)
            gem_r = rotor(("sA", "sB"), [P, 512], "gps")      # non-attn phases
            qkv_r = rotor(("sC",), [P, 512], "qps")           # banded qkv
            mlp_r = rotor(("sC", "p0", "p1"), [P, 512], "mps")
            kv_r = rotor(("p0", "p1"), [P, 512], "kvps")

            # ---------- DoubleRow GEMM (transposed layout) ----------
            def gemm8(w8, col0, n_oc, h8, Tn, post, ps_get=None, bias_i=None,
                      bias_grp=None):
                ps_get = ps_get or gem_r
                step = 512 if Tn % 512 == 0 else NT
                nss = [(k * step, min(step, Tn - k * step))
                       for k in range((Tn + step - 1) // step)]
                for (o, n) in nss:
                    for oc in range(n_oc):
                        ps = ps_get()
                        for sp in range(EC // 2):
                            nc.tensor.matmul(
                                ps[:, :n],
                                w8[:, 2 * sp:2 * sp + 2,
                                   col0 + oc * P:col0 + (oc + 1) * P],
                                h8[:, 2 * sp:2 * sp + 2, o:o + n],
                                start=(sp == 0), stop=(sp == EC // 2 - 1),
                                perf_mode=DR)
                        if use_bias and bias_grp is not None:
                            nc.vector.tensor_scalar(
                                ps[:, :n], ps[:, :n],
                                bslice(bias_i, bias_grp, oc), None, OP.add)
                        post(oc, o, n, ps)

            def psum_to_sbuf8(dst_ap, ps_ap, eng="dve", scale=None):
                """copy matmul accumulator to sbuf (fp8/bf16 out)"""
                with nc.allow_low_precision(reason="fp8 GEMM inputs"):
                    if eng == "act":
                        act(dst_ap, ps_ap, AF.Copy,
                            scale=1.0 if scale is None else scale)
                    elif scale is None:
                        nc.vector.tensor_copy(dst_ap, ps_ap)
                    else:
                        nc.vector.tensor_scalar(dst_ap, ps_ap, scale, None,
                                                OP.mult)

            # ---------- DVE exp via repeated squaring:
            # exp(s*x) ~ (1 + y + y^2/2)^8 with y = s*x/8  (rel err < 1% for
            # |s*x| <= 1.8).  Uses only tensor_scalar / tensor_tensor, which
            # run in 2x/4x DVE modes on bf16 -- offloads softmax exp from the
            # saturated Activation engine.

            def poly_exp(dst_ap, ps_ap):
                y = p_tmp.tile([P, 2, 512], b16, tag="pey", name="pey")
                u = p_tmp.tile([P, 2, 512], b16, tag="peu", name="peu")
                n2 = ps_ap.shape[-1]
                ya = y[:, :, :n2]
                ua = u[:, :, :n2]
                with nc.allow_low_precision(reason="poly exp"):
                    # t = y/sqrt(2) + 1/sqrt(2);  z = t^2 + 0.5 = 1 + y + y^2/2
                    nc.vector.tensor_scalar(ya, ps_ap, SC_EXP / 8 * 0.70710678,
                                            0.70710678, OP.mult, OP.add)
                    nc.vector.tensor_mul(ua, ya, ya)
                    nc.vector.tensor_scalar(ua, ua, 0.5, None, OP.add)
                    nc.vector.tensor_mul(ua, ua, ua)
                    nc.vector.tensor_mul(ua, ua, ua)
                    nc.vector.tensor_mul(dst_ap, ua, ua)

            # ---------- attention finish (shared) ----------
            def attn_finish(po_t, h_, qt0, qn, ot_strips):
                po, po_tag = po_t
                hp, hh = h_ // 2, h_ % 2
                ou = p_tmp.tile([VD, 512], b16, tag="ou", name="ou", bufs=4)
                with nc.allow_low_precision(reason="bf16 softmax normalizer"):
                    nc.vector.tensor_copy(ou[:, :qn], po[:, :qn])  # frees bank
                    linv = p_sm.tile([1, 512], b16, tag="linv", name="linv", bufs=4)
                    nc.vector.reciprocal(linv[:, :qn], ou[HD:HD + 1, :qn])
                lB = p_ps.tile([P, 512], f32, tag=po_tag, name="lB")
                nc.tensor.matmul(lB[:HD, :qn], ones[0:1, :HD], linv[:, :qn],
                                 start=True, stop=True)
                with nc.allow_low_precision(reason="fp8 GEMM inputs"):
                    nc.vector.tensor_mul(
                        ot_strips[hp][HD * hh:HD * (hh + 1), qt0:qt0 + qn],
                        ou[:HD, :qn], lB[:HD, :qn])

            # ==================================================================
            # BLOCK 0: global attention
            # ==================================================================
            wqkv = p_wqkv.tile([P, EC, 3 * E], f8, tag="wqkv", name="wqkv0")
            nc.sync.dma_start(wqkv[:], wq[0][:])
            wout = p_wout.tile([P, EC, E], f8, tag="wout", name="wout0")
            nc.sync.dma_start(wout[:], wo[0][:])

            mark("b0.ln1+q")
            h1 = layernorm(xs, T, "h")
            # Q projection (ext window)
            qt_t = p_qt.tile([P, EC, T], f8, tag="qt", name="qt0")
            gemm8(wqkv, 0, EC, h1, T,
                  lambda oc, o, n, ps: psum_to_sbuf8(qt_t[:, oc, o:o + n],
                                                     ps[:, :n], eng="act"),
                  bias_i=0, bias_grp="qkv")

            mark("b0.kv")
            ot0 = p_ot.tile([P, EC, T], f8, tag="ot", name="ot0")
            ot_strips = [ot0[:, s, :] for s in range(EC)]
            x1 = [p_x.tile([P, T], b16, tag=f"x{s}", name=f"x1_{s}")
                  for s in range(EC)]

            def oproj_chunk(q0, qn_):
                for oc in range(EC):
                    ps = oproj_r()
                    for sp in range(EC // 2):
                        nc.tensor.matmul(
                            ps[:, :qn_],
                            wout[:, 2 * sp:2 * sp + 2, oc * P:(oc + 1) * P],
                            ot0[:, 2 * sp:2 * sp + 2, q0:q0 + qn_],
                            start=(sp == 0), stop=(sp == EC // 2 - 1),
                            perf_mode=DR)
                    if use_bias:
                        nc.vector.tensor_scalar(ps[:, :qn_], ps[:, :qn_],
                                                bslice(0, "out", oc),
                                                None, OP.add)
                    with nc.allow_low_precision(reason="bf16 residual"):
                        nc.vector.scalar_tensor_tensor(
                            x1[oc][:, q0:q0 + qn_], ps[:, :qn_], IWS,
                            xs[oc][:, q0:q0 + qn_], OP.mult, OP.add)
            with tc.tile_pool(name="kvfull", bufs=1) as p_kv, \
                 tc.tile_pool(name="xpan", bufs=2 if not use_bias else 1) as p_xp:
                ktf = p_kv.tile([P, EC, S], f8, tag="ktf", name="ktf")
                vf = p_kv.tile([P, NKC, H, VD], f8, tag="vf", name="vf")
                nc.vector.memset(vf[:, :, :, HD:HD + 1], 1.0)
                nc.vector.memset(vf[:, :, :, HD + 1:VD], 0.0)
                PAN = 1024
                for pan in range(S // PAN):
                    xp = p_xp.tile([P, EC, PAN], b16, tag="xp", name="xp")
                    for s in range(EC):
                        nc.sync.dma_start(xp[:, s, :],
                                          xfT[P * s:P * (s + 1), PAN * pan:PAN * (pan + 1)])
                    hp8 = layernorm([xp[:, s, :] for s in range(EC)], PAN,
                                    "hp")
                    # K^T columns for this panel
                    for half in range(PAN // 512):
                        hsl = slice(512 * half, 512 * (half + 1))
                        for oc in range(EC):
                            ps = kv_r()
                            for sp in range(EC // 2):
                                nc.tensor.matmul(
                                    ps[:],
                                    wqkv[:, 2 * sp:2 * sp + 2,
                                         E + oc * P:E + (oc + 1) * P],
                                    hp8[:, 2 * sp:2 * sp + 2, hsl],
                                    start=(sp == 0), stop=(sp == EC // 2 - 1),
                                    perf_mode=DR)
                            if use_bias:
                                nc.vector.tensor_scalar(ps[:], ps[:],
                                                        bslice(0, "qkv", EC + oc),
                                                        None, OP.add)
                            psum_to_sbuf8(
                                ktf[:, oc,
                                    PAN * pan + 512 * half:PAN * pan + 512 * (half + 1)],
                                ps[:], eng="act")
                    # V (normal layout) for this panel
                    for tck in range(PAN // P):
                        ps = kv_r()
                        for sp in range(EC // 2):
                            nc.tensor.matmul(
                                ps[:],
                                hp8[:, 2 * sp:2 * sp + 2,
                                    tck * P:(tck + 1) * P],
                                wqkv[:, 2 * sp:2 * sp + 2, 2 * E:3 * E],
                                start=(sp == 0), stop=(sp == EC // 2 - 1),
                                perf_mode=DR)
                        kc = pan * (PAN // P) + tck
                        if use_bias:
                            add_vbias(0, ps)
                        with nc.allow_low_precision(reason="fp8 V"):
                            act(vf[:, kc, :, 0:HD],
                                ps[:].rearrange("p (h d) -> p h d", h=H),
                                AF.Copy, scale=IWS)
                mark("b0.attn")
                # attention: paired scores -> one exp per 2 k-chunks ->
                # DoubleRow AV accumulate
                for (q0, qn_) in ((0, 512), (512, 512), (1024, 128)):
                    n_poly = POLY_PER if qn_ == 512 else POLY_PER // 2
                    stride = (NKC // 2) // max(n_poly, 1)
                    for h_ in range(H):
                        poly_cs = set(range(0, NKC // 2, stride)[:n_poly])
                        hp, hh = h_ // 2, h_ % 2
                        po_t = po_g()
                        po = po_t[0]

                        def score_pair(sp2, c):
                            for i in (0, 1):
                                kc = 2 * c + i
                                nc.tensor.matmul(
                                    sp2[:, i, :qn_],
                                    ktf[HD * hh:HD * (hh + 1), hp,
                                        kc * P:(kc + 1) * P],
                                    qt_t[HD * hh:HD * (hh + 1), hp,
                                         q0:q0 + qn_],
                                    start=True, stop=True)

                        # poly pairs: scores+DVE-exp emitted early, AV deferred
                        # until a few ACT pairs later so the in-order PE stream
                        # never waits on the DVE chain.
                        n_emitted = [0]
                        pend = []

                        def av(pr8, c, last=False):
                            nc.tensor.matmul(
                                po[:, :qn_],
                                vf[:, 2 * c:2 * c + 2, h_, :],
                                pr8[:, :, :qn_],
                                start=(n_emitted[0] == 0), stop=last,
                                perf_mode=DR)
                            n_emitted[0] += 1

                        def start_poly(c):
                            sp2 = sc2_c()
                            score_pair(sp2, c)
                            pr8 = p_probs.tile([P, 2, 512], f8, tag="pr",
                                               name="pr")
                            poly_exp(pr8[:, :, :qn_], sp2[:, :, :qn_])
                            pend.append((pr8, c))

                        act_cs = [c for c in range(NKC // 2)
                                  if c not in poly_cs]
                        total = NKC // 2
                        held = []   # score tiles awaiting exp+AV (lookahead 1)

                        def flush_act(last=False):
                            sp2h, ch = held.pop(0)
                            pr8 = p_probs.tile([P, 2, 512], f8, tag="pr",
                                               name="pr")
                            act(pr8[:, :, :qn_], sp2h[:, :, :qn_], AF.Exp,
                                scale=SC_EXP)
                            av(pr8, ch, last=last)

                        for j, c in enumerate(act_cs):
                            if j % stride == 0 and poly_cs:
                                # drain previous poly, launch next
                                if pend:
                                    pr8p, cp = pend.pop(0)
                                    av(pr8p, cp)
                                nxt = min(poly_cs)
                                poly_cs.discard(nxt)
                                start_poly(nxt)
                            sp2 = sc2_g()
                            score_pair(sp2, c)
                            held.append((sp2, c))
                            if len(held) > 1:
                                flush_act()
                        while held:
                            flush_act(last=(n_emitted[0] == total - 1
                                            and not pend))
                        while pend:
                            pr8p, cp = pend.pop(0)
                            av(pr8p, cp, last=(n_emitted[0] == total - 1))
                        attn_finish(po_t, h_, q0, qn_, ot_strips)
                    oproj_chunk(q0, qn_)

            # ---- pools for the post-block0 phases (opened after kvfull frees,
            # closed at end of emission so repeat>1 can reopen) ----
            _lstack = contextlib.ExitStack()
            lpool = lambda name, bufs, **kw: _lstack.enter_context(
                tc.tile_pool(name=name, bufs=bufs, **kw))
            p_v = lpool("v", 1)          # V_ext [P, NTC, H, VD] fp8
            p_g = lpool("g", 2)          # gelu out [P, FFC, NT] fp8
            p_wfc1 = lpool("wfc1", 2)
            p_wfc2 = lpool("wfc2", 2)

            # ---------- MLP (ln2 + fc1 + gelu + fc2 + residual) ----------
            def mlp(i, x_strips, final=False):
                h2 = layernorm(x_strips, T, "h")
                ii = 2 * i if W_COMP_MLP else i
                wf1 = p_wfc1.tile([P, EC, FF], f8, tag="wfc1", name=f"wfc1_{i}")
                nc.sync.dma_start(wf1[:], w1[ii][:])
                wf2 = p_wfc2.tile([P, FFC, E], f8, tag="wfc2", name=f"wfc2_{i}")
                nc.sync.dma_start(wf2[:], w2[ii][:])
                if W_COMP_MLP:
                    wf1l = p_wfc1.tile([P, EC, FF], f8, tag="wfc1l",
                                       name=f"wfc1l_{i}", bufs=1)
                    nc.sync.dma_start(wf1l[:], w1[ii + 1][:])
                    wf2l = p_wfc2.tile([P, FFC, E], f8, tag="wfc2l",
                                       name=f"wfc2l_{i}", bufs=1)
                    nc.sync.dma_start(wf2l[:], w2[ii + 1][:])
                xdt = f32 if final else b16
                x_new = [p_x.tile([P, T], xdt, tag=f"x{s}", name=f"xm{i}_{s}")
                         for s in range(EC)]
                for nt in range(NQT):
                    o0 = nt * NT
                    g = p_g.tile([P, FFC, NT], f8, tag="g", name="g")
                    for fcp in range(FFC // 2):
                        ps2 = sc2_g()
                        for j in (0, 1):
                            fc = 2 * fcp + j
                            n_pass = 4 if W_COMP_MLP else 2
                            for k in range(n_pass):
                                wsrc = wf1 if k < 2 else wf1l
                                sp = k % 2
                                nc.tensor.matmul(
                                    ps2[:, j, :NT],
                                    wsrc[:, 2 * sp:2 * sp + 2,
                                         fc * P:(fc + 1) * P],
                                    h2[:, 2 * sp:2 * sp + 2, o0:o0 + NT],
                                    start=(k == 0), stop=(k == n_pass - 1),
                                    perf_mode=DR)
                        with nc.allow_low_precision(reason="fp8 gelu"):
                            if use_bias:
                                for j in (0, 1):
                                    act(g[:, 2 * fcp + j, :], ps2[:, j, :NT],
                                        AF.Gelu_apprx_tanh, scale=IWS,
                                        bias=bslice(i, "fc1", 2 * fcp + j))
                            else:
                                act(g[:, 2 * fcp:2 * fcp + 2, :],
                                    ps2[:, :, :NT],
                                    AF.Gelu_apprx_tanh, scale=IWS)
                    for oc in range(EC):
                        ps = mlp_r()
                        n_pass = FFC if W_COMP_MLP else FFC // 2
                        for k in range(n_pass):
                            wsrc = wf2 if k < FFC // 2 else wf2l
                            fp_ = k % (FFC // 2)
                            nc.tensor.matmul(
                                ps[:, :NT],
                                wsrc[:, 2 * fp_:2 * fp_ + 2,
                                     oc * P:(oc + 1) * P],
                                g[:, 2 * fp_:2 * fp_ + 2, :],
                                start=(k == 0), stop=(k == n_pass - 1),
                                perf_mode=DR)
                        if use_bias:
                            nc.vector.tensor_scalar(ps[:, :NT], ps[:, :NT],
                                                    bslice(i, "fc2", oc),
                                                    None, OP.add)
                        with nc.allow_low_precision(reason="bf16 residual"):
                            nc.vector.scalar_tensor_tensor(
                                x_new[oc][:, o0:o0 + NT], ps[:, :NT], IWS,
                                x_strips[oc][:, o0:o0 + NT], OP.mult, OP.add)
                        if final:
                            # ship each finished output chunk immediately so
                            # the tail DMA overlaps the remaining MLP chunks
                            nc.sync.dma_start(
                                yT[P * oc:P * (oc + 1), o0:o0 + NT],
                                x_new[oc][:, o0:o0 + NT])
                return x_new

            mark("b0.proj+mlp")
            # out projection is fused into the attention loop above
            x1 = mlp(0, x1)

            # ==================================================================
            # BLOCKS 1, 2: banded attention
            # ==================================================================
            mark("banded")
            x_cur = x1
            for i in (1, 2):
                lower = (i == 1)
                mark(f"b{i}.ln1qkv")
                wqkv = p_wqkv.tile([P, EC, 3 * E], f8, tag="wqkv",
                                   name=f"wqkv{i}")
                nc.sync.dma_start(wqkv[:], wq[i][:])
                wout = p_wout.tile([P, EC, E], f8, tag="wout", name=f"wout{i}")
                nc.sync.dma_start(wout[:], wo[i][:])
                h1 = layernorm(x_cur, T, "h")
                qt_t = p_qt.tile([P, EC, T], f8, tag="qt", name=f"qt{i}")
                kt_t = p_kt.tile([P, EC, T], f8, tag="kt", name=f"kt{i}")
                v_t = p_v.tile([P, NTC, H, VD], f8, tag="v", name=f"v{i}")
                nc.vector.memset(v_t[:, :, :, HD:HD + 1], 1.0)
                nc.vector.memset(v_t[:, :, :, HD + 1:VD], 0.0)
                otb = p_ot.tile([P, EC, T], f8, tag="ot", name=f"ot{i}")
                ot_strips = [otb[:, s, :] for s in range(EC)]
                x_new = [p_x.tile([P, T], b16, tag=f"x{s}", name=f"xa{i}_{s}")
                         for s in range(EC)]

                def oproj_bchunk(qt, i=i, x_new=x_new, x_cur=x_cur, wout=wout):
                    o0 = qt * NT
                    for oc in range(EC):
                        ps = oproj_r()
                        for sp in range(EC // 2):
                            nc.tensor.matmul(
                                ps[:, :NT],
                                wout[:, 2 * sp:2 * sp + 2,
                                     oc * P:(oc + 1) * P],
                                otb[:, 2 * sp:2 * sp + 2, o0:o0 + NT],
                                start=(sp == 0), stop=(sp == EC // 2 - 1),
                                perf_mode=DR)
                        if use_bias:
                            nc.vector.tensor_scalar(ps[:, :NT], ps[:, :NT],
                                                    bslice(i, "out", oc),
                                                    None, OP.add)
                        with nc.allow_low_precision(reason="bf16 residual"):
                            nc.vector.scalar_tensor_tensor(
                                x_new[oc][:, o0:o0 + NT], ps[:, :NT], IWS,
                                x_cur[oc][:, o0:o0 + NT], OP.mult, OP.add)
                mark(f"b{i}.attn")
                # per query tile: groups of (kc0, n_kc, mask, window) where a
                # 2-group is a DoubleRow pair over the union window.
                # c-window tables (relative k-chunk c -> query window):
                WINS = ([(0, 15), (0, 143), (128, 143), (256, 128)] if lower
                        else [(0, 143), (113, 143), (241, 143), (369, 15)])

                def union(c0, c1):
                    lo = min(WINS[c0][0], WINS[c1][0])
                    hi = max(WINS[c0][0] + WINS[c0][1],
                             WINS[c1][0] + WINS[c1][1])
                    return (lo, hi - lo)

                def groups_for(qt):
                    # valid relative chunks: kc = 3qt + (c-1 | c)
                    cs = [c for c in range(4)
                          if 0 <= 3 * qt + (c - 1 if lower else c) < NTC]
                    kc0 = lambda c: 3 * qt + (c - 1 if lower else c)
                    bnd = "lo" if lower else "up"
                    gs = []
                    if cs == [0, 1, 2, 3]:
                        gs.append((kc0(0), 2, masks[f"m2_{bnd}01"], union(0, 1),
                                   (0, 1)))
                        gs.append((kc0(2), 2, masks[f"m2_{bnd}23"], union(2, 3),
                                   (2, 3)))
                    elif cs == [1, 2, 3]:
                        gs.append((kc0(1), 2, masks["m2_lo12"], union(1, 2),
                                   (1, 2)))
                        gs.append((kc0(3), 1, masks["m2_lo23"], WINS[3],
                                   (3, None)))
                    elif cs == [0, 1, 2]:
                        gs.append((kc0(0), 2, masks["m2_up01"], union(0, 1),
                                   (0, 1)))
                        gs.append((kc0(2), 1, masks["m2_up23"], WINS[2],
                                   (2, None)))
                    else:
                        raise AssertionError(cs)
                    return gs

                def emit_attn_qt(qt):
                    for h_ in range(H):
                        hp, hh = h_ // 2, h_ % 2
                        po_t = po_b()
                        po = po_t[0]
                        covered = []
                        gs = groups_for(qt)
                        sps = []
                        for (kcs, nk, mask_t, (qo, qw), cpair) in gs:
                            sp2 = sc2_g()
                            for ii in range(nk):
                                nc.tensor.matmul(
                                    sp2[:, ii, qo:qo + qw],
                                    kt_t[HD * hh:HD * (hh + 1), hp,
                                         (kcs + ii) * P:(kcs + ii + 1) * P],
                                    qt_t[HD * hh:HD * (hh + 1), hp,
                                         qt * NT + qo:qt * NT + qo + qw],
                                    start=True, stop=True)
                            sps.append(sp2)
                        for gi, (kcs, nk, mask_t, (qo, qw), cpair) in enumerate(gs):
                            sp2 = sps[gi]
                            pr8 = p_probs.tile([P, 2, 512], f8, tag="pr",
                                               name="pr")
                            act(pr8[:, :nk, qo:qo + qw], sp2[:, :nk, qo:qo + qw],
                                AF.Exp, scale=SC_EXP)
                            prm = p_probs.tile([P, 2, 512], f8, tag="prm",
                                               name="prm")
                            with nc.allow_low_precision(reason="fp8 probs"):
                                if nk == 2:
                                    nc.gpsimd.tensor_mul(
                                        prm[:, :2, qo:qo + qw],
                                        pr8[:, :2, qo:qo + qw],
                                        mask_t[:, :, qo:qo + qw])
                                else:
                                    # single: mask half cpair[0]%2 of the pair
                                    mi = cpair[0] % 2
                                    nc.gpsimd.tensor_mul(
                                        prm[:, 0, qo:qo + qw],
                                        pr8[:, 0, qo:qo + qw],
                                        mask_t[:, mi, qo:qo + qw])
                            # AV into po, split into fresh/accum column parts
                            parts = []
                            pos = qo
                            for (clo, chi) in covered + [(qo + qw, qo + qw)]:
                                if pos >= qo + qw:
                                    break
                                if chi <= pos:
                                    continue
                                if clo > pos:
                                    parts.append((pos, min(clo, qo + qw), True))
                                if clo < qo + qw:
                                    lo = max(clo, pos)
                                    hi = min(chi, qo + qw)
                                    if lo < hi:
                                        parts.append((lo, hi, False))
                                pos = max(pos, chi)
                            for (lo, hi, is_new) in parts:
                                if nk == 2:
                                    nc.tensor.matmul(
                                        po[:, lo:hi],
                                        v_t[:, kcs:kcs + 2, h_, :],
                                        prm[:, :2, lo:hi],
                                        start=is_new, stop=False,
                                        perf_mode=DR, skip_group_check=True)
                                else:
                                    nc.tensor.matmul(
                                        po[:, lo:hi],
                                        v_t[:, kcs, h_, :],
                                        prm[:, 0, lo:hi],
                                        start=is_new, stop=False,
                                        skip_group_check=True)
                            covered = sorted(covered + [(qo, qo + qw)])
                            merged = []
                            for (lo, hi) in covered:
                                if merged and lo <= merged[-1][1]:
                                    merged[-1] = (merged[-1][0],
                                                  max(hi, merged[-1][1]))
                                else:
                                    merged.append((lo, hi))
                            covered = merged
                        attn_finish(po_t, h_, qt * NT, NT, ot_strips)

                # emit qkv per token tile, with each attention query tile
                # interleaved as soon as its K/V columns exist (in-order
                # engine streams otherwise serialize attention behind the
                # whole projection)
                for nt in range(NQT):
                    o0 = nt * NT
                    for oc in range(EC):
                        ps = qkv_r()
                        for sp in range(EC // 2):
                            nc.tensor.matmul(
                                ps[:, :NT],
                                wqkv[:, 2 * sp:2 * sp + 2, oc * P:(oc + 1) * P],
                                h1[:, 2 * sp:2 * sp + 2, o0:o0 + NT],
                                start=(sp == 0), stop=(sp == EC // 2 - 1),
                                perf_mode=DR)
                        if use_bias:
                            nc.vector.tensor_scalar(ps[:, :NT], ps[:, :NT],
                                                    bslice(i, "qkv", oc),
                                                    None, OP.add)
                        psum_to_sbuf8(qt_t[:, oc, o0:o0 + NT], ps[:, :NT],
                                      eng="act")
                        ps = qkv_r()
                        for sp in range(EC // 2):
                            nc.tensor.matmul(
                                ps[:, :NT],
                                wqkv[:, 2 * sp:2 * sp + 2,
                                     E + oc * P:E + (oc + 1) * P],
                                h1[:, 2 * sp:2 * sp + 2, o0:o0 + NT],
                                start=(sp == 0), stop=(sp == EC // 2 - 1),
                                perf_mode=DR)
                        if use_bias:
                            nc.vector.tensor_scalar(ps[:, :NT], ps[:, :NT],
                                                    bslice(i, "qkv", EC + oc),
                                                    None, OP.add)
                        psum_to_sbuf8(kt_t[:, oc, o0:o0 + NT], ps[:, :NT],
                                      eng="act")
                    for tck in range(3 * nt, 3 * nt + 3):
                        ps = qkv_r()
                        for sp in range(EC // 2):
                            nc.tensor.matmul(
                                ps[:],
                                h1[:, 2 * sp:2 * sp + 2,
                                   tck * P:(tck + 1) * P],
                                wqkv[:, 2 * sp:2 * sp + 2, 2 * E:3 * E],
                                start=(sp == 0), stop=(sp == EC // 2 - 1),
                                perf_mode=DR)
                        if use_bias:
                            add_vbias(i, ps)
                        with nc.allow_low_precision(reason="fp8 V"):
                            act(v_t[:, tck, :, 0:HD],
                                ps[:].rearrange("p (h d) -> p h d", h=H),
                                AF.Copy, scale=IWS)
                    aq = nt if lower else nt - 1
                    if aq >= 0:
                        emit_attn_qt(aq)
                        oproj_bchunk(aq)
                if not lower:
                    emit_attn_qt(NQT - 1)
                    oproj_bchunk(NQT - 1)
                mark(f"b{i}.projmlp")
                x_cur = mlp(i, x_new, final=(i == 2))

            mark("out")
            _lstack.close()

        for _rep in range(repeat):
            _emit_once()

        # record build-order instruction -> phase map (before scheduling)
        _PHASE_OF.clear()
        names = [ins.name for bb in nc.main_func.blocks for ins in bb.instructions]
        bounds = [n for _, n in _PHASE_MARKS]
        labels = [l for l, _ in _PHASE_MARKS]
        import bisect as _bis
        for idx, nm in enumerate(names):
            j = _bis.bisect_right(bounds, idx) - 1
            _PHASE_OF[nm] = labels[j] if j >= 0 else "pre"

    nc.compile()
    return nc


# ------------------------------------------------------------------
# cached executor (compile once, run many)
# ------------------------------------------------------------------
class _Exec:
    def __init__(self, use_bias: bool):
        import jax
        import concourse.mybir as mybir
        from concourse import bass2jax
        from concourse.bass2jax import install_neuronx_cc_hook, _bass_exec_p
        from jax.sharding import Mesh, PartitionSpec
        from jax.experimental.shard_map import shard_map

        install_neuronx_cc_hook()
        nc = _build_nc(use_bias)
        self.nc = nc

        part_name = (nc.partition_id_tensor.name
                     if nc.partition_id_tensor is not None else None)
        in_names, out_names, out_avals = [], [], []
        self.zero_shapes = []
        for alloc in nc.m.functions[0].allocations:
            if not isinstance(alloc, mybir.MemoryLocationSet):
                continue
            name = alloc.memorylocations[0].name
            if alloc.kind == "ExternalInput":
                if name != part_name:
                    in_names.append(name)
            elif alloc.kind == "ExternalOutput":
                out_names.append(name)
                shape = tuple(alloc.tensor_shape)
                dtype = mybir.dt.np(alloc.dtype)
                out_avals.append(jax.core.ShapedArray(shape, dtype))
                self.zero_shapes.append((shape, dtype))
        n_params = len(in_names)
        all_in = in_names + out_names
        if part_name is not None:
            all_in = all_in + [part_name]
        self.in_names = in_names
        self.out_names = out_names
        n_outs = len(out_names)

        def _body(*args):
            operands = list(args)
            if part_name is not None:
                operands.append(bass2jax.partition_id_tensor())
            outs = _bass_exec_p.bind(
                *operands,
                out_avals=tuple(out_avals),
                in_names=tuple(all_in),
                out_names=tuple(out_names),
                lowering_input_output_aliases=(),
                sim_require_finite=True,
                sim_require_nnan=True,
                nc=nc,
            )
            return tuple(outs)
        self._body = _body

        devices = jax.devices()[:N_CORES]
        mesh = Mesh(np.asarray(devices), ("core",))
        in_specs = (PartitionSpec("core"),) * (n_params + n_outs)
        out_specs = (PartitionSpec("core"),) * n_outs
        donate = tuple(range(n_params, n_params + n_outs))
        self.fn = jax.jit(
            shard_map(_body, mesh=mesh, in_specs=in_specs,
                      out_specs=out_specs, check_rep=False),
            donate_argnums=donate, keep_unused=True)
        self.out_avals = out_avals

    def bench(self, in_maps, iters=10):
        """device-resident-input timing: returns per-iteration seconds"""
        import time
        import jax
        from jax.sharding import Mesh, PartitionSpec, NamedSharding
        if not hasattr(self, "_bench_fn"):
            from jax.experimental.shard_map import shard_map
            devices = jax.devices()[:N_CORES]
            mesh = Mesh(np.asarray(devices), ("core",))
            n_in = len(self.in_names) + len(self.zero_shapes)
            self._bench_fn = jax.jit(
                shard_map(self._body, mesh=mesh,
                          in_specs=(PartitionSpec("core"),) * n_in,
                          out_specs=(PartitionSpec("core"),) * len(self.out_names),
                          check_rep=False),
                keep_unused=True)
            self._bench_sharding = NamedSharding(mesh, PartitionSpec("core"))
        concat_in = [
            np.concatenate([np.asarray(in_maps[c][n]) for c in range(N_CORES)], axis=0)
            for n in self.in_names
        ] + [np.zeros((N_CORES * s[0], *s[1:]), d) for (s, d) in self.zero_shapes]
        import jax
        dev_in = [jax.device_put(a, self._bench_sharding) for a in concat_in]
        out = jax.block_until_ready(self._bench_fn(*dev_in))  # warm/compile
        t0 = time.time()
        for _ in range(iters):
            out = self._bench_fn(*dev_in)
        jax.block_until_ready(out)
        return (time.time() - t0) / iters

    def run(self, in_maps):
        """in_maps: list of 8 dicts name->np.ndarray. returns list of dicts"""
        concat_in = [
            np.concatenate([np.asarray(in_maps[c][n]) for c in range(N_CORES)], axis=0)
            for n in self.in_names
        ]
        concat_zeros = [np.zeros((N_CORES * s[0], *s[1:]), d)
                        for (s, d) in self.zero_shapes]
        outs = self.fn(*concat_in, *concat_zeros)
        import jax
        outs = jax.block_until_ready(outs)
        res = []
        for c in range(N_CORES):
            d = {}
            for idx, n in enumerate(self.out_names):
                shp = self.out_avals[idx].shape
                d[n] = np.asarray(outs[idx]).reshape(N_CORES, *shp)[c]
            res.append(d)
        return res


def _get_exec(use_bias: bool) -> "_Exec":
    key = bool(use_bias)
    if key not in _EXEC_CACHE:
        _EXEC_CACHE[key] = _Exec(key)
    return _EXEC_CACHE[key]


# ------------------------------------------------------------------
# host-side input prep
# ------------------------------------------------------------------
def _band_masks():
    ki = np.arange(P)[:, None]
    qi = np.arange(NT)[None, :]
    m = {}
    for c in range(4):
        # lower band, k-chunk kc = 3t + (c-1):  0 <= (i-k) <= W-1 with
        # i-k = qi - ki + 128*(1-c)
        d = qi - ki + P * (1 - c)
        m[f"lo{c}"] = ((d >= 0) & (d <= W_BAND - 1)).astype(bf16)
        # upper band, k-chunk kc = 3t + c:  0 <= (k-i) <= W-1 with
        # k-i = ki - qi + 128*c
        d = ki - qi + P * c
        m[f"up{c}"] = ((d >= 0) & (d <= W_BAND - 1)).astype(bf16)
    out = {}
    for bnd in ("lo", "up"):
        for (nm, a, b) in (("01", 0, 1), ("23", 2, 3)):
            out[f"m2_{bnd}{nm}"] = np.ascontiguousarray(
                np.stack([m[f"{bnd}{a}"], m[f"{bnd}{b}"]], axis=1))
    out["m2_lo12"] = np.ascontiguousarray(
        np.stack([m["lo1"], m["lo2"]], axis=1))
    return out


def kernel(x, ln1_w, ln1_b, ln2_w, ln2_b, qkv_w, qkv_b, out_w, out_b,
           fc1_w, fc1_b, fc2_w, fc2_b):
    x = np.asarray(x, np.float32)
    ln1_w = np.asarray(ln1_w, np.float32); ln1_b = np.asarray(ln1_b, np.float32)
    ln2_w = np.asarray(ln2_w, np.float32); ln2_b = np.asarray(ln2_b, np.float32)
    qkv_w = np.asarray(qkv_w, np.float32); qkv_b = np.asarray(qkv_b, np.float32)
    out_w = np.asarray(out_w, np.float32); out_b = np.asarray(out_b, np.float32)
    fc1_w = np.asarray(fc1_w, np.float32); fc1_b = np.asarray(fc1_b, np.float32)
    fc2_w = np.asarray(fc2_w, np.float32); fc2_b = np.asarray(fc2_b, np.float32)

    # fold LN affine into following projection weights (exact):
    #   h = z*w + b  =>  h @ Wt.T = z @ (W*w).T + b @ W.T
    wq_f, wo_f, w1_f, w2_f = [], [], [], []
    biases = []
    for i in range(3):
        qw = qkv_w[i] * ln1_w[i][None, :]
        qb = qkv_b[i] + qkv_w[i] @ ln1_b[i]
        f1 = fc1_w[i] * ln2_w[i][None, :]
        f1b = fc1_b[i] + fc1_w[i] @ ln2_b[i]
        wq_f.append(qw); w1_f.append(f1)
        wo_f.append(out_w[i]); w2_f.append(fc2_w[i])
        # packed per-feature bias tile [P, 32] (q, k, out, fc1, fc2) + v row.
        # q/k/out/fc2 biases ride the x64-scaled psum; fc1 bias is consumed
        # by the gelu activation in true units.
        bias_pack = np.zeros((P, 32), np.float32)
        bias_pack[:, 0:8] = WS * qb[:2 * E].reshape(8, P).T
        bias_pack[:, 8:12] = WS * out_b[i].reshape(4, P).T
        bias_pack[:, 12:28] = f1b.reshape(16, P).T
        bias_pack[:, 28:32] = WS * fc2_b[i].reshape(4, P).T
        biases.append((bias_pack, WS * qb[2 * E:].reshape(1, E)))
    use_bias = any(np.abs(b).max() > 0 or np.abs(np.asarray(v, np.float32)).max() > 0
                   for b, v in biases)

    ex = _get_exec(use_bias)

    masks = _band_masks()
    # weight tensors: x64 pre-scale into fp8e4, [chunk, P, out] layout
    weights = {}
    for i in range(3):
        pm = lambda a, C: np.ascontiguousarray(
            a.reshape(C, P, -1).transpose(1, 0, 2).reshape(P, -1))
        weights[f"qkvT{i}"] = pm(WS * wq_f[i].T, EC).astype(f8e4)
        weights[f"outT{i}"] = pm(WS * wo_f[i].T, EC).astype(f8e4)
        w1s = pm(WS * w1_f[i].T, EC)
        w2s = pm(WS * w2_f[i].T, FFC)
        weights[f"fc1T{i}"] = w1s.astype(f8e4)
        weights[f"fc2T{i}"] = w2s.astype(f8e4)
        if W_COMP_MLP:
            weights[f"fc1L{i}"] = (
                w1s - weights[f"fc1T{i}"].astype(np.float32)).astype(f8e4)
            weights[f"fc2L{i}"] = (
                w2s - weights[f"fc2T{i}"].astype(np.float32)).astype(f8e4)
        if use_bias:
            weights[f"bias{i}"] = biases[i][0]
            weights[f"vbias{i}"] = biases[i][1].astype(bf16)

    in_maps = []
    for core in range(N_CORES):
        b, c = divmod(core, CHUNKS)
        e0 = EXT_STARTS[c]
        xT_ext = np.ascontiguousarray(x[b, e0:e0 + T, :].T).astype(bf16)
        xfT = np.ascontiguousarray(x[b].T).astype(bf16)
        m = {"xT": xT_ext, "xfT": xfT, **weights, **masks}
        in_maps.append(m)

    res = ex.run(in_maps)

    out = np.empty((B, S, E), np.float32)
    for core in range(N_CORES):
        b, c = divmod(core, CHUNKS)
        yT_ = res[core]["yT"]               # [E, T]
        off = OWN_OFF[c]
        out[b, OWN * c:OWN * (c + 1), :] = yT_[:, off:off + OWN].T
    return out


# revision 43
# speedup vs baseline: 1.3212x; 1.0270x over previous
"""Trainium2 Bass kernel for nn_AlternateAttention (3-block transformer:
global attention, lower-band attention, upper-band attention, each with MLP).

Sharding: 8 cores = 2 batches x 4 sequence chunks of 1024 tokens, each core
processing an extended window of 1152 tokens (64-token halo each side) so the
banded blocks need no inter-core communication. Block 0 (global attention)
needs full-sequence K/V; each core recomputes them from the replicated raw
input (LN1+KV projection over the full 4096 tokens of its batch).

Layout: activations live TRANSPOSED in SBUF ([feature, token]) so projections
are natural matmuls. All projection weights are cast to fp8e4 (scaled x64 so
N(0,0.02) weights stay in the normal range) and GEMMs run in DoubleRow perf
mode (two 128-deep k-tiles per instruction at 0.5 cycles/row).  The 1/64
de-scale is folded into the psum->sbuf consumer (activation scale or
scalar_tensor_tensor) -- for attention scores both Q and K carry x64 so the
softmax exp scale absorbs the 4096.  Attention probabilities and V are fp8
so AV also runs DoubleRow; exp reads score PAIRS from 2-bank [P,2,512] PSUM
tiles to halve the Activation-engine fixed overhead.
"""
import contextlib
import numpy as np
import ml_dtypes

# ---- problem constants (hardcoded per contract) ----
B, S, E, H, W_BAND, FF = 2, 4096, 512, 8, 16, 2048
HD = E // H                      # 64
N_CORES = 8
CHUNKS = 4                       # sequence chunks per batch
OWN = S // CHUNKS                # 1024
HALO = 64
T = OWN + 2 * HALO               # 1152 = 9*128
P = 128
NT = 384                         # token tile (3 per T)
NQT = T // NT                    # 3
NKC = S // P                     # 32 k-chunks for global attention
NTC = T // P                     # 9 token chunks of ext window
EC = E // P                      # 4 feature strips
FFC = FF // P                    # 16
VD = HD + 2                      # V tile row pitch: 66 so DoubleRow pair stride (H*VD) % 16 == 0

WS = 64.0                        # weight pre-scale into fp8
POLY_PER = 4                     # DVE-exp'd score pairs per (head, q-chunk)
IWS = 1.0 / WS
SC_EXP = 0.125 / (WS * WS)       # exp scale absorbing q/k weight scales

EXT_STARTS = [max(0, min(OWN * c - HALO, S - T)) for c in range(CHUNKS)]
OWN_OFF = [OWN * c - EXT_STARTS[c] for c in range(CHUNKS)]

_EXEC_CACHE = {}
_PHASE_MARKS = []
_PHASE_OF = {}

bf16 = ml_dtypes.bfloat16
f8e4 = ml_dtypes.float8_e4m3


# ------------------------------------------------------------------
# device program
# ------------------------------------------------------------------
def _patch_act_tables():
    # The bacc table-load placement maps exp -> 'exp_and_others' and
    # ln -> 'natural_log', thrashing the ACT table RAM (~1.3us per switch,
    # dozens of switches). Restrict the choice to the two sets that cover
    # everything this kernel uses so exp/ln never evict each other.
    import concourse.hw_specs as hw_specs
    import concourse.bacc as bacc_mod
    import concourse.bass_interp as bass_interp
    if getattr(hw_specs, "_aa_patched", False):
        return
    orig = hw_specs.get_activation_tables
    keep = {"natural_log_exp_and_others", "gelu_apprx_tanh_and_others"}

    def _gat(arch):
        tabs = orig(arch)
        return {k: (v if k in keep else set()) for k, v in tabs.items()}

    hw_specs.get_activation_tables = _gat
    bacc_mod.get_activation_tables = _gat
    bass_interp.get_activation_tables = _gat
    hw_specs._aa_patched = True


def _build_nc(use_bias: bool, repeat: int = 1):
    import concourse.bacc as bacc
    import concourse.mybir as mybir
    import concourse.tile as tile

    _patch_act_tables()

    f32 = mybir.dt.float32
    b16 = mybir.dt.bfloat16
    f8 = mybir.dt.float8e4
    AF = mybir.ActivationFunctionType
    OP = mybir.AluOpType
    DR = mybir.MatmulPerfMode.DoubleRow

    nc = bacc.Bacc("TRN2", target_bir_lowering=False, debug=False,
                   num_devices=N_CORES)
    _PHASE_MARKS.clear()

    def mark(label):
        n = sum(len(b.instructions) for b in nc.main_func.blocks)
        _PHASE_MARKS.append((label, n))

    # ---- dram tensors ----
    xT = nc.dram_tensor("xT", [E, T], b16, kind="ExternalInput")
    xfT = nc.dram_tensor("xfT", [E, S], b16, kind="ExternalInput")
    wq, wo, w1, w2, bias_d = [], [], [], [], []
    for i in range(3):
        wq.append(nc.dram_tensor(f"qkvT{i}", [P, EC * 3 * E], f8, kind="ExternalInput"))
        wo.append(nc.dram_tensor(f"outT{i}", [P, EC * E], f8, kind="ExternalInput"))
        w1.append(nc.dram_tensor(f"fc1T{i}", [P, EC * FF], f8, kind="ExternalInput"))
        w2.append(nc.dram_tensor(f"fc2T{i}", [P, FFC * E], f8, kind="ExternalInput"))
        if W_COMP_MLP:
            w1.append(nc.dram_tensor(f"fc1L{i}", [P, EC * FF], f8, kind="ExternalInput"))
            w2.append(nc.dram_tensor(f"fc2L{i}", [P, FFC * E], f8, kind="ExternalInput"))
        if use_bias:
            # packed per-feature biases for transposed-layout outputs:
            # [q(4xP) | k(4xP) | out(4xP) | fc1(16xP) | fc2(4xP)] -> [P, 32]
            bias_d.append(nc.dram_tensor(f"bias{i}", [P, 32], f32, kind="ExternalInput"))
            bias_d.append(nc.dram_tensor(f"vbias{i}", [1, E], b16, kind="ExternalInput"))
    # paired 0/1 bf16 stripe-mask tiles for banded attention: [P, 2, NT]
    mask_d = {}
    for nm in ("m2_lo01", "m2_lo23", "m2_lo12", "m2_up01", "m2_up23"):
        mask_d[nm] = nc.dram_tensor(nm, [P, 2, NT], b16, kind="ExternalInput")
    yT = nc.dram_tensor("yT", [E, T], f32, kind="ExternalOutput")

    from concourse.tile import add_dep_helper as _adh

    with tile.TileContext(nc) as tc, contextlib.ExitStack() as ctx:
        pool = lambda name, bufs, **kw: ctx.enter_context(
            tc.tile_pool(name=name, bufs=bufs, **kw))

        # Order ACT instructions across table-set boundaries (exp/ln vs gelu)
        # so the activation-table RAM isn't thrashed (~1.3us per reload).
        # Within a set no ordering is imposed.
        _act_sets = {AF.Gelu_apprx_tanh: 1}
        _prev_set = [None]
        _prev_insts = [[]]
        _cur_insts = [[]]

        def act(out, in_, func, **kw):
            inst = nc.scalar.activation(out, in_, func, **kw)
            if func == AF.Copy:
                return inst
            s = _act_sets.get(func, 0)
            if _prev_set[0] is None:
                _prev_set[0] = s
            if s != _prev_set[0]:
                _prev_insts[0] = _cur_insts[0][-64:]
                _cur_insts[0] = []
                _prev_set[0] = s
            for p in _prev_insts[0]:
                _adh(inst.ins, p.ins, sync=True, reason="act-table batch order")
            _cur_insts[0].append(inst)
            return inst

        # ---- pools live for the whole kernel ----
        p_x = pool("x", 2)           # residual strips f32, tags x0..x3
        p_h = pool("h", 2)           # LN output fp8 [P, EC, Tn]
        p_sq = pool("sq", 4)         # per-nt squares bf16
        p_qt = pool("qt", 1)         # QT [P, EC, T] fp8
        p_kt = pool("kt", 1)         # KT (banded) [P, EC, T] fp8
        p_ot = pool("ot", 1)         # attention out strips fp8
        p_wqkv = pool("wqkv", 2)     # qkv weights [P, EC, 3E] fp8
        p_wout = pool("wout", 2)
        p_probs = pool("probs", 8)   # exp outputs fp8 [P, 2, 512]
        p_tmp = pool("tmp", 2)       # temporaries
        p_sm = pool("sm", 2)         # small [1, n] stat vectors
        p_c = pool("const", 1)       # ones, masks, biases
        # PSUM: 8 banks = two 2-bank score slots + four single banks
        p_ps = pool("ps", 1, space="PSUM")
        # sA/sB: [P,2,512] paired attention scores (also misc gemm accum)
        # p0..p3: po accumulators / LN stats / broadcasts / gemm accum

        ones = p_c.tile([P, P], b16)
        nc.vector.memset(ones[:], 1.0)
        eps1 = p_c.tile([1, 1], f32)
        nc.vector.memset(eps1[:], 1e-5)
        masks = {}
        for nm, d in mask_d.items():
            mt = p_c.tile([P, 2, NT], b16, tag=f"mask_{nm}", name=f"mask_{nm}")
            nc.sync.dma_start(mt[:], d[:])
            masks[nm] = mt
        bias_t, vbias_t = [], []
        if use_bias:
            for i in range(3):
                bt = p_c.tile([P, 32], f32, tag=f"bias{i}", name=f"bias{i}")
                nc.sync.dma_start(bt[:], bias_d[2 * i][:])
                bias_t.append(bt)
                vt = p_c.tile([1, E], b16, tag=f"vbias{i}", name=f"vbias{i}")
                nc.sync.dma_start(vt[:], bias_d[2 * i + 1][:])
                vbias_t.append(vt)

        def bslice(i, group, oc):
            base = {"qkv": 0, "out": 8, "fc1": 12, "fc2": 28}[group]
            return bias_t[i][:, base + oc:base + oc + 1]

        def add_vbias(i, ps):
            # V projection output is in normal layout [token, feat]: bias
            # varies along the free dim -> broadcast with a K=1 matmul.
            vb = p_ps.tile([P, 512], f32, tag="p0", name="vb")
            nc.tensor.matmul(vb[:], ones[0:1, :], vbias_t[i][:],
                             start=True, stop=True)
            vbs = p_tmp.tile([P, 512], f32, tag="vbs", name="vbs")
            nc.vector.tensor_copy(vbs[:], vb[:])
            nc.vector.tensor_add(ps[:], ps[:], vbs[:])

        def _emit_once():
            # load residual strips
            xs = []
            for s in range(EC):
                t = p_x.tile([P, T], b16, tag=f"x{s}", name=f"xin{s}")
                nc.sync.dma_start(t[:], xT[P * s:P * (s + 1), :])
                xs.append(t)

            # ---------- layernorm ----------
            def layernorm(x_strips, Tn, htag,
                          stat_tags=("sA",), bc_tags=("sB", "sC"),
                          sq_eng=None):
                """x_strips: 4 bf16 strips [P, Tn] -> fp8 [P,EC,Tn].
                Stats via ones-matmul partition reductions (bf16), squares on
                GPSIMD (SBUF->SBUF), per-token mean/rstd broadcast via K=1
                matmuls, normalize on DVE in 2x bf16 mode, fp8 output."""
                sq_eng = sq_eng or nc.gpsimd
                h8 = p_h.tile([P, EC, Tn], f8, tag=htag, name=htag)
                step = 512 if Tn % 512 == 0 else NT
                nss = [(k * step, min(step, Tn - k * step))
                       for k in range((Tn + step - 1) // step)]
                for si, (o, n) in enumerate(nss):
                    sl = slice(o, o + n)
                    s12 = p_ps.tile([1, 1024], f32, tag=stat_tags[0], name="s12")
                    s1 = s12[:, 0:512]
                    s2 = s12[:, 512:1024]
                    for s in range(EC):
                        rhs_x = x_strips[s][:, sl]
                        nc.tensor.matmul(s1[:, :n], ones[:, 0:1], rhs_x,
                                         start=(s == 0), stop=(s == EC - 1))
                        sq_nt = p_sq.tile([P, 512], b16, tag="sqn", name="sqn")
                        sq_eng.tensor_mul(sq_nt[:, :n], rhs_x, rhs_x)
                        nc.tensor.matmul(s2[:, :n], ones[:, 0:1], sq_nt[:, :n],
                                         start=(s == 0), stop=(s == EC - 1))
                    with nc.allow_low_precision(reason="bf16 LN stats"):
                        m_b = p_sm.tile([1, 512], b16, tag="m_b", name="m_b")
                        nc.vector.tensor_scalar(m_b[:, :n], s1[:, :n], 1.0 / E,
                                                None, OP.mult)
                        stt = p_sm.tile([1, 1024], b16, tag="stt", name="stt")
                        sa, sb = stt[:, 0:n], stt[:, 512:512 + n]
                        nc.vector.tensor_scalar(sa, s2[:, :n], 1.0 / E, None,
                                                OP.mult)
                        nc.vector.tensor_mul(sb, m_b[:, :n], m_b[:, :n])
                        nc.vector.tensor_sub(sa, sa, sb)
                        act(sb, sa, AF.Ln, bias=eps1[:])
                        lnv = sb
                        r_b = p_sm.tile([1, 512], b16, tag="r_b", name="r_b")
                        act(r_b[:, :n], lnv, AF.Exp, scale=-0.5)
                    bc = p_ps.tile([P, 2, 512], f32,
                                   tag=bc_tags[si % len(bc_tags)], name="bc")
                    mB, rB = bc[:, 0, :], bc[:, 1, :]
                    nc.tensor.matmul(mB[:, :n], ones[0:1, :], m_b[:, :n],
                                     start=True, stop=True)
                    nc.tensor.matmul(rB[:, :n], ones[0:1, :], r_b[:, :n],
                                     start=True, stop=True)
                    mBs = p_tmp.tile([P, 512], b16, tag="mBs", name="mBs")
                    act(mBs[:, :n], mB[:, :n], AF.Copy)
                    rBs = p_tmp.tile([P, 512], b16, tag="rBs", name="rBs")
                    act(rBs[:, :n], rB[:, :n], AF.Copy)
                    with nc.allow_low_precision(reason="fp8 GEMM inputs"):
                        for s in range(EC):
                            t0 = p_tmp.tile([P, 512], b16, tag="lnt", name="lnt")
                            nc.vector.tensor_sub(t0[:, :n], x_strips[s][:, sl],
                                                 mBs[:, :n])
                            nc.vector.tensor_mul(h8[:, s, sl], t0[:, :n],
                                                 rBs[:, :n])
                return h8

            # ---------- PSUM rotors ----------
            def rotor(tags, shape, name, tagged=False):
                st = [0]

                def get():
                    t = tags[st[0] % len(tags)]
                    st[0] += 1
                    tl = p_ps.tile(shape, f32, tag=t, name=name)
                    return (tl, t) if tagged else tl
                return get

            sc2_g = rotor(("sA", "sB"), [P, 2, 512], "sc2")   # paired scores
            sc2_c = rotor(("sC",), [P, 2, 512], "sc2c")       # poly-exp pairs
            po_g = rotor(("p0", "p1"), [VD, 512], "po", tagged=True)
            oproj_r = rotor(("p0", "p1"), [P, 512], "ops")
            po_b = rotor(("p0", "p1"), [VD, 512], "po", tagged=True)
            gem_r = rotor(("sA", "sB"), [P, 512], "gps")      # non-attn phases
            qkv_r = rotor(("sC",), [P, 512], "qps")           # banded qkv
            mlp_r = rotor(("sC", "p0", "p1"), [P, 512], "mps")
            kv_r = rotor(("p0", "p1"), [P, 512], "kvps")

            # ---------- DoubleRow GEMM (transposed layout) ----------
            def gemm8(w8, col0, n_oc, h8, Tn, post, ps_get=None, bias_i=None,
                      bias_grp=None):
                ps_get = ps_get or gem_r
                step = 512 if Tn % 512 == 0 else NT
                nss = [(k * step, min(step, Tn - k * step))
                       for k in range((Tn + step - 1) // step)]
                for (o, n) in nss:
                    for oc in range(n_oc):
                        ps = ps_get()
                        for sp in range(EC // 2):
                            nc.tensor.matmul(
                                ps[:, :n],
                                w8[:, 2 * sp:2 * sp + 2,
                                   col0 + oc * P:col0 + (oc + 1) * P],
                                h8[:, 2 * sp:2 * sp + 2, o:o + n],
                                start=(sp == 0), stop=(sp == EC // 2 - 1),
                                perf_mode=DR)
                        if use_bias and bias_grp is not None:
                            nc.vector.tensor_scalar(
                                ps[:, :n], ps[:, :n],
                                bslice(bias_i, bias_grp, oc), None, OP.add)
                        post(oc, o, n, ps)

            def psum_to_sbuf8(dst_ap, ps_ap, eng="dve", scale=None):
                """copy matmul accumulator to sbuf (fp8/bf16 out)"""
                with nc.allow_low_precision(reason="fp8 GEMM inputs"):
                    if eng == "act":
                        act(dst_ap, ps_ap, AF.Copy,
                            scale=1.0 if scale is None else scale)
                    elif scale is None:
                        nc.vector.tensor_copy(dst_ap, ps_ap)
                    else:
                        nc.vector.tensor_scalar(dst_ap, ps_ap, scale, None,
                                                OP.mult)

            # ---------- DVE exp via repeated squaring:
            # exp(s*x) ~ (1 + y + y^2/2)^8 with y = s*x/8  (rel err < 1% for
            # |s*x| <= 1.8).  Uses only tensor_scalar / tensor_tensor, which
            # run in 2x/4x DVE modes on bf16 -- offloads softmax exp from the
            # saturated Activation engine.

            def poly_exp(dst_ap, ps_ap):
                y = p_tmp.tile([P, 2, 512], b16, tag="pey", name="pey")
                u = p_tmp.tile([P, 2, 512], b16, tag="peu", name="peu")
                n2 = ps_ap.shape[-1]
                ya = y[:, :, :n2]
                ua = u[:, :, :n2]
                with nc.allow_low_precision(reason="poly exp"):
                    # t = y/sqrt(2) + 1/sqrt(2);  z = t^2 + 0.5 = 1 + y + y^2/2
                    nc.vector.tensor_scalar(ya, ps_ap, SC_EXP / 8 * 0.70710678,
                                            0.70710678, OP.mult, OP.add)
                    nc.vector.tensor_mul(ua, ya, ya)
                    nc.vector.tensor_scalar(ua, ua, 0.5, None, OP.add)
                    nc.vector.tensor_mul(ua, ua, ua)
                    nc.vector.tensor_mul(ua, ua, ua)
                    nc.vector.tensor_mul(dst_ap, ua, ua)

            # ---------- attention finish (shared) ----------
            def attn_finish(po_t, h_, qt0, qn, ot_strips):
                po, po_tag = po_t
                hp, hh = h_ // 2, h_ % 2
                ou = p_tmp.tile([VD, 512], b16, tag="ou", name="ou", bufs=4)
                with nc.allow_low_precision(reason="bf16 softmax normalizer"):
                    nc.vector.tensor_copy(ou[:, :qn], po[:, :qn])  # frees bank
                    linv = p_sm.tile([1, 512], b16, tag="linv", name="linv", bufs=4)
                    nc.vector.reciprocal(linv[:, :qn], ou[HD:HD + 1, :qn])
                lB = p_ps.tile([P, 512], f32, tag=po_tag, name="lB")
                nc.tensor.matmul(lB[:HD, :qn], ones[0:1, :HD], linv[:, :qn],
                                 start=True, stop=True)
                with nc.allow_low_precision(reason="fp8 GEMM inputs"):
                    nc.vector.tensor_mul(
                        ot_strips[hp][HD * hh:HD * (hh + 1), qt0:qt0 + qn],
                        ou[:HD, :qn], lB[:HD, :qn])

            # ==================================================================
            # BLOCK 0: global attention
            # ==================================================================
            wqkv = p_wqkv.tile([P, EC, 3 * E], f8, tag="wqkv", name="wqkv0")
            nc.sync.dma_start(wqkv[:], wq[0][:])
            wout = p_wout.tile([P, EC, E], f8, tag="wout", name="wout0")
            nc.sync.dma_start(wout[:], wo[0][:])

            mark("b0.ln1+q")
            h1 = layernorm(xs, T, "h")
            # Q projection (ext window)
            qt_t = p_qt.tile([P, EC, T], f8, tag="qt", name="qt0")
            gemm8(wqkv, 0, EC, h1, T,
                  lambda oc, o, n, ps: psum_to_sbuf8(qt_t[:, oc, o:o + n],
                                                     ps[:, :n], eng="act"),
                  bias_i=0, bias_grp="qkv")

            mark("b0.kv")
            ot0 = p_ot.tile([P, EC, T], f8, tag="ot", name="ot0")
            ot_strips = [ot0[:, s, :] for s in range(EC)]
            x1 = [p_x.tile([P, T], b16, tag=f"x{s}", name=f"x1_{s}")
                  for s in range(EC)]

            def oproj_chunk(q0, qn_):
                for oc in range(EC):
                    ps = oproj_r()
                    for sp in range(EC // 2):
                        nc.tensor.matmul(
                            ps[:, :qn_],
                            wout[:, 2 * sp:2 * sp + 2, oc * P:(oc + 1) * P],
                            ot0[:, 2 * sp:2 * sp + 2, q0:q0 + qn_],
                            start=(sp == 0), stop=(sp == EC // 2 - 1),
                            perf_mode=DR)
                    if use_bias:
                        nc.vector.tensor_scalar(ps[:, :qn_], ps[:, :qn_],
                                                bslice(0, "out", oc),
                                                None, OP.add)
                    with nc.allow_low_precision(reason="bf16 residual"):
                        nc.vector.scalar_tensor_tensor(
                            x1[oc][:, q0:q0 + qn_], ps[:, :qn_], IWS,
                            xs[oc][:, q0:q0 + qn_], OP.mult, OP.add)
            with tc.tile_pool(name="kvfull", bufs=1) as p_kv, \
                 tc.tile_pool(name="xpan", bufs=2 if not use_bias else 1) as p_xp:
                ktf = p_kv.tile([P, EC, S], f8, tag="ktf", name="ktf")
                vf = p_kv.tile([P, NKC, H, VD], f8, tag="vf", name="vf")
                nc.vector.memset(vf[:, :, :, HD:HD + 1], 1.0)
                nc.vector.memset(vf[:, :, :, HD + 1:VD], 0.0)
                PAN = 1024
                for pan in range(S // PAN):
                    xp = p_xp.tile([P, EC, PAN], b16, tag="xp", name="xp")
                    for s in range(EC):
                        nc.sync.dma_start(xp[:, s, :],
                                          xfT[P * s:P * (s + 1), PAN * pan:PAN * (pan + 1)])
                    hp8 = layernorm([xp[:, s, :] for s in range(EC)], PAN,
                                    "hp")
                    # K^T columns for this panel
                    for half in range(PAN // 512):
                        hsl = slice(512 * half, 512 * (half + 1))
                        for oc in range(EC):
                            ps = kv_r()
                            for sp in range(EC // 2):
                                nc.tensor.matmul(
                                    ps[:],
                                    wqkv[:, 2 * sp:2 * sp + 2,
                                         E + oc * P:E + (oc + 1) * P],
                                    hp8[:, 2 * sp:2 * sp + 2, hsl],
                                    start=(sp == 0), stop=(sp == EC // 2 - 1),
                                    perf_mode=DR)
                            if use_bias:
                                nc.vector.tensor_scalar(ps[:], ps[:],
                                                        bslice(0, "qkv", EC + oc),
                                                        None, OP.add)
                            psum_to_sbuf8(
                                ktf[:, oc,
                                    PAN * pan + 512 * half:PAN * pan + 512 * (half + 1)],
                                ps[:], eng="act")
                    # V (normal layout) for this panel
                    for tck in range(PAN // P):
                        ps = kv_r()
                        for sp in range(EC // 2):
                            nc.tensor.matmul(
                                ps[:],
                                hp8[:, 2 * sp:2 * sp + 2,
                                    tck * P:(tck + 1) * P],
                                wqkv[:, 2 * sp:2 * sp + 2, 2 * E:3 * E],
                                start=(sp == 0), stop=(sp == EC // 2 - 1),
                                perf_mode=DR)
                        kc = pan * (PAN // P) + tck
                        if use_bias:
                            add_vbias(0, ps)
                        with nc.allow_low_precision(reason="fp8 V"):
                            act(vf[:, kc, :, 0:HD],
                                ps[:].rearrange("p (h d) -> p h d", h=H),
                                AF.Copy, scale=IWS)
                mark("b0.attn")
                # attention: paired scores -> one exp per 2 k-chunks ->
                # DoubleRow AV accumulate
                # tail chunk (qn=128): pack 4 k-chunks per score tile, one
                # exp per quad (free 512), two DoubleRow AVs
                q0t, qnt = 1024, 128
                for h_ in range(H):
                    hp, hh = h_ // 2, h_ % 2
                    po_t = po_g()
                    po = po_t[0]
                    for qd in range(NKC // 4):
                        sp2 = sc2_g()
                        for i in range(4):
                            kc = 4 * qd + i
                            nc.tensor.matmul(
                                sp2[:, i % 2, (i // 2) * qnt:(i // 2 + 1) * qnt],
                                ktf[HD * hh:HD * (hh + 1), hp,
                                    kc * P:(kc + 1) * P],
                                qt_t[HD * hh:HD * (hh + 1), hp,
                                     q0t:q0t + qnt],
                                start=True, stop=True)
                        pr8 = p_probs.tile([P, 2, 512], f8, tag="pr",
                                           name="pr")
                        act(pr8[:, :, :2 * qnt], sp2[:, :, :2 * qnt], AF.Exp,
                            scale=SC_EXP)
                        for j in (0, 1):
                            nc.tensor.matmul(
                                po[:, :qnt],
                                vf[:, 4 * qd + 2 * j:4 * qd + 2 * j + 2,
                                   h_, :],
                                pr8[:, :, j * qnt:(j + 1) * qnt],
                                start=(qd == 0 and j == 0),
                                stop=(qd == NKC // 4 - 1 and j == 1),
                                perf_mode=DR)
                    attn_finish(po_t, h_, q0t, qnt, ot_strips)
                oproj_chunk(q0t, qnt)

                for (q0, qn_) in ((0, 512), (512, 512)):
                    n_poly = POLY_PER if qn_ == 512 else POLY_PER // 2
                    stride = (NKC // 2) // max(n_poly, 1)
                    for h_ in range(H):
                        poly_cs = set(range(0, NKC // 2, stride)[:n_poly])
                        hp, hh = h_ // 2, h_ % 2
                        po_t = po_g()
                        po = po_t[0]

                        def score_pair(sp2, c):
                            for i in (0, 1):
                                kc = 2 * c + i
                                nc.tensor.matmul(
                                    sp2[:, i, :qn_],
                                    ktf[HD * hh:HD * (hh + 1), hp,
                                        kc * P:(kc + 1) * P],
                                    qt_t[HD * hh:HD * (hh + 1), hp,
                                         q0:q0 + qn_],
                                    start=True, stop=True)

                        # poly pairs: scores+DVE-exp emitted early, AV deferred
                        # until a few ACT pairs later so the in-order PE stream
                        # never waits on the DVE chain.
                        n_emitted = [0]
                        pend = []

                        def av(pr8, c, last=False):
                            nc.tensor.matmul(
                                po[:, :qn_],
                                vf[:, 2 * c:2 * c + 2, h_, :],
                                pr8[:, :, :qn_],
                                start=(n_emitted[0] == 0), stop=last,
                                perf_mode=DR)
                            n_emitted[0] += 1

                        def start_poly(c):
                            sp2 = sc2_c()
                            score_pair(sp2, c)
                            pr8 = p_probs.tile([P, 2, 512], f8, tag="pr",
                                               name="pr")
                            poly_exp(pr8[:, :, :qn_], sp2[:, :, :qn_])
                            pend.append((pr8, c))

                        act_cs = [c for c in range(NKC // 2)
                                  if c not in poly_cs]
                        total = NKC // 2
                        held = []   # score tiles awaiting exp+AV (lookahead 1)

                        def flush_act(last=False):
                            sp2h, ch = held.pop(0)
                            pr8 = p_probs.tile([P, 2, 512], f8, tag="pr",
                                               name="pr")
                            act(pr8[:, :, :qn_], sp2h[:, :, :qn_], AF.Exp,
                                scale=SC_EXP)
                            av(pr8, ch, last=last)

                        for j, c in enumerate(act_cs):
                            if j % stride == 0 and poly_cs:
                                # drain previous poly, launch next
                                if pend:
                                    pr8p, cp = pend.pop(0)
                                    av(pr8p, cp)
                                nxt = min(poly_cs)
                                poly_cs.discard(nxt)
                                start_poly(nxt)
                            sp2 = sc2_g()
                            score_pair(sp2, c)
                            held.append((sp2, c))
                            if len(held) > 1:
                                flush_act()
                        while held:
                            flush_act(last=(n_emitted[0] == total - 1
                                            and not pend))
                        while pend:
                            pr8p, cp = pend.pop(0)
                            av(pr8p, cp, last=(n_emitted[0] == total - 1))
                        attn_finish(po_t, h_, q0, qn_, ot_strips)
                    oproj_chunk(q0, qn_)

            # ---- pools for the post-block0 phases (opened after kvfull frees,
            # closed at end of emission so repeat>1 can reopen) ----
            _lstack = contextlib.ExitStack()
            lpool = lambda name, bufs, **kw: _lstack.enter_context(
                tc.tile_pool(name=name, bufs=bufs, **kw))
            p_v = lpool("v", 1)          # V_ext [P, NTC, H, VD] fp8
            p_g = lpool("g", 2)          # gelu out [P, FFC, NT] fp8
            p_wfc1 = lpool("wfc1", 2)
            p_wfc2 = lpool("wfc2", 2)

            # ---------- MLP (ln2 + fc1 + gelu + fc2 + residual) ----------
            def mlp(i, x_strips, final=False):
                h2 = layernorm(x_strips, T, "h")
                ii = 2 * i if W_COMP_MLP else i
                wf1 = p_wfc1.tile([P, EC, FF], f8, tag="wfc1", name=f"wfc1_{i}")
                nc.sync.dma_start(wf1[:], w1[ii][:])
                wf2 = p_wfc2.tile([P, FFC, E], f8, tag="wfc2", name=f"wfc2_{i}")
                nc.sync.dma_start(wf2[:], w2[ii][:])
                if W_COMP_MLP:
                    wf1l = p_wfc1.tile([P, EC, FF], f8, tag="wfc1l",
                                       name=f"wfc1l_{i}", bufs=1)
                    nc.sync.dma_start(wf1l[:], w1[ii + 1][:])
                    wf2l = p_wfc2.tile([P, FFC, E], f8, tag="wfc2l",
                                       name=f"wfc2l_{i}", bufs=1)
                    nc.sync.dma_start(wf2l[:], w2[ii + 1][:])
                xdt = f32 if final else b16
                x_new = [p_x.tile([P, T], xdt, tag=f"x{s}", name=f"xm{i}_{s}")
                         for s in range(EC)]
                for nt in range(NQT):
                    o0 = nt * NT
                    g = p_g.tile([P, FFC, NT], f8, tag="g", name="g")
                    for fcp in range(FFC // 2):
                        ps2 = sc2_g()
                        for j in (0, 1):
                            fc = 2 * fcp + j
                            n_pass = 4 if W_COMP_MLP else 2
                            for k in range(n_pass):
                                wsrc = wf1 if k < 2 else wf1l
                                sp = k % 2
                                nc.tensor.matmul(
                                    ps2[:, j, :NT],
                                    wsrc[:, 2 * sp:2 * sp + 2,
                                         fc * P:(fc + 1) * P],
                                    h2[:, 2 * sp:2 * sp + 2, o0:o0 + NT],
                                    start=(k == 0), stop=(k == n_pass - 1),
                                    perf_mode=DR)
                        with nc.allow_low_precision(reason="fp8 gelu"):
                            if use_bias:
                                for j in (0, 1):
                                    act(g[:, 2 * fcp + j, :], ps2[:, j, :NT],
                                        AF.Gelu_apprx_tanh, scale=IWS,
                                        bias=bslice(i, "fc1", 2 * fcp + j))
                            else:
                                act(g[:, 2 * fcp:2 * fcp + 2, :],
                                    ps2[:, :, :NT],
                                    AF.Gelu_apprx_tanh, scale=IWS)
                    for oc in range(EC):
                        ps = mlp_r()
                        n_pass = FFC if W_COMP_MLP else FFC // 2
                        for k in range(n_pass):
                            wsrc = wf2 if k < FFC // 2 else wf2l
                            fp_ = k % (FFC // 2)
                            nc.tensor.matmul(
                                ps[:, :NT],
                                wsrc[:, 2 * fp_:2 * fp_ + 2,
                                     oc * P:(oc + 1) * P],
                                g[:, 2 * fp_:2 * fp_ + 2, :],
                                start=(k == 0), stop=(k == n_pass - 1),
                                perf_mode=DR)
                        if use_bias:
                            nc.vector.tensor_scalar(ps[:, :NT], ps[:, :NT],
                                                    bslice(i, "fc2", oc),
                                                    None, OP.add)
                        with nc.allow_low_precision(reason="bf16 residual"):
                            nc.vector.scalar_tensor_tensor(
                                x_new[oc][:, o0:o0 + NT], ps[:, :NT], IWS,
                                x_strips[oc][:, o0:o0 + NT], OP.mult, OP.add)
                        if final:
                            # ship each finished output chunk immediately so
                            # the tail DMA overlaps the remaining MLP chunks
                            nc.sync.dma_start(
                                yT[P * oc:P * (oc + 1), o0:o0 + NT],
                                x_new[oc][:, o0:o0 + NT])
                return x_new

            mark("b0.proj+mlp")
            # out projection is fused into the attention loop above
            x1 = mlp(0, x1)

            # ==================================================================
            # BLOCKS 1, 2: banded attention
            # ==================================================================
            mark("banded")
            x_cur = x1
            for i in (1, 2):
                lower = (i == 1)
                mark(f"b{i}.ln1qkv")
                wqkv = p_wqkv.tile([P, EC, 3 * E], f8, tag="wqkv",
                                   name=f"wqkv{i}")
                nc.sync.dma_start(wqkv[:], wq[i][:])
                wout = p_wout.tile([P, EC, E], f8, tag="wout", name=f"wout{i}")
                nc.sync.dma_start(wout[:], wo[i][:])
                h1 = layernorm(x_cur, T, "h")
                qt_t = p_qt.tile([P, EC, T], f8, tag="qt", name=f"qt{i}")
                kt_t = p_kt.tile([P, EC, T], f8, tag="kt", name=f"kt{i}")
                v_t = p_v.tile([P, NTC, H, VD], f8, tag="v", name=f"v{i}")
                nc.vector.memset(v_t[:, :, :, HD:HD + 1], 1.0)
                nc.vector.memset(v_t[:, :, :, HD + 1:VD], 0.0)
                otb = p_ot.tile([P, EC, T], f8, tag="ot", name=f"ot{i}")
                ot_strips = [otb[:, s, :] for s in range(EC)]
                x_new = [p_x.tile([P, T], b16, tag=f"x{s}", name=f"xa{i}_{s}")
                         for s in range(EC)]

                def oproj_bchunk(qt, i=i, x_new=x_new, x_cur=x_cur, wout=wout):
                    o0 = qt * NT
                    for oc in range(EC):
                        ps = oproj_r()
                        for sp in range(EC // 2):
                            nc.tensor.matmul(
                                ps[:, :NT],
                                wout[:, 2 * sp:2 * sp + 2,
                                     oc * P:(oc + 1) * P],
                                otb[:, 2 * sp:2 * sp + 2, o0:o0 + NT],
                                start=(sp == 0), stop=(sp == EC // 2 - 1),
                                perf_mode=DR)
                        if use_bias:
                            nc.vector.tensor_scalar(ps[:, :NT], ps[:, :NT],
                                                    bslice(i, "out", oc),
                                                    None, OP.add)
                        with nc.allow_low_precision(reason="bf16 residual"):
                            nc.vector.scalar_tensor_tensor(
                                x_new[oc][:, o0:o0 + NT], ps[:, :NT], IWS,
                                x_cur[oc][:, o0:o0 + NT], OP.mult, OP.add)
                mark(f"b{i}.attn")
                # per query tile: groups of (kc0, n_kc, mask, window) where a
                # 2-group is a DoubleRow pair over the union window.
                # c-window tables (relative k-chunk c -> query window):
                WINS = ([(0, 15), (0, 143), (128, 143), (256, 128)] if lower
                        else [(0, 143), (113, 143), (241, 143), (369, 15)])

                def union(c0, c1):
                    lo = min(WINS[c0][0], WINS[c1][0])
                    hi = max(WINS[c0][0] + WINS[c0][1],
                             WINS[c1][0] + WINS[c1][1])
                    return (lo, hi - lo)

                def groups_for(qt):
                    # valid relative chunks: kc = 3qt + (c-1 | c)
                    cs = [c for c in range(4)
                          if 0 <= 3 * qt + (c - 1 if lower else c) < NTC]
                    kc0 = lambda c: 3 * qt + (c - 1 if lower else c)
                    bnd = "lo" if lower else "up"
                    gs = []
                    if cs == [0, 1, 2, 3]:
                        gs.append((kc0(0), 2, masks[f"m2_{bnd}01"], union(0, 1),
                                   (0, 1)))
                        gs.append((kc0(2), 2, masks[f"m2_{bnd}23"], union(2, 3),
                                   (2, 3)))
                    elif cs == [1, 2, 3]:
                        gs.append((kc0(1), 2, masks["m2_lo12"], union(1, 2),
                                   (1, 2)))
                        gs.append((kc0(3), 1, masks["m2_lo23"], WINS[3],
                                   (3, None)))
                    elif cs == [0, 1, 2]:
                        gs.append((kc0(0), 2, masks["m2_up01"], union(0, 1),
                                   (0, 1)))
                        gs.append((kc0(2), 1, masks["m2_up23"], WINS[2],
                                   (2, None)))
                    else:
                        raise AssertionError(cs)
                    return gs

                def emit_attn_qt(qt):
                    for h_ in range(H):
                        hp, hh = h_ // 2, h_ % 2
                        po_t = po_b()
                        po = po_t[0]
                        covered = []
                        gs = groups_for(qt)
                        sps = []
                        for (kcs, nk, mask_t, (qo, qw), cpair) in gs:
                            sp2 = sc2_g()
                            for ii in range(nk):
                                nc.tensor.matmul(
                                    sp2[:, ii, qo:qo + qw],
                                    kt_t[HD * hh:HD * (hh + 1), hp,
                                         (kcs + ii) * P:(kcs + ii + 1) * P],
                                    qt_t[HD * hh:HD * (hh + 1), hp,
                                         qt * NT + qo:qt * NT + qo + qw],
                                    start=True, stop=True)
                            sps.append(sp2)
                        for gi, (kcs, nk, mask_t, (qo, qw), cpair) in enumerate(gs):
                            sp2 = sps[gi]
                            pr8 = p_probs.tile([P, 2, 512], f8, tag="pr",
                                               name="pr")
                            act(pr8[:, :nk, qo:qo + qw], sp2[:, :nk, qo:qo + qw],
                                AF.Exp, scale=SC_EXP)
                            prm = p_probs.tile([P, 2, 512], f8, tag="prm",
                                               name="prm")
                            with nc.allow_low_precision(reason="fp8 probs"):
                                if nk == 2:
                                    nc.gpsimd.tensor_mul(
                                        prm[:, :2, qo:qo + qw],
                                        pr8[:, :2, qo:qo + qw],
                                        mask_t[:, :, qo:qo + qw])
                                else:
                                    # single: mask half cpair[0]%2 of the pair
                                    mi = cpair[0] % 2
                                    nc.gpsimd.tensor_mul(
                                        prm[:, 0, qo:qo + qw],
                                        pr8[:, 0, qo:qo + qw],
                                        mask_t[:, mi, qo:qo + qw])
                            # AV into po, split into fresh/accum column parts
                            parts = []
                            pos = qo
                            for (clo, chi) in covered + [(qo + qw, qo + qw)]:
                                if pos >= qo + qw:
                                    break
                                if chi <= pos:
                                    continue
                                if clo > pos:
                                    parts.append((pos, min(clo, qo + qw), True))
                                if clo < qo + qw:
                                    lo = max(clo, pos)
                                    hi = min(chi, qo + qw)
                                    if lo < hi:
                                        parts.append((lo, hi, False))
                                pos = max(pos, chi)
                            for (lo, hi, is_new) in parts:
                                if nk == 2:
                                    nc.tensor.matmul(
                                        po[:, lo:hi],
                                        v_t[:, kcs:kcs + 2, h_, :],
                                        prm[:, :2, lo:hi],
                                        start=is_new, stop=False,
                                        perf_mode=DR, skip_group_check=True)
                                else:
                                    nc.tensor.matmul(
                                        po[:, lo:hi],
                                        v_t[:, kcs, h_, :],
                                        prm[:, 0, lo:hi],
                                        start=is_new, stop=False,
                                        skip_group_check=True)
                            covered = sorted(covered + [(qo, qo + qw)])
                            merged = []
                            for (lo, hi) in covered:
                                if merged and lo <= merged[-1][1]:
                                    merged[-1] = (merged[-1][0],
                                                  max(hi, merged[-1][1]))
                                else:
                                    merged.append((lo, hi))
                            covered = merged
                        attn_finish(po_t, h_, qt * NT, NT, ot_strips)

                # emit qkv per token tile, with each attention query tile
                # interleaved as soon as its K/V columns exist (in-order
                # engine streams otherwise serialize attention behind the
                # whole projection)
                for nt in range(NQT):
                    o0 = nt * NT
                    for oc in range(EC):
                        ps = qkv_r()
                        for sp in range(EC // 2):
                            nc.tensor.matmul(
                                ps[:, :NT],
                                wqkv[:, 2 * sp:2 * sp + 2, oc * P:(oc + 1) * P],
                                h1[:, 2 * sp:2 * sp + 2, o0:o0 + NT],
                                start=(sp == 0), stop=(sp == EC // 2 - 1),
                                perf_mode=DR)
                        if use_bias:
                            nc.vector.tensor_scalar(ps[:, :NT], ps[:, :NT],
                                                    bslice(i, "qkv", oc),
                                                    None, OP.add)
                        psum_to_sbuf8(qt_t[:, oc, o0:o0 + NT], ps[:, :NT],
                                      eng="act")
                        ps = qkv_r()
                        for sp in range(EC // 2):
                            nc.tensor.matmul(
                                ps[:, :NT],
                                wqkv[:, 2 * sp:2 * sp + 2,
                                     E + oc * P:E + (oc + 1) * P],
                                h1[:, 2 * sp:2 * sp + 2, o0:o0 + NT],
                                start=(sp == 0), stop=(sp == EC // 2 - 1),
                                perf_mode=DR)
                        if use_bias:
                            nc.vector.tensor_scalar(ps[:, :NT], ps[:, :NT],
                                                    bslice(i, "qkv", EC + oc),
                                                    None, OP.add)
                        psum_to_sbuf8(kt_t[:, oc, o0:o0 + NT], ps[:, :NT],
                                      eng="act")
                    for tck in range(3 * nt, 3 * nt + 3):
                        ps = qkv_r()
                        for sp in range(EC // 2):
                            nc.tensor.matmul(
                                ps[:],
                                h1[:, 2 * sp:2 * sp + 2,
                                   tck * P:(tck + 1) * P],
                                wqkv[:, 2 * sp:2 * sp + 2, 2 * E:3 * E],
                                start=(sp == 0), stop=(sp == EC // 2 - 1),
                                perf_mode=DR)
                        if use_bias:
                            add_vbias(i, ps)
                        with nc.allow_low_precision(reason="fp8 V"):
                            act(v_t[:, tck, :, 0:HD],
                                ps[:].rearrange("p (h d) -> p h d", h=H),
                                AF.Copy, scale=IWS)
                    aq = nt if lower else nt - 1
                    if aq >= 0:
                        emit_attn_qt(aq)
                        oproj_bchunk(aq)
                if not lower:
                    emit_attn_qt(NQT - 1)
                    oproj_bchunk(NQT - 1)
                mark(f"b{i}.projmlp")
                x_cur = mlp(i, x_new, final=(i == 2))

            mark("out")
            _lstack.close()

        for _rep in range(repeat):
            _emit_once()

        # record build-order instruction -> phase map (before scheduling)
        _PHASE_OF.clear()
        names = [ins.name for bb in nc.main_func.blocks for ins in bb.instructions]
        bounds = [n for _, n in _PHASE_MARKS]
        labels = [l for l, _ in _PHASE_MARKS]
        import bisect as _bis
        for idx, nm in enumerate(names):
            j = _bis.bisect_right(bounds, idx) - 1
            _PHASE_OF[nm] = labels[j] if j >= 0 else "pre"

    nc.compile()
    return nc


# ------------------------------------------------------------------
# cached executor (compile once, run many)
# ------------------------------------------------------------------
class _Exec:
    def __init__(self, use_bias: bool):
        import jax
        import concourse.mybir as mybir
        from concourse import bass2jax
        from concourse.bass2jax import install_neuronx_cc_hook, _bass_exec_p
        from jax.sharding import Mesh, PartitionSpec
        from jax.experimental.shard_map import shard_map

        install_neuronx_cc_hook()
        nc = _build_nc(use_bias)
        self.nc = nc

        part_name = (nc.partition_id_tensor.name
                     if nc.partition_id_tensor is not None else None)
        in_names, out_names, out_avals = [], [], []
        self.zero_shapes = []
        for alloc in nc.m.functions[0].allocations:
            if not isinstance(alloc, mybir.MemoryLocationSet):
                continue
            name = alloc.memorylocations[0].name
            if alloc.kind == "ExternalInput":
                if name != part_name:
                    in_names.append(name)
            elif alloc.kind == "ExternalOutput":
                out_names.append(name)
                shape = tuple(alloc.tensor_shape)
                dtype = mybir.dt.np(alloc.dtype)
                out_avals.append(jax.core.ShapedArray(shape, dtype))
                self.zero_shapes.append((shape, dtype))
        n_params = len(in_names)
        all_in = in_names + out_names
        if part_name is not None:
            all_in = all_in + [part_name]
        self.in_names = in_names
        self.out_names = out_names
        n_outs = len(out_names)

        def _body(*args):
            operands = list(args)
            if part_name is not None:
                operands.append(bass2jax.partition_id_tensor())
            outs = _bass_exec_p.bind(
                *operands,
                out_avals=tuple(out_avals),
                in_names=tuple(all_in),
                out_names=tuple(out_names),
                lowering_input_output_aliases=(),
                sim_require_finite=True,
                sim_require_nnan=True,
                nc=nc,
            )
            return tuple(outs)
        self._body = _body

        devices = jax.devices()[:N_CORES]
        mesh = Mesh(np.asarray(devices), ("core",))
        in_specs = (PartitionSpec("core"),) * (n_params + n_outs)
        out_specs = (PartitionSpec("core"),) * n_outs
        donate = tuple(range(n_params, n_params + n_outs))
        self.fn = jax.jit(
            shard_map(_body, mesh=mesh, in_specs=in_specs,
                      out_specs=out_specs, check_rep=False),
            donate_argnums=donate, keep_unused=True)
        self.out_avals = out_avals

    def bench(self, in_maps, iters=10):
        """device-resident-input timing: returns per-iteration seconds"""
        import time
        import jax
        from jax.sharding import Mesh, PartitionSpec, NamedSharding
        if not hasattr(self, "_bench_fn"):
            from jax.experimental.shard_map import shard_map
            devices = jax.devices()[:N_CORES]
            mesh = Mesh(np.asarray(devices), ("core",))
            n_in = len(self.in_names) + len(self.zero_shapes)
            self._bench_fn = jax.jit(
                shard_map(self._body, mesh=mesh,
                          in_specs=(PartitionSpec("core"),) * n_in,
                          out_specs=(PartitionSpec("core"),) * len(self.out_names),
                          check_rep=False),
                keep_unused=True)
            self._bench_sharding = NamedSharding(mesh, PartitionSpec("core"))
        concat_in = [
            np.concatenate([np.asarray(in_maps[c][n]) for c in range(N_CORES)], axis=0)
            for n in self.in_names
        ] + [np.zeros((N_CORES * s[0], *s[1:]), d) for (s, d) in self.zero_shapes]
        import jax
        dev_in = [jax.device_put(a, self._bench_sharding) for a in concat_in]
        out = jax.block_until_ready(self._bench_fn(*dev_in))  # warm/compile
        t0 = time.time()
        for _ in range(iters):
            out = self._bench_fn(*dev_in)
        jax.block_until_ready(out)
        return (time.time() - t0) / iters

    def run(self, in_maps):
        """in_maps: list of 8 dicts name->np.ndarray. returns list of dicts"""
        concat_in = [
            np.concatenate([np.asarray(in_maps[c][n]) for c in range(N_CORES)], axis=0)
            for n in self.in_names
        ]
        concat_zeros = [np.zeros((N_CORES * s[0], *s[1:]), d)
                        for (s, d) in self.zero_shapes]
        outs = self.fn(*concat_in, *concat_zeros)
        import jax
        outs = jax.block_until_ready(outs)
        res = []
        for c in range(N_CORES):
            d = {}
            for idx, n in enumerate(self.out_names):
                shp = self.out_avals[idx].shape
                d[n] = np.asarray(outs[idx]).reshape(N_CORES, *shp)[c]
            res.append(d)
        return res


def _get_exec(use_bias: bool) -> "_Exec":
    key = bool(use_bias)
    if key not in _EXEC_CACHE:
        _EXEC_CACHE[key] = _Exec(key)
    return _EXEC_CACHE[key]


# ------------------------------------------------------------------
# host-side input prep
# ------------------------------------------------------------------
def _band_masks():
    ki = np.arange(P)[:, None]
    qi = np.arange(NT)[None, :]
    m = {}
    for c in range(4):
        # lower band, k-chunk kc = 3t + (c-1):  0 <= (i-k) <= W-1 with
        # i-k = qi - ki + 128*(1-c)
        d = qi - ki + P * (1 - c)
        m[f"lo{c}"] = ((d >= 0) & (d <= W_BAND - 1)).astype(bf16)
        # upper band, k-chunk kc = 3t + c:  0 <= (k-i) <= W-1 with
        # k-i = ki - qi + 128*c
        d = ki - qi + P * c
        m[f"up{c}"] = ((d >= 0) & (d <= W_BAND - 1)).astype(bf16)
    out = {}
    for bnd in ("lo", "up"):
        for (nm, a, b) in (("01", 0, 1), ("23", 2, 3)):
            out[f"m2_{bnd}{nm}"] = np.ascontiguousarray(
                np.stack([m[f"{bnd}{a}"], m[f"{bnd}{b}"]], axis=1))
    out["m2_lo12"] = np.ascontiguousarray(
        np.stack([m["lo1"], m["lo2"]], axis=1))
    return out


def kernel(x, ln1_w, ln1_b, ln2_w, ln2_b, qkv_w, qkv_b, out_w, out_b,
           fc1_w, fc1_b, fc2_w, fc2_b):
    x = np.asarray(x, np.float32)
    ln1_w = np.asarray(ln1_w, np.float32); ln1_b = np.asarray(ln1_b, np.float32)
    ln2_w = np.asarray(ln2_w, np.float32); ln2_b = np.asarray(ln2_b, np.float32)
    qkv_w = np.asarray(qkv_w, np.float32); qkv_b = np.asarray(qkv_b, np.float32)
    out_w = np.asarray(out_w, np.float32); out_b = np.asarray(out_b, np.float32)
    fc1_w = np.asarray(fc1_w, np.float32); fc1_b = np.asarray(fc1_b, np.float32)
    fc2_w = np.asarray(fc2_w, np.float32); fc2_b = np.asarray(fc2_b, np.float32)

    # fold LN affine into following projection weights (exact):
    #   h = z*w + b  =>  h @ Wt.T = z @ (W*w).T + b @ W.T
    wq_f, wo_f, w1_f, w2_f = [], [], [], []
    biases = []
    for i in range(3):
        qw = qkv_w[i] * ln1_w[i][None, :]
        qb = qkv_b[i] + qkv_w[i] @ ln1_b[i]
        f1 = fc1_w[i] * ln2_w[i][None, :]
        f1b = fc1_b[i] + fc1_w[i] @ ln2_b[i]
        wq_f.append(qw); w1_f.append(f1)
        wo_f.append(out_w[i]); w2_f.append(fc2_w[i])
        # packed per-feature bias tile [P, 32] (q, k, out, fc1, fc2) + v row.
        # q/k/out/fc2 biases ride the x64-scaled psum; fc1 bias is consumed
        # by the gelu activation in true units.
        bias_pack = np.zeros((P, 32), np.float32)
        bias_pack[:, 0:8] = WS * qb[:2 * E].reshape(8, P).T
        bias_pack[:, 8:12] = WS * out_b[i].reshape(4, P).T
        bias_pack[:, 12:28] = f1b.reshape(16, P).T
        bias_pack[:, 28:32] = WS * fc2_b[i].reshape(4, P).T
        biases.append((bias_pack, WS * qb[2 * E:].reshape(1, E)))
    use_bias = any(np.abs(b).max() > 0 or np.abs(np.asarray(v, np.float32)).max() > 0
                   for b, v in biases)

    ex = _get_exec(use_bias)

    masks = _band_masks()
    # weight tensors: x64 pre-scale into fp8e4, [chunk, P, out] layout
    weights = {}
    for i in range(3):
        pm = lambda a, C: np.ascontiguousarray(
            a.reshape(C, P, -1).transpose(1, 0, 2).reshape(P, -1))
        weights[f"qkvT{i}"] = pm(WS * wq_f[i].T, EC).astype(f8e4)
        weights[f"outT{i}"] = pm(WS * wo_f[i].T, EC).astype(f8e4)
        w1s = pm(WS * w1_f[i].T, EC)
        w2s = pm(WS * w2_f[i].T, FFC)
        weights[f"fc1T{i}"] = w1s.astype(f8e4)
        weights[f"fc2T{i}"] = w2s.astype(f8e4)
        if W_COMP_MLP:
            weights[f"fc1L{i}"] = (
                w1s - weights[f"fc1T{i}"].astype(np.float32)).astype(f8e4)
            weights[f"fc2L{i}"] = (
                w2s - weights[f"fc2T{i}"].astype(np.float32)).astype(f8e4)
        if use_bias:
            weights[f"bias{i}"] = biases[i][0]
            weights[f"vbias{i}"] = biases[i][1].astype(bf16)

    in_maps = []
    for core in range(N_CORES):
        b, c = divmod(core, CHUNKS)
        e0 = EXT_STARTS[c]
        xT_ext = np.ascontiguousarray(x[b, e0:e0 + T, :].T).astype(bf16)
        xfT = np.ascontiguousarray(x[b].T).astype(bf16)
        m = {"xT": xT_ext, "xfT": xfT, **weights, **masks}
        in_maps.append(m)

    res = ex.run(in_maps)

    out = np.empty((B, S, E), np.float32)
    for core in range(N_CORES):
        b, c = divmod(core, CHUNKS)
        yT_ = res[core]["yT"]               # [E, T]
        off = OWN_OFF[c]
        out[b, OWN * c:OWN * (c + 1), :] = yT_[:, off:off + OWN].T
    return out
